# revision 34
# baseline (speedup 1.0000x reference)
"""Trainium2 Bass kernel for nn_MemoryRetriever (cross-attention memory retriever).

v2: mask-compacted keys.  The per-key boolean mask keeps ~half of the 31290
memory tokens; the host gathers only surviving keys (plus their RoPE table
columns) and shards them across the 8 cores (PC keys/core, padded to a
multiple of 512; padding is masked via a -1e30 exp bias confined to the last
4 key tiles).  Each core computes K/V projections + RMSNorm + 3D-RoPE for its
shard, full Q (replicated, small), local masked-softmax partials
(un-normalized numerator + denominator), then one AllReduce combines partials
and each core output-projects its own 64-query slice.

Engine balance (vs v1): all elementwise math in bf16 (2x DVE mode); the
per-key RMSNorm scale and 1/sqrt(hd) are folded into the rope cos/sin tables
so Exp runs with scalar scale/bias over 2-tile-wide PSUM pairs; sum-of-squares
runs as near-free ones-stationary PE matmuls; attnV accumulates in PSUM across
all key tiles per 2-head group and is DMA'd straight from PSUM into the
collective buffer; softmax denominators tree-reduce on DVE in bf16 and pack
into one PSUM bank via indicator-column matmuls.
"""

import sys
import numpy as np

sys.path.insert(0, "/opt/trn_rl_repo")

DIM = 1024
HEADS = 8
HD = 128
SQ = 512
SK = 31290
N_CORES = 8
QS = SQ // N_CORES
EPS = 1e-6
NEG = -1.0e30

_cache = {}


def _build(nt, flags, pair=True):
    key = ("nc", nt, flags, pair)
    if key in _cache:
        return _cache[key]

    import concourse.bass as bass
    import concourse.tile as tile
    from concourse import mybir, bacc

    f32 = mybir.dt.float32
    bf16 = mybir.dt.bfloat16
    AF = mybir.ActivationFunctionType
    has_bq, has_bk, has_bv, has_bo = flags

    pc = nt * 128          # keys per core (padded)
    nblk = nt // 4         # 512-key blocks
    npair = nt // 2        # tile pairs (wide exp)

    nc = bacc.Bacc("TRN2", target_bir_lowering=False, debug=False,
                   num_devices=N_CORES)

    def din(name, shape, dt=f32):
        return nc.dram_tensor(name, list(shape), dt, kind="ExternalInput").ap()

    # per-core sharded inputs
    memT = din("memT", [DIM, pc], bf16)      # compacted mem shard, feature-major
    ctk = din("ctk", [HD, pc], bf16)         # K rope cos (pair-major rows)
    stk = din("stk", [HD, pc], bf16)
    mbias = din("mbias", [128, 4])           # pad bias (0/-1e30) for last 4 tiles
    # shared inputs
    xT = din("xT", [DIM, SQ], bf16)
    wq = din("wq", [128, 8, 8, 128], bf16)   # [p,i,o,m] = Wq.T[i*128+p, o*128+m]
    wk = din("wk", [128, 8, 8, 128], bf16)
    wo = din("wo", [128, 8, 8, 128], bf16)
    wv = din("wv", [128, 8, DIM], bf16)      # [p,i,o] = Wv.T[i*128+p, o]
    ctq = din("ctq", [128, 8, SQ], bf16)     # q rope cos (gq*gk folded, per head)
    stq = din("stq", [128, 8, SQ], bf16)
    bq_t = din("bq_t", [128, 8])
    bk_t = din("bk_t", [128, 8])
    bo_t = din("bo_t", [128, 8])
    bv_t = din("bv_t", [128, DIM])
    pmat = din("pmat", [128, 128], bf16)     # P.T for rope pair swap (+-1)
    ones_c = din("ones_c", [128, 1], bf16)
    ones_r = din("ones_r", [1, 128])
    ecols = din("ecols", [128, 8, 8], bf16)  # ecols[p,h,m] = (m==h)

    outT = nc.dram_tensor("outT", [DIM, SQ], f32, kind="ExternalOutput").ap()

    import os as _os
    _sim = _os.environ.get("KSIM", "0") == "1"
    cat = nc.dram_tensor("cat", [DIM + HEADS, SQ], bf16)
    cat_sh = nc.dram_tensor("cat_sh", [DIM + HEADS, SQ], bf16,
                            addr_space="Shared")

    with tile.TileContext(nc) as tc:
        with tc.tile_pool(name="consts", bufs=1) as consts, \
             tc.tile_pool(name="resid", bufs=1) as resid:

            # ---- small constants (cheap DMAs, SP queue) ----
            pt_s = consts.tile([128, 128], bf16)
            nc.sync.dma_start(pt_s[:], pmat)
            ones_s = consts.tile([128, 1], bf16)
            nc.sync.dma_start(ones_s[:], ones_c)
            ones_rs = consts.tile([1, 128], f32)
            nc.sync.dma_start(ones_rs[:], ones_r)
            mb_s = consts.tile([128, 4], f32)
            nc.sync.dma_start(mb_s[:], mbias)
            ec_s = consts.tile([128, 8, 8], bf16)
            nc.sync.dma_start(ec_s[:], ecols)
            zcol = consts.tile([128, 1], f32)
            nc.vector.memset(zcol[:], 0.0)
            ep128 = consts.tile([1, 1], f32)
            nc.vector.memset(ep128[:], 128.0 * EPS)
            epsk = consts.tile([128, 1], f32)
            nc.vector.memset(epsk[:], 128.0 * EPS)
            epsq = consts.tile([1, 1], f32)
            nc.vector.memset(epsq[:], EPS)
            if has_bq:
                bq_s = consts.tile([128, 8], f32)
                nc.sync.dma_start(bq_s[:], bq_t)
            if has_bk:
                bk_s = consts.tile([128, 8], f32)
                nc.sync.dma_start(bk_s[:], bk_t)
            if has_bo:
                bo_s = consts.tile([128, 8], f32)
                nc.sync.dma_start(bo_s[:], bo_t)
            if has_bv:
                bv_s = consts.tile([128, DIM], f32)
                nc.sync.dma_start(bv_s[:], bv_t)

            # ---- resident tensors; DMAs issued in first-use order ----
            wk_s = resid.tile([128, 8, 8, 128], bf16)
            memt_blks = [resid.tile([128, 8, 512], bf16, tag=f"memt{b}",
                                    name=f"memt{b}")
                         for b in range(2)]
            wqo_s = resid.tile([128, 8, 8, 128], bf16)  # wq then wo
            xt_s = resid.tile([128, 8, SQ], bf16)
            wv_s = resid.tile([128, 8, DIM], bf16)
            kr = resid.tile([128, 8, pc], bf16)      # rope'd K (unnormalized)
            qT = resid.tile([128, 8, SQ], bf16)      # rope'd+normalized Q
            nsum = resid.tile([128, 8, SQ], bf16)    # numerator accumulators
            dsums = resid.tile([128, 8, SQ], bf16)   # exp-sum per head
            rs_s = resid.tile([128, nt], f32)        # per-key rms scale

            nc.sync.dma_start(wk_s[:], wk)
            nc.sync.dma_start(memt_blks[0][:],
                              memT[:, 0:512].rearrange("(i p) t -> p i t",
                                                       p=128))

            # ============ phase A: K-proj + rope per 512-key block ============
            with tc.tile_pool(name="kpool", bufs=2) as kpool, \
                 tc.tile_pool(name="pp_kv", bufs=2, space="PSUM") as pp_kv, \
                 tc.tile_pool(name="pp_sw", bufs=2, space="PSUM") as pp_sw, \
                 tc.tile_pool(name="pp_rs", bufs=2, space="PSUM") as pp_rs:
                ctk_s = kpool.tile([128, pc], bf16, tag="ctk", bufs=1)
                nc.sync.dma_start(ctk_s[:], ctk)
                stk_s = kpool.tile([128, pc], bf16, tag="stk", bufs=1)
                nc.sync.dma_start(stk_s[:], stk)
                nc.sync.dma_start(memt_blks[1][:],
                                  memT[:, 512:1024].rearrange(
                                      "(i p) t -> p i t", p=128))
                nc.sync.dma_start(wv_s[:], wv)
                nc.sync.dma_start(xt_s[:],
                                  xT.rearrange("(i p) q -> p i q", p=128))
                nc.sync.dma_start(wqo_s[:], wq)
                for b in range(nblk):
                    c0 = b * 512
                    memt = memt_blks[b % 2]
                    if b >= 2:
                        nc.sync.dma_start(
                            memt[:],
                            memT[:, c0:c0 + 512].rearrange("(i p) t -> p i t",
                                                           p=128))
                    yk = kpool.tile([128, 8, 512], bf16, tag="yk")
                    sw = kpool.tile([128, 8, 512], bf16, tag="sw")
                    if pair:
                        ps_rs = pp_rs.tile([1, 512], f32, tag="psrs",
                                           name="ps_rs")
                    else:
                        ps_rs = pp_rs.tile([128, 4], f32, tag="psrs",
                                           name="ps_rs")
                    for o in range(8):
                        ps_y = pp_kv.tile([128, 512], f32, tag="ps")
                        for i in range(8):
                            nc.tensor.matmul(ps_y[:], wk_s[:, i, o, :],
                                             memt[:, i, :],
                                             start=(i == 0), stop=(i == 7))
                        if has_bk:
                            nc.scalar.activation(yk[:, o, :], ps_y[:],
                                                 AF.Identity,
                                                 bias=bk_s[:, o:o + 1])
                        else:
                            nc.scalar.activation(yk[:, o, :], ps_y[:], AF.Copy)
                        ps_sw = pp_sw.tile([128, 512], f32, tag="ps")
                        nc.tensor.matmul(ps_sw[:], pt_s[:], yk[:, o, :])
                        nc.scalar.activation(sw[:, o, :], ps_sw[:], AF.Copy)
                        ysq = kpool.tile([128, 512], bf16, tag="ysqo", bufs=2)
                        nc.vector.tensor_mul(ysq[:], yk[:, o, :], yk[:, o, :])
                        if pair:
                            nc.tensor.matmul(ps_rs[:], ones_s[:], ysq[:],
                                             start=(o == 0), stop=(o == 7))
                        else:
                            for tt in range(4):
                                nc.tensor.matmul(
                                    ps_rs[:, tt:tt + 1],
                                    ysq[:, tt * 128:(tt + 1) * 128], ones_s[:],
                                    start=(o == 0), stop=(o == 7))
                    # rs = (1/sqrt(HD))/sqrt(ms+eps) = 1/sqrt(sum/8+128eps)
                    if pair:
                        rroot = kpool.tile([1, 512], f32, tag="rroot")
                        nc.scalar.activation(rroot[:], ps_rs[:], AF.Sqrt,
                                             bias=ep128[:], scale=0.125)
                        rr = kpool.tile([1, 512], f32, tag="rr")
                        nc.vector.reciprocal(rr[:], rroot[:])
                        rsb = kpool.tile([128, 512], f32, tag="rsb")
                        nc.gpsimd.partition_broadcast(rsb[:], rr[:])
                        cts = kpool.tile([128, 512], bf16, tag="cts")
                        nc.vector.tensor_mul(cts[:], ctk_s[:, c0:c0 + 512],
                                             rsb[:])
                        sts = kpool.tile([128, 512], bf16, tag="sts")
                        nc.vector.tensor_mul(sts[:], stk_s[:, c0:c0 + 512],
                                             rsb[:])
                    else:
                        rroot = kpool.tile([128, 4], f32, tag="rroot")
                        nc.scalar.activation(rroot[:], ps_rs[:], AF.Sqrt,
                                             bias=epsk[:], scale=0.125)
                        nc.vector.reciprocal(rs_s[:, b * 4:b * 4 + 4],
                                             rroot[:])
                        cts = ctk_s[:, c0:c0 + 512]
                        sts = stk_s[:, c0:c0 + 512]
                    for o in range(8):
                        t1 = kpool.tile([128, 512], bf16, tag="t1")
                        nc.vector.tensor_mul(t1[:], yk[:, o, :], cts[:])
                        t2 = kpool.tile([128, 512], bf16, tag="t2")
                        nc.vector.tensor_mul(t2[:], sw[:, o, :], sts[:])
                        nc.vector.tensor_add(kr[:, o, c0:c0 + 512], t1[:], t2[:])
                    if b >= 2:
                        # refill this ring slot for phase B (blocks 0/1)
                        c2 = (b - 2) * 512
                        nc.sync.dma_start(
                            memt[:],
                            memT[:, c2:c2 + 512].rearrange("(i p) t -> p i t",
                                                           p=128))

            # ========= phase Q + B (shared scope: no drain between) =========
            with tc.tile_pool(name="qlate", bufs=1) as qlate, \
                 tc.tile_pool(name="bpool", bufs=2) as bpool, \
                 tc.tile_pool(name="ptsp", bufs=3) as ptsp, \
                 tc.tile_pool(name="pp_v", bufs=3, space="PSUM") as pp_v, \
                 tc.tile_pool(name="pp_sc", bufs=2, space="PSUM") as pp_sc, \
                 tc.tile_pool(name="pp_qrs", bufs=1, space="PSUM") as pp_qrs:
                qtab_cm = tc.tile_pool(name="qtab", bufs=1)
                qtab = qtab_cm.__enter__()
                ctq_s = qtab.tile([128, 8, SQ], bf16)
                nc.sync.dma_start(ctq_s[:], ctq)
                stq_s = qtab.tile([128, 8, SQ], bf16)
                nc.sync.dma_start(stq_s[:], stq)
                yq = qlate.tile([128, 8, SQ], bf16)
                swq = qlate.tile([128, 8, SQ], bf16)

                ps_rsq = pp_qrs.tile([1, SQ], f32)
                for o in range(8):
                    ps2q = pp_sc.tile([128, 2, 512], f32, tag="ps2")
                    for i in range(8):
                        nc.tensor.matmul(ps2q[:, 0, :], wqo_s[:, i, o, :],
                                         xt_s[:, i, :],
                                         start=(i == 0), stop=(i == 7))
                    if has_bq:
                        nc.scalar.activation(yq[:, o, :], ps2q[:, 0, :],
                                             AF.Identity,
                                             bias=bq_s[:, o:o + 1])
                    else:
                        nc.scalar.activation(yq[:, o, :], ps2q[:, 0, :],
                                             AF.Copy)
                    nc.tensor.matmul(ps2q[:, 1, :], pt_s[:], yq[:, o, :])
                    nc.scalar.activation(swq[:, o, :], ps2q[:, 1, :], AF.Copy)
                    ysq = qlate.tile([128, SQ], bf16, tag="ysqq", bufs=2)
                    nc.vector.tensor_mul(ysq[:], yq[:, o, :], yq[:, o, :])
                    nc.tensor.matmul(ps_rsq[:], ones_s[:], ysq[:],
                                     start=(o == 0), stop=(o == 7))
                # V-proj helpers (phase B work)
                def v_chain(b, vt, tt, oh):
                    memt = memt_blks[b % 2]
                    ps_v = pp_v.tile([128, 512], f32, tag="ps", name="ps_v")
                    for i in range(8):
                        nc.tensor.matmul(
                            ps_v[:], memt[:, i, tt * 128:(tt + 1) * 128],
                            wv_s[:, i, oh * 512:(oh + 1) * 512],
                            start=(i == 0), stop=(i == 7))
                    if has_bv:
                        nc.vector.tensor_add(
                            vt[:, tt, oh * 512:(oh + 1) * 512], ps_v[:],
                            bv_s[:, oh * 512:(oh + 1) * 512])
                    elif (tt + oh) % 2 == 0:
                        nc.scalar.activation(
                            vt[:, tt, oh * 512:(oh + 1) * 512], ps_v[:],
                            AF.Copy)
                    else:
                        nc.vector.tensor_copy(
                            vt[:, tt, oh * 512:(oh + 1) * 512], ps_v[:])

                def v_proj(b, vt):
                    for tt in range(4):
                        for oh in range(2):
                            v_chain(b, vt, tt, oh)

                vts = [None] * nblk
                for b in range(2):
                    vts[b] = bpool.tile([128, 4, DIM], bf16, tag="vt",
                                        name=f"vt{b}", bufs=3)
                    v_proj(b, vts[b])
                    if b + 2 < nblk:
                        c2 = (b + 2) * 512
                        nc.sync.dma_start(
                            memt_blks[b % 2][:],
                            memT[:, c2:c2 + 512].rearrange("(i p) t -> p i t",
                                                           p=128))
                for b in range(2, nblk):
                    vts[b] = bpool.tile([128, 4, DIM], bf16, tag="vt",
                                        name=f"vt{b}", bufs=3)
                # finish Q: rmsnorm scale + rope (wide in-place DVE ops)
                rrootq = qlate.tile([1, SQ], f32)
                nc.scalar.activation(rrootq[:], ps_rsq[:], AF.Sqrt,
                                     bias=epsq[:], scale=1.0 / DIM)
                nc.vector.reciprocal(rrootq[:], rrootq[:])
                rsbf = qlate.tile([128, SQ], f32)
                nc.gpsimd.partition_broadcast(rsbf[:], rrootq[:])
                nc.vector.tensor_mul(yq[:], yq[:], ctq_s[:])
                nc.vector.tensor_mul(swq[:], swq[:], stq_s[:])
                nc.vector.tensor_add(yq[:], yq[:], swq[:])
                for o in range(8):
                    nc.vector.tensor_mul(qT[:, o, :], yq[:, o, :], rsbf[:])
                qtab_cm.__exit__(None, None, None)

                # ---- attention blocks (V-proj for b+2 interleaved) ----
                for b in range(nblk):
                    vt = vts[b]
                    for h in range(8):
                        pts = ptsp.tile([128, 4, SQ], bf16, tag="pts")
                        for tp in range(2):
                            ps2 = pp_sc.tile([128, 2, 512], f32, tag="ps2")
                            for half in range(2):
                                lt = 2 * tp + half
                                gt = b * 4 + lt
                                nc.tensor.matmul(
                                    ps2[:, half, :],
                                    kr[:, h, gt * 128:(gt + 1) * 128],
                                    qT[:, h, :])
                            if not pair:
                                for half in range(2):
                                    lt = 2 * tp + half
                                    gt = b * 4 + lt
                                    bias_ap = (mb_s[:, lt:lt + 1]
                                               if b == nblk - 1 else zcol[:])
                                    nc.scalar.activation(
                                        pts[:, lt, :], ps2[:, half, :],
                                        AF.Exp, bias=bias_ap,
                                        scale=rs_s[:, gt:gt + 1])
                            elif b == nblk - 1:
                                for half in range(2):
                                    lt = 2 * tp + half
                                    nc.scalar.activation(
                                        pts[:, lt, :], ps2[:, half, :],
                                        AF.Exp, bias=mb_s[:, lt:lt + 1])
                            else:
                                nc.scalar.activation(
                                    pts[:, 2 * tp:2 * tp + 2, :], ps2[:],
                                    AF.Exp, bias=zcol[:])
                        ps_a = pp_v.tile([128, SQ], f32, tag="ps")
                        for tt in range(4):
                            nc.tensor.matmul(
                                ps_a[:], vt[:, tt, h * 128:(h + 1) * 128],
                                pts[:, tt, :], start=(tt == 0), stop=(tt == 3))
                        if b == 0:
                            nc.vector.tensor_copy(nsum[:, h, :], ps_a[:])
                        else:
                            nc.vector.tensor_add(nsum[:, h, :], nsum[:, h, :],
                                                 ps_a[:])
                        # exp-sum partial for this block (bf16 tree)
                        nc.vector.tensor_add(pts[:, 0:2, :], pts[:, 0:2, :],
                                             pts[:, 2:4, :])
                        if b == 0:
                            nc.gpsimd.tensor_add(dsums[:, h, :], pts[:, 0, :],
                                                 pts[:, 1, :])
                        else:
                            nc.vector.tensor_add(pts[:, 0, :], pts[:, 0, :],
                                                 pts[:, 1, :])
                            nc.gpsimd.tensor_add(dsums[:, h, :], dsums[:, h, :],
                                                 pts[:, 0, :])
                        if h == 3 and b == nblk - 1:
                            nc.sync.dma_start(
                                cat[0:512, :].rearrange("(h p) q -> p h q",
                                                        p=128),
                                nsum[:, 0:4, :])
                            if _sim:
                                nc.gpsimd.dma_start(cat_sh[0:512, :],
                                                    cat[0:512, :])
                    if b + 2 < nblk:
                        v_proj(b + 2, vts[b + 2])
                nc.sync.dma_start(
                    cat[512:DIM, :].rearrange("(h p) q -> p h q", p=128),
                    nsum[:, 4:8, :])
                # wo into the wq buffer for the tail
                nc.sync.dma_start(wqo_s[:], wo)

            with tc.tile_pool(name="dpool", bufs=1) as dpool, \
                 tc.tile_pool(name="pp_dn", bufs=1, space="PSUM") as pp_dn:
                ps_den = pp_dn.tile([8, SQ], f32)
                for h in range(8):
                    nc.tensor.matmul(ps_den[:], ec_s[:, h, :], dsums[:, h, :],
                                     start=(h == 0), stop=(h == 7))
                dcp = dpool.tile([8, SQ], bf16)
                nc.vector.tensor_copy(dcp[:], ps_den[:])
                nc.sync.dma_start(cat[DIM:DIM + HEADS, :], dcp[:])

            if _sim:
                nc.gpsimd.dma_start(cat_sh[512:DIM + HEADS, :],
                                    cat[512:DIM + HEADS, :])
            else:
                nc.gpsimd.collective_compute(
                    "AllReduce", mybir.AluOpType.add,
                    replica_groups=[list(range(N_CORES))],
                    ins=[cat[:]], outs=[cat_sh[:]])

            # ========== per-core output projection on its query slice ==========
            with tc.tile_pool(name="tail", bufs=1) as tail, \
                 tc.tile_pool(name="pp_t", bufs=2, space="PSUM") as pp_t:
                nred = tail.tile([128, 8, QS], bf16)
                dred = tail.tile([1, 8, QS], bf16)
                pid = nc.sync.partition_id()
                qoff = pid * QS
                nc.sync.dma_start(
                    nred[:],
                    cat_sh[0:DIM, bass.ds(qoff, QS)].rearrange(
                        "(h p) q -> p h q", p=128))
                nc.sync.dma_start(
                    dred[:],
                    cat_sh[DIM:DIM + HEADS, bass.ds(qoff, QS)].rearrange(
                        "(o h) q -> o h q", o=1))
                rd = tail.tile([1, 8 * QS], f32)
                nc.vector.reciprocal(rd[:], dred.rearrange("o h q -> o (h q)")[:])
                ps_rd = pp_t.tile([128, 8 * QS], f32, tag="psrd")
                nc.tensor.matmul(ps_rd[:], ones_rs[:], rd[:])
                nsc = tail.tile([128, 8, QS], bf16)
                nc.vector.tensor_mul(nsc.rearrange("p h q -> p (h q)")[:],
                                     nred.rearrange("p h q -> p (h q)")[:],
                                     ps_rd[:])
                out_sb = tail.tile([128, 8, QS], f32)
                for e in range(8):
                    ps_o = pp_t.tile([128, QS], f32, tag="ps")
                    for o in range(8):
                        nc.tensor.matmul(ps_o[:], wqo_s[:, o, e, :],
                                         nsc[:, o, :],
                                         start=(o == 0), stop=(o == 7))
                    if has_bo:
                        nc.scalar.activation(out_sb[:, e, :], ps_o[:],
                                             AF.Identity, bias=bo_s[:, e:e + 1])
                    else:
                        nc.scalar.activation(out_sb[:, e, :], ps_o[:], AF.Copy)
                nc.sync.dma_start(
                    outT.rearrange("(e p) q -> p e q", p=128)[:, :, 0:QS],
                    out_sb[:])

    nc.compile()
    _cache[key] = nc
    return nc


def _prep(x, mem, mask, cos_q, sin_q, cos_k, sin_k,
          Wq, bq, Wk, bk, Wv, bv, Wo, bo, gq, gk):
    import ml_dtypes
    f = np.float32
    bf = ml_dtypes.bfloat16
    x = np.asarray(x, f).reshape(SQ, DIM)
    mem = np.asarray(mem, f).reshape(SK, DIM)
    mask = np.asarray(mask).reshape(SK)
    cos_q = np.asarray(cos_q, f)
    sin_q = np.asarray(sin_q, f)
    cos_k = np.asarray(cos_k, f)
    sin_k = np.asarray(sin_k, f)
    Wq, Wk, Wv, Wo = (np.asarray(w, f) for w in (Wq, Wk, Wv, Wo))
    bq, bk, bv, bo, gq, gk = (np.asarray(v, f) for v in (bq, bk, bv, bo, gq, gk))

    if not np.allclose(gk, 1.0):
        gkp = gk.reshape(-1, 2)
        assert np.allclose(gkp[:, 0], gkp[:, 1]), "unsupported non-pairwise gk"

    flags = tuple(bool(np.any(v != 0.0)) for v in (bq, bk, bv, bo))

    # compact keys: keep only unmasked, shard evenly, pad per-core to 512
    idx = np.flatnonzero(mask)
    keep = len(idx)
    percore = -(-keep // N_CORES)                # ceil
    pc = max(512, -(-percore // 512) * 512)      # pad to multiple of 512
    nt = pc // 128

    def tile_w(WT):  # [1024,1024] (in,out of W.T) -> [p, i, o, m]
        return np.ascontiguousarray(
            WT.reshape(8, 128, 8, 128).transpose(1, 0, 2, 3)).astype(bf)

    ii = np.arange(128)
    jj = ii // 2
    partner = ii ^ 1

    # fold gq (and pairwise gk) into the q rope tables; sin pairs with
    # partner's gq
    gq_t = (gq * gk).reshape(8, 128)
    gq_sin = (gq.reshape(8, 128)[:, partner] * gk.reshape(8, 128))
    cq = cos_q[:, jj].T                # [128, SQ]
    sq = sin_q[:, jj].T
    ctq = np.ascontiguousarray(
        (cq[None, :, :] * gq_t[:, :, None]).transpose(1, 0, 2)).astype(bf)
    stq = np.ascontiguousarray(
        (sq[None, :, :] * gq_sin[:, :, None]).transpose(1, 0, 2)).astype(bf)

    PT = np.zeros((128, 128), f)
    even = ii[ii % 2 == 0]
    PT[even + 1, even] = -1.0
    PT[even, even + 1] = 1.0

    ec = np.zeros((128, 8, 8), f)
    for h in range(8):
        ec[:, h, h] = 1.0

    shared = {
        "xT": np.ascontiguousarray(x.T).astype(bf),
        "wq": tile_w(Wq.T), "wk": tile_w(Wk.T), "wo": tile_w(Wo.T),
        "wv": np.ascontiguousarray(
            Wv.T.reshape(8, 128, DIM).transpose(1, 0, 2)).astype(bf),
        "ctq": ctq, "stq": stq,
        "bq_t": np.ascontiguousarray(bq.reshape(8, 128).T),
        "bk_t": np.ascontiguousarray(bk.reshape(8, 128).T),
        "bo_t": np.ascontiguousarray(bo.reshape(8, 128).T),
        "bv_t": np.ascontiguousarray(np.tile(bv, (128, 1))),
        "pmat": PT.astype(bf),
        "ones_c": np.ones((128, 1), bf),
        "ecols": ec.astype(bf),
    }

    # global compacted + padded arrays
    tot = N_CORES * pc
    memT_full = np.zeros((DIM, tot), bf)
    ctk_full = np.zeros((HD, tot), bf)
    stk_full = np.zeros((HD, tot), bf)
    mb_full = np.full(tot, NEG, f)

    ck = cos_k[:, jj].T.astype(f)   # [128, SK]
    sk_t = sin_k[:, jj].T.astype(f)
    counts = [keep // N_CORES + (1 if c < keep % N_CORES else 0)
              for c in range(N_CORES)]
    off = 0
    for c in range(N_CORES):
        sl = idx[off:off + counts[c]]
        off += counts[c]
        d0 = c * pc
        memT_full[:, d0:d0 + len(sl)] = mem[sl].T.astype(bf)
        ctk_full[:, d0:d0 + len(sl)] = ck[:, sl].astype(bf)
        stk_full[:, d0:d0 + len(sl)] = sk_t[:, sl].astype(bf)
        mb_full[d0:d0 + len(sl)] = 0.0
        assert pc - len(sl) < 512, "padding must fit in last 4 tiles"

    in_maps = []
    for c in range(N_CORES):
        s = slice(c * pc, (c + 1) * pc)
        m = dict(shared)
        m["memT"] = np.ascontiguousarray(memT_full[:, s])
        m["ctk"] = np.ascontiguousarray(ctk_full[:, s])
        m["stk"] = np.ascontiguousarray(stk_full[:, s])
        # bias columns for the last 4 tiles only
        mb = mb_full[s][-512:].reshape(4, 128).T
        m["mbias"] = np.ascontiguousarray(mb)
        in_maps.append(m)
    return in_maps, nt, flags


def kernel(**inputs):
    from concourse.bass_utils import run_bass_kernel_spmd
    in_maps, nt, flags = _prep(**inputs)
    nc = _build(nt, flags)
    res = run_bass_kernel_spmd(nc, in_maps, list(range(N_CORES)))
    parts = [res.results[c]["outT"][:, 0:QS].T for c in range(N_CORES)]
    out = np.concatenate(parts, axis=0)
    return out[None].astype(np.float32)


# revision 36
# speedup vs baseline: 1.0282x; 1.0282x over previous
"""Trainium2 Bass kernel for nn_MemoryRetriever (cross-attention memory retriever).

v2: mask-compacted keys.  The per-key boolean mask keeps ~half of the 31290
memory tokens; the host gathers only surviving keys (plus their RoPE table
columns) and shards them across the 8 cores (PC keys/core, padded to a
multiple of 512; padding is masked via a -1e30 exp bias confined to the last
4 key tiles).  Each core computes K/V projections + RMSNorm + 3D-RoPE for its
shard, full Q (replicated, small), local masked-softmax partials
(un-normalized numerator + denominator), then one AllReduce combines partials
and each core output-projects its own 64-query slice.

Engine balance (vs v1): all elementwise math in bf16 (2x DVE mode); the
per-key RMSNorm scale and 1/sqrt(hd) are folded into the rope cos/sin tables
so Exp runs with scalar scale/bias over 2-tile-wide PSUM pairs; sum-of-squares
runs as near-free ones-stationary PE matmuls; attnV accumulates in PSUM across
all key tiles per 2-head group and is DMA'd straight from PSUM into the
collective buffer; softmax denominators tree-reduce on DVE in bf16 and pack
into one PSUM bank via indicator-column matmuls.
"""

import sys
import numpy as np

sys.path.insert(0, "/opt/trn_rl_repo")

DIM = 1024
HEADS = 8
HD = 128
SQ = 512
SK = 31290
N_CORES = 8
QS = SQ // N_CORES
EPS = 1e-6
NEG = -1.0e30

_cache = {}


def _build(nt, flags, pair=True):
    key = ("nc", nt, flags, pair)
    if key in _cache:
        return _cache[key]

    import concourse.bass as bass
    import concourse.tile as tile
    from concourse import mybir, bacc

    f32 = mybir.dt.float32
    bf16 = mybir.dt.bfloat16
    AF = mybir.ActivationFunctionType
    has_bq, has_bk, has_bv, has_bo = flags

    pc = nt * 128          # keys per core (padded)
    nblk = nt // 4         # 512-key blocks
    npair = nt // 2        # tile pairs (wide exp)

    nc = bacc.Bacc("TRN2", target_bir_lowering=False, debug=False,
                   num_devices=N_CORES)

    def din(name, shape, dt=f32):
        return nc.dram_tensor(name, list(shape), dt, kind="ExternalInput").ap()

    # per-core sharded inputs
    memT = din("memT", [DIM, pc], bf16)      # compacted mem shard, feature-major
    ctk = din("ctk", [HD, pc], bf16)         # K rope cos (pair-major rows)
    stk = din("stk", [HD, pc], bf16)
    mbias = din("mbias", [128, 4])           # pad bias (0/-1e30) for last 4 tiles
    # shared inputs
    xT = din("xT", [DIM, SQ], bf16)
    wq = din("wq", [128, 8, 8, 128], bf16)   # [p,i,o,m] = Wq.T[i*128+p, o*128+m]
    wk = din("wk", [128, 8, 8, 128], bf16)
    wo = din("wo", [128, 8, 8, 128], bf16)
    wv = din("wv", [128, 8, DIM], bf16)      # [p,i,o] = Wv.T[i*128+p, o]
    ctq = din("ctq", [128, 8, SQ], bf16)     # q rope cos (gq*gk folded, per head)
    stq = din("stq", [128, 8, SQ], bf16)
    bq_t = din("bq_t", [128, 8])
    bk_t = din("bk_t", [128, 8])
    bo_t = din("bo_t", [128, 8])
    bv_t = din("bv_t", [128, DIM])
    pmat = din("pmat", [128, 128], bf16)     # P.T for rope pair swap (+-1)
    ones_c = din("ones_c", [128, 1], bf16)
    ones_r = din("ones_r", [1, 128])
    ecols = din("ecols", [128, 8, 8], bf16)  # ecols[p,h,m] = (m==h)

    outT = nc.dram_tensor("outT", [DIM, SQ], f32, kind="ExternalOutput").ap()

    import os as _os
    _sim = _os.environ.get("KSIM", "0") == "1"
    cat = nc.dram_tensor("cat", [DIM + HEADS, SQ], bf16)
    cat_sh = nc.dram_tensor("cat_sh", [DIM + HEADS, SQ], bf16,
                            addr_space="Shared")

    with tile.TileContext(nc) as tc:
        with tc.tile_pool(name="consts", bufs=1) as consts, \
             tc.tile_pool(name="resid", bufs=1) as resid:

            # ---- resident tensors; DMAs issued in first-use order ----
            wk0_s = resid.tile([128, 1, 8, 128], bf16)
            wkr_s = resid.tile([128, 7, 8, 128], bf16)
            memt_blks = [resid.tile([128, 8, 512], bf16, tag=f"memt{b}",
                                    name=f"memt{b}")
                         for b in range(2)]
            wqo_s = resid.tile([128, 8, 8, 128], bf16)  # wq then wo
            xt_s = resid.tile([128, 8, SQ], bf16)
            wv_s = resid.tile([128, 8, DIM], bf16)
            kr = resid.tile([128, 8, pc], bf16)      # rope'd K (unnormalized)
            qT = resid.tile([128, 8, SQ], bf16)      # rope'd+normalized Q
            nsum = resid.tile([128, 8, SQ], bf16)    # numerator accumulators
            dsums = resid.tile([128, 8, SQ], bf16)   # exp-sum per head
            rs_s = resid.tile([128, nt], f32)        # per-key rms scale

            nc.sync.dma_start(memt_blks[0][:],
                              memT[:, 0:512].rearrange("(i p) t -> p i t",
                                                       p=128))
            nc.sync.dma_start(wk0_s[:], wk[:, 0:1, :, :])
            nc.sync.dma_start(wkr_s[:], wk[:, 1:8, :, :])
            # ---- small constants (cheap DMAs, SP queue) ----
            pt_s = consts.tile([128, 128], bf16)
            nc.sync.dma_start(pt_s[:], pmat)
            ones_s = consts.tile([128, 1], bf16)
            nc.sync.dma_start(ones_s[:], ones_c)
            ones_rs = consts.tile([1, 128], f32)
            nc.sync.dma_start(ones_rs[:], ones_r)
            mb_s = consts.tile([128, 4], f32)
            nc.sync.dma_start(mb_s[:], mbias)
            ec_s = consts.tile([128, 8, 8], bf16)
            nc.sync.dma_start(ec_s[:], ecols)
            zcol = consts.tile([128, 1], f32)
            nc.vector.memset(zcol[:], 0.0)
            ep128 = consts.tile([1, 1], f32)
            nc.vector.memset(ep128[:], 128.0 * EPS)
            epsk = consts.tile([128, 1], f32)
            nc.vector.memset(epsk[:], 128.0 * EPS)
            epsq = consts.tile([1, 1], f32)
            nc.vector.memset(epsq[:], EPS)
            if has_bq:
                bq_s = consts.tile([128, 8], f32)
                nc.sync.dma_start(bq_s[:], bq_t)
            if has_bk:
                bk_s = consts.tile([128, 8], f32)
                nc.sync.dma_start(bk_s[:], bk_t)
            if has_bo:
                bo_s = consts.tile([128, 8], f32)
                nc.sync.dma_start(bo_s[:], bo_t)
            if has_bv:
                bv_s = consts.tile([128, DIM], f32)
                nc.sync.dma_start(bv_s[:], bv_t)


            # ============ phase A: K-proj + rope per 512-key block ============
            with tc.tile_pool(name="kpool", bufs=2) as kpool, \
                 tc.tile_pool(name="pp_kv", bufs=2, space="PSUM") as pp_kv, \
                 tc.tile_pool(name="pp_sw", bufs=2, space="PSUM") as pp_sw, \
                 tc.tile_pool(name="pp_rs", bufs=2, space="PSUM") as pp_rs:
                ctk_s = kpool.tile([128, pc], bf16, tag="ctk", bufs=1)
                nc.sync.dma_start(ctk_s[:], ctk)
                stk_s = kpool.tile([128, pc], bf16, tag="stk", bufs=1)
                nc.sync.dma_start(stk_s[:], stk)
                nc.sync.dma_start(memt_blks[1][:],
                                  memT[:, 512:1024].rearrange(
                                      "(i p) t -> p i t", p=128))
                nc.sync.dma_start(wv_s[:], wv)
                nc.sync.dma_start(xt_s[:],
                                  xT.rearrange("(i p) q -> p i q", p=128))
                nc.sync.dma_start(wqo_s[:], wq)
                for b in range(nblk):
                    c0 = b * 512
                    memt = memt_blks[b % 2]
                    if b >= 2:
                        nc.sync.dma_start(
                            memt[:],
                            memT[:, c0:c0 + 512].rearrange("(i p) t -> p i t",
                                                           p=128))
                    yk = kpool.tile([128, 8, 512], bf16, tag="yk")
                    sw = kpool.tile([128, 8, 512], bf16, tag="sw")
                    if pair:
                        ps_rs = pp_rs.tile([1, 512], f32, tag="psrs",
                                           name="ps_rs")
                    else:
                        ps_rs = pp_rs.tile([128, 4], f32, tag="psrs",
                                           name="ps_rs")
                    for o in range(8):
                        ps_y = pp_kv.tile([128, 512], f32, tag="ps")
                        for i in range(8):
                            wsl = (wk0_s[:, 0, i, :] if o == 0
                                   else wkr_s[:, o - 1, i, :])
                            nc.tensor.matmul(ps_y[:], wsl,
                                             memt[:, i, :],
                                             start=(i == 0), stop=(i == 7))
                        if has_bk:
                            nc.scalar.activation(yk[:, o, :], ps_y[:],
                                                 AF.Identity,
                                                 bias=bk_s[:, o:o + 1])
                        else:
                            nc.scalar.activation(yk[:, o, :], ps_y[:], AF.Copy)
                        ps_sw = pp_sw.tile([128, 512], f32, tag="ps")
                        nc.tensor.matmul(ps_sw[:], pt_s[:], yk[:, o, :])
                        nc.scalar.activation(sw[:, o, :], ps_sw[:], AF.Copy)
                        ysq = kpool.tile([128, 512], bf16, tag="ysqo", bufs=2)
                        nc.vector.tensor_mul(ysq[:], yk[:, o, :], yk[:, o, :])
                        if pair:
                            nc.tensor.matmul(ps_rs[:], ones_s[:], ysq[:],
                                             start=(o == 0), stop=(o == 7))
                        else:
                            for tt in range(4):
                                nc.tensor.matmul(
                                    ps_rs[:, tt:tt + 1],
                                    ysq[:, tt * 128:(tt + 1) * 128], ones_s[:],
                                    start=(o == 0), stop=(o == 7))
                    # rs = (1/sqrt(HD))/sqrt(ms+eps) = 1/sqrt(sum/8+128eps)
                    if pair:
                        rroot = kpool.tile([1, 512], f32, tag="rroot")
                        nc.scalar.activation(rroot[:], ps_rs[:], AF.Sqrt,
                                             bias=ep128[:], scale=0.125)
                        rr = kpool.tile([1, 512], f32, tag="rr")
                        nc.vector.reciprocal(rr[:], rroot[:])
                        rsb = kpool.tile([128, 512], f32, tag="rsb")
                        nc.gpsimd.partition_broadcast(rsb[:], rr[:])
                        cts = kpool.tile([128, 512], bf16, tag="cts")
                        nc.vector.tensor_mul(cts[:], ctk_s[:, c0:c0 + 512],
                                             rsb[:])
                        sts = kpool.tile([128, 512], bf16, tag="sts")
                        nc.vector.tensor_mul(sts[:], stk_s[:, c0:c0 + 512],
                                             rsb[:])
                    else:
                        rroot = kpool.tile([128, 4], f32, tag="rroot")
                        nc.scalar.activation(rroot[:], ps_rs[:], AF.Sqrt,
                                             bias=epsk[:], scale=0.125)
                        nc.vector.reciprocal(rs_s[:, b * 4:b * 4 + 4],
                                             rroot[:])
                        cts = ctk_s[:, c0:c0 + 512]
                        sts = stk_s[:, c0:c0 + 512]
                    for o in range(8):
                        t1 = kpool.tile([128, 512], bf16, tag="t1")
                        nc.vector.tensor_mul(t1[:], yk[:, o, :], cts[:])
                        t2 = kpool.tile([128, 512], bf16, tag="t2")
                        nc.vector.tensor_mul(t2[:], sw[:, o, :], sts[:])
                        nc.vector.tensor_add(kr[:, o, c0:c0 + 512], t1[:], t2[:])
                    if b >= 2:
                        # refill this ring slot for phase B (blocks 0/1)
                        c2 = (b - 2) * 512
                        nc.sync.dma_start(
                            memt[:],
                            memT[:, c2:c2 + 512].rearrange("(i p) t -> p i t",
                                                           p=128))

            # ========= phase Q + B (shared scope: no drain between) =========
            with tc.tile_pool(name="qlate", bufs=1) as qlate, \
                 tc.tile_pool(name="bpool", bufs=2) as bpool, \
                 tc.tile_pool(name="ptsp", bufs=3) as ptsp, \
                 tc.tile_pool(name="pp_v", bufs=3, space="PSUM") as pp_v, \
                 tc.tile_pool(name="pp_sc", bufs=2, space="PSUM") as pp_sc, \
                 tc.tile_pool(name="pp_qrs", bufs=1, space="PSUM") as pp_qrs:
                qtab_cm = tc.tile_pool(name="qtab", bufs=1)
                qtab = qtab_cm.__enter__()
                ctq_s = qtab.tile([128, 8, SQ], bf16)
                nc.sync.dma_start(ctq_s[:], ctq)
                stq_s = qtab.tile([128, 8, SQ], bf16)
                nc.sync.dma_start(stq_s[:], stq)
                yq = qlate.tile([128, 8, SQ], bf16)
                swq = qlate.tile([128, 8, SQ], bf16)

                ps_rsq = pp_qrs.tile([1, SQ], f32)
                for o in range(8):
                    ps2q = pp_sc.tile([128, 2, 512], f32, tag="ps2")
                    for i in range(8):
                        nc.tensor.matmul(ps2q[:, 0, :], wqo_s[:, o, i, :],
                                         xt_s[:, i, :],
                                         start=(i == 0), stop=(i == 7))
                    if has_bq:
                        nc.scalar.activation(yq[:, o, :], ps2q[:, 0, :],
                                             AF.Identity,
                                             bias=bq_s[:, o:o + 1])
                    else:
                        nc.scalar.activation(yq[:, o, :], ps2q[:, 0, :],
                                             AF.Copy)
                    nc.tensor.matmul(ps2q[:, 1, :], pt_s[:], yq[:, o, :])
                    nc.scalar.activation(swq[:, o, :], ps2q[:, 1, :], AF.Copy)
                    ysq = qlate.tile([128, SQ], bf16, tag="ysqq", bufs=2)
                    nc.vector.tensor_mul(ysq[:], yq[:, o, :], yq[:, o, :])
                    nc.tensor.matmul(ps_rsq[:], ones_s[:], ysq[:],
                                     start=(o == 0), stop=(o == 7))
                # V-proj helpers (phase B work)
                def v_chain(b, vt, tt, oh):
                    memt = memt_blks[b % 2]
                    ps_v = pp_v.tile([128, 512], f32, tag="ps", name="ps_v")
                    for i in range(8):
                        nc.tensor.matmul(
                            ps_v[:], memt[:, i, tt * 128:(tt + 1) * 128],
                            wv_s[:, i, oh * 512:(oh + 1) * 512],
                            start=(i == 0), stop=(i == 7))
                    if has_bv:
                        nc.vector.tensor_add(
                            vt[:, tt, oh * 512:(oh + 1) * 512], ps_v[:],
                            bv_s[:, oh * 512:(oh + 1) * 512])
                    elif (tt + oh) % 2 == 0:
                        nc.scalar.activation(
                            vt[:, tt, oh * 512:(oh + 1) * 512], ps_v[:],
                            AF.Copy)
                    else:
                        nc.vector.tensor_copy(
                            vt[:, tt, oh * 512:(oh + 1) * 512], ps_v[:])

                def v_proj(b, vt):
                    for tt in range(4):
                        for oh in range(2):
                            v_chain(b, vt, tt, oh)

                vts = [None] * nblk
                for b in range(2):
                    vts[b] = bpool.tile([128, 4, DIM], bf16, tag="vt",
                                        name=f"vt{b}", bufs=3)
                    v_proj(b, vts[b])
                    if b + 2 < nblk:
                        c2 = (b + 2) * 512
                        nc.sync.dma_start(
                            memt_blks[b % 2][:],
                            memT[:, c2:c2 + 512].rearrange("(i p) t -> p i t",
                                                           p=128))
                for b in range(2, nblk):
                    vts[b] = bpool.tile([128, 4, DIM], bf16, tag="vt",
                                        name=f"vt{b}", bufs=3)
                # finish Q: rmsnorm scale + rope (wide in-place DVE ops)
                rrootq = qlate.tile([1, SQ], f32)
                nc.scalar.activation(rrootq[:], ps_rsq[:], AF.Sqrt,
                                     bias=epsq[:], scale=1.0 / DIM)
                nc.vector.reciprocal(rrootq[:], rrootq[:])
                rsbf = qlate.tile([128, SQ], f32)
                nc.gpsimd.partition_broadcast(rsbf[:], rrootq[:])
                nc.vector.tensor_mul(yq[:], yq[:], ctq_s[:])
                nc.vector.tensor_mul(swq[:], swq[:], stq_s[:])
                nc.vector.tensor_add(yq[:], yq[:], swq[:])
                for o in range(8):
                    nc.vector.tensor_mul(qT[:, o, :], yq[:, o, :], rsbf[:])
                qtab_cm.__exit__(None, None, None)

                # ---- attention blocks (V-proj for b+2 interleaved) ----
                for b in range(nblk):
                    vt = vts[b]
                    for h in range(8):
                        pts = ptsp.tile([128, 4, SQ], bf16, tag="pts")
                        for tp in range(2):
                            ps2 = pp_sc.tile([128, 2, 512], f32, tag="ps2")
                            for half in range(2):
                                lt = 2 * tp + half
                                gt = b * 4 + lt
                                nc.tensor.matmul(
                                    ps2[:, half, :],
                                    kr[:, h, gt * 128:(gt + 1) * 128],
                                    qT[:, h, :])
                            if not pair:
                                for half in range(2):
                                    lt = 2 * tp + half
                                    gt = b * 4 + lt
                                    bias_ap = (mb_s[:, lt:lt + 1]
                                               if b == nblk - 1 else zcol[:])
                                    nc.scalar.activation(
                                        pts[:, lt, :], ps2[:, half, :],
                                        AF.Exp, bias=bias_ap,
                                        scale=rs_s[:, gt:gt + 1])
                            elif b == nblk - 1:
                                for half in range(2):
                                    lt = 2 * tp + half
                                    nc.scalar.activation(
                                        pts[:, lt, :], ps2[:, half, :],
                                        AF.Exp, bias=mb_s[:, lt:lt + 1])
                            else:
                                nc.scalar.activation(
                                    pts[:, 2 * tp:2 * tp + 2, :], ps2[:],
                                    AF.Exp, bias=zcol[:])
                        ps_a = pp_v.tile([128, SQ], f32, tag="ps")
                        for tt in range(4):
                            nc.tensor.matmul(
                                ps_a[:], vt[:, tt, h * 128:(h + 1) * 128],
                                pts[:, tt, :], start=(tt == 0), stop=(tt == 3))
                        if b == 0:
                            nc.vector.tensor_copy(nsum[:, h, :], ps_a[:])
                        else:
                            nc.vector.tensor_add(nsum[:, h, :], nsum[:, h, :],
                                                 ps_a[:])
                        # exp-sum partial for this block (bf16 tree)
                        nc.vector.tensor_add(pts[:, 0:2, :], pts[:, 0:2, :],
                                             pts[:, 2:4, :])
                        if b == 0:
                            nc.gpsimd.tensor_add(dsums[:, h, :], pts[:, 0, :],
                                                 pts[:, 1, :])
                        else:
                            nc.vector.tensor_add(pts[:, 0, :], pts[:, 0, :],
                                                 pts[:, 1, :])
                            nc.gpsimd.tensor_add(dsums[:, h, :], dsums[:, h, :],
                                                 pts[:, 0, :])
                        if h == 3 and b == nblk - 1:
                            nc.sync.dma_start(
                                cat[0:512, :].rearrange("(h p) q -> p h q",
                                                        p=128),
                                nsum[:, 0:4, :])
                            if _sim:
                                nc.gpsimd.dma_start(cat_sh[0:512, :],
                                                    cat[0:512, :])
                    if b + 2 < nblk:
                        v_proj(b + 2, vts[b + 2])
                nc.sync.dma_start(
                    cat[512:DIM, :].rearrange("(h p) q -> p h q", p=128),
                    nsum[:, 4:8, :])
                # wo into the wq buffer for the tail
                nc.sync.dma_start(wqo_s[:], wo)

            with tc.tile_pool(name="dpool", bufs=1) as dpool, \
                 tc.tile_pool(name="pp_dn", bufs=1, space="PSUM") as pp_dn:
                ps_den = pp_dn.tile([8, SQ], f32)
                for h in range(8):
                    nc.tensor.matmul(ps_den[:], ec_s[:, h, :], dsums[:, h, :],
                                     start=(h == 0), stop=(h == 7))
                dcp = dpool.tile([8, SQ], bf16)
                nc.vector.tensor_copy(dcp[:], ps_den[:])
                nc.sync.dma_start(cat[DIM:DIM + HEADS, :], dcp[:])

            if _sim:
                nc.gpsimd.dma_start(cat_sh[512:DIM + HEADS, :],
                                    cat[512:DIM + HEADS, :])
            else:
                nc.gpsimd.collective_compute(
                    "AllReduce", mybir.AluOpType.add,
                    replica_groups=[list(range(N_CORES))],
                    ins=[cat[:]], outs=[cat_sh[:]])

            # ========== per-core output projection on its query slice ==========
            with tc.tile_pool(name="tail", bufs=1) as tail, \
                 tc.tile_pool(name="pp_t", bufs=2, space="PSUM") as pp_t:
                nred = tail.tile([128, 8, QS], bf16)
                dred = tail.tile([1, 8, QS], bf16)
                pid = nc.sync.partition_id()
                qoff = pid * QS
                nc.sync.dma_start(
                    nred[:],
                    cat_sh[0:DIM, bass.ds(qoff, QS)].rearrange(
                        "(h p) q -> p h q", p=128))
                nc.sync.dma_start(
                    dred[:],
                    cat_sh[DIM:DIM + HEADS, bass.ds(qoff, QS)].rearrange(
                        "(o h) q -> o h q", o=1))
                rd = tail.tile([1, 8 * QS], f32)
                nc.vector.reciprocal(rd[:], dred.rearrange("o h q -> o (h q)")[:])
                ps_rd = pp_t.tile([128, 8 * QS], f32, tag="psrd")
                nc.tensor.matmul(ps_rd[:], ones_rs[:], rd[:])
                nsc = tail.tile([128, 8, QS], bf16)
                nc.vector.tensor_mul(nsc.rearrange("p h q -> p (h q)")[:],
                                     nred.rearrange("p h q -> p (h q)")[:],
                                     ps_rd[:])
                out_sb = tail.tile([128, 8, QS], f32)
                for e in range(8):
                    ps_o = pp_t.tile([128, QS], f32, tag="ps")
                    for o in range(8):
                        nc.tensor.matmul(ps_o[:], wqo_s[:, e, o, :],
                                         nsc[:, o, :],
                                         start=(o == 0), stop=(o == 7))
                    if has_bo:
                        nc.scalar.activation(out_sb[:, e, :], ps_o[:],
                                             AF.Identity, bias=bo_s[:, e:e + 1])
                    else:
                        nc.scalar.activation(out_sb[:, e, :], ps_o[:], AF.Copy)
                nc.sync.dma_start(
                    outT.rearrange("(e p) q -> p e q", p=128)[:, :, 0:QS],
                    out_sb[:])

    nc.compile()
    _cache[key] = nc
    return nc


def _prep(x, mem, mask, cos_q, sin_q, cos_k, sin_k,
          Wq, bq, Wk, bk, Wv, bv, Wo, bo, gq, gk):
    import ml_dtypes
    f = np.float32
    bf = ml_dtypes.bfloat16
    x = np.asarray(x, f).reshape(SQ, DIM)
    mem = np.asarray(mem, f).reshape(SK, DIM)
    mask = np.asarray(mask).reshape(SK)
    cos_q = np.asarray(cos_q, f)
    sin_q = np.asarray(sin_q, f)
    cos_k = np.asarray(cos_k, f)
    sin_k = np.asarray(sin_k, f)
    Wq, Wk, Wv, Wo = (np.asarray(w, f) for w in (Wq, Wk, Wv, Wo))
    bq, bk, bv, bo, gq, gk = (np.asarray(v, f) for v in (bq, bk, bv, bo, gq, gk))

    if not np.allclose(gk, 1.0):
        gkp = gk.reshape(-1, 2)
        assert np.allclose(gkp[:, 0], gkp[:, 1]), "unsupported non-pairwise gk"

    flags = tuple(bool(np.any(v != 0.0)) for v in (bq, bk, bv, bo))

    # compact keys: keep only unmasked, shard evenly, pad per-core to 512
    idx = np.flatnonzero(mask)
    keep = len(idx)
    percore = -(-keep // N_CORES)                # ceil
    pc = max(512, -(-percore // 512) * 512)      # pad to multiple of 512
    nt = pc // 128

    def tile_w(WT):  # [1024,1024] (in,out of W.T) -> [p, o, i, m]
        return np.ascontiguousarray(
            WT.reshape(8, 128, 8, 128).transpose(1, 2, 0, 3)).astype(bf)

    ii = np.arange(128)
    jj = ii // 2
    partner = ii ^ 1

    # fold gq (and pairwise gk) into the q rope tables; sin pairs with
    # partner's gq
    gq_t = (gq * gk).reshape(8, 128)
    gq_sin = (gq.reshape(8, 128)[:, partner] * gk.reshape(8, 128))
    cq = cos_q[:, jj].T                # [128, SQ]
    sq = sin_q[:, jj].T
    ctq = np.ascontiguousarray(
        (cq[None, :, :] * gq_t[:, :, None]).transpose(1, 0, 2)).astype(bf)
    stq = np.ascontiguousarray(
        (sq[None, :, :] * gq_sin[:, :, None]).transpose(1, 0, 2)).astype(bf)

    PT = np.zeros((128, 128), f)
    even = ii[ii % 2 == 0]
    PT[even + 1, even] = -1.0
    PT[even, even + 1] = 1.0

    ec = np.zeros((128, 8, 8), f)
    for h in range(8):
        ec[:, h, h] = 1.0

    shared = {
        "xT": np.ascontiguousarray(x.T).astype(bf),
        "wq": tile_w(Wq.T), "wk": tile_w(Wk.T), "wo": tile_w(Wo.T),
        "wv": np.ascontiguousarray(
            Wv.T.reshape(8, 128, DIM).transpose(1, 0, 2)).astype(bf),
        "ctq": ctq, "stq": stq,
        "bq_t": np.ascontiguousarray(bq.reshape(8, 128).T),
        "bk_t": np.ascontiguousarray(bk.reshape(8, 128).T),
        "bo_t": np.ascontiguousarray(bo.reshape(8, 128).T),
        "bv_t": np.ascontiguousarray(np.tile(bv, (128, 1))),
        "pmat": PT.astype(bf),
        "ones_c": np.ones((128, 1), bf),
        "ones_r": np.ones((1, 128), f),
        "ecols": ec.astype(bf),
    }

    # global compacted + padded arrays
    tot = N_CORES * pc
    memT_full = np.zeros((DIM, tot), bf)
    ctk_full = np.zeros((HD, tot), bf)
    stk_full = np.zeros((HD, tot), bf)
    mb_full = np.full(tot, NEG, f)

    ck = cos_k[:, jj].T.astype(f)   # [128, SK]
    sk_t = sin_k[:, jj].T.astype(f)
    counts = [keep // N_CORES + (1 if c < keep % N_CORES else 0)
              for c in range(N_CORES)]
    off = 0
    for c in range(N_CORES):
        sl = idx[off:off + counts[c]]
        off += counts[c]
        d0 = c * pc
        memT_full[:, d0:d0 + len(sl)] = mem[sl].T.astype(bf)
        ctk_full[:, d0:d0 + len(sl)] = ck[:, sl].astype(bf)
        stk_full[:, d0:d0 + len(sl)] = sk_t[:, sl].astype(bf)
        mb_full[d0:d0 + len(sl)] = 0.0
        assert pc - len(sl) < 512, "padding must fit in last 4 tiles"

    in_maps = []
    for c in range(N_CORES):
        s = slice(c * pc, (c + 1) * pc)
        m = dict(shared)
        m["memT"] = np.ascontiguousarray(memT_full[:, s])
        m["ctk"] = np.ascontiguousarray(ctk_full[:, s])
        m["stk"] = np.ascontiguousarray(stk_full[:, s])
        # bias columns for the last 4 tiles only
        mb = mb_full[s][-512:].reshape(4, 128).T
        m["mbias"] = np.ascontiguousarray(mb)
        in_maps.append(m)
    return in_maps, nt, flags


def kernel(**inputs):
    from concourse.bass_utils import run_bass_kernel_spmd
    in_maps, nt, flags = _prep(**inputs)
    nc = _build(nt, flags)
    res = run_bass_kernel_spmd(nc, in_maps, list(range(N_CORES)))
    parts = [res.results[c]["outT"][:, 0:QS].T for c in range(N_CORES)]
    out = np.concatenate(parts, axis=0)
    return out[None].astype(np.float32)


# revision 37
# speedup vs baseline: 1.0308x; 1.0025x over previous
"""Trainium2 Bass kernel for nn_MemoryRetriever (cross-attention memory retriever).

v2: mask-compacted keys.  The per-key boolean mask keeps ~half of the 31290
memory tokens; the host gathers only surviving keys (plus their RoPE table
columns) and shards them across the 8 cores (PC keys/core, padded to a
multiple of 512; padding is masked via a -1e30 exp bias confined to the last
4 key tiles).  Each core computes K/V projections + RMSNorm + 3D-RoPE for its
shard, full Q (replicated, small), local masked-softmax partials
(un-normalized numerator + denominator), then one AllReduce combines partials
and each core output-projects its own 64-query slice.

Engine balance (vs v1): all elementwise math in bf16 (2x DVE mode); the
per-key RMSNorm scale and 1/sqrt(hd) are folded into the rope cos/sin tables
so Exp runs with scalar scale/bias over 2-tile-wide PSUM pairs; sum-of-squares
runs as near-free ones-stationary PE matmuls; attnV accumulates in PSUM across
all key tiles per 2-head group and is DMA'd straight from PSUM into the
collective buffer; softmax denominators tree-reduce on DVE in bf16 and pack
into one PSUM bank via indicator-column matmuls.
"""

import sys
import numpy as np

sys.path.insert(0, "/opt/trn_rl_repo")

DIM = 1024
HEADS = 8
HD = 128
SQ = 512
SK = 31290
N_CORES = 8
QS = SQ // N_CORES
EPS = 1e-6
NEG = -1.0e30

_cache = {}


def _build(nt, flags, pair=True):
    key = ("nc", nt, flags, pair)
    if key in _cache:
        return _cache[key]

    import concourse.bass as bass
    import concourse.tile as tile
    from concourse import mybir, bacc

    f32 = mybir.dt.float32
    bf16 = mybir.dt.bfloat16
    AF = mybir.ActivationFunctionType
    has_bq, has_bk, has_bv, has_bo = flags

    pc = nt * 128          # keys per core (padded)
    nblk = nt // 4         # 512-key blocks
    npair = nt // 2        # tile pairs (wide exp)

    nc = bacc.Bacc("TRN2", target_bir_lowering=False, debug=False,
                   num_devices=N_CORES)

    def din(name, shape, dt=f32):
        return nc.dram_tensor(name, list(shape), dt, kind="ExternalInput").ap()

    # per-core sharded inputs
    memT = din("memT", [DIM, pc], bf16)      # compacted mem shard, feature-major
    ctk = din("ctk", [HD, pc], bf16)         # K rope cos (pair-major rows)
    stk = din("stk", [HD, pc], bf16)
    mbias = din("mbias", [128, 4])           # pad bias (0/-1e30) for last 4 tiles
    # shared inputs
    xT = din("xT", [DIM, SQ], bf16)
    wq = din("wq", [128, 8, 8, 128], bf16)   # [p,i,o,m] = Wq.T[i*128+p, o*128+m]
    wk = din("wk", [128, 8, 8, 128], bf16)
    wo = din("wo", [128, 8, 8, 128], bf16)
    wv = din("wv", [128, 8, DIM], bf16)      # [p,i,o] = Wv.T[i*128+p, o]
    ctq = din("ctq", [128, 8, SQ], bf16)     # q rope cos (gq*gk folded, per head)
    stq = din("stq", [128, 8, SQ], bf16)
    bq_t = din("bq_t", [128, 8])
    bk_t = din("bk_t", [128, 8])
    bo_t = din("bo_t", [128, 8])
    bv_t = din("bv_t", [128, DIM])
    pmat = din("pmat", [128, 128], bf16)     # P.T for rope pair swap (+-1)
    ones_c = din("ones_c", [128, 1], bf16)
    ones_r = din("ones_r", [1, 128])
    ecols = din("ecols", [128, 8, 8], bf16)  # ecols[p,h,m] = (m==h)

    outT = nc.dram_tensor("outT", [DIM, SQ], f32, kind="ExternalOutput").ap()

    import os as _os
    _sim = _os.environ.get("KSIM", "0") == "1"
    cat = nc.dram_tensor("cat", [DIM + HEADS, SQ], bf16)
    cat_sh = nc.dram_tensor("cat_sh", [DIM + HEADS, SQ], bf16,
                            addr_space="Shared")

    with tile.TileContext(nc) as tc:
        with tc.tile_pool(name="consts", bufs=1) as consts, \
             tc.tile_pool(name="resid", bufs=1) as resid:

            # ---- resident tensors; DMAs issued in first-use order ----
            wk0_s = resid.tile([128, 1, 8, 128], bf16)
            wkr_s = resid.tile([128, 7, 8, 128], bf16)
            memt_blks = [resid.tile([128, 8, 512], bf16, tag=f"memt{b}",
                                    name=f"memt{b}")
                         for b in range(2)]
            wqo_s = resid.tile([128, 8, 8, 128], bf16)  # wq then wo
            xt_s = resid.tile([128, 8, SQ], bf16)
            wv_s = resid.tile([128, 8, DIM], bf16)
            kr = resid.tile([128, 8, pc], bf16)      # rope'd K (unnormalized)
            qT = resid.tile([128, 8, SQ], bf16)      # rope'd+normalized Q
            nsum = resid.tile([128, 8, SQ], bf16)    # numerator accumulators
            dsums = resid.tile([128, 8, SQ], bf16)   # exp-sum per head
            rs_s = resid.tile([128, nt], f32)        # per-key rms scale

            nc.sync.dma_start(memt_blks[0][:],
                              memT[:, 0:512].rearrange("(i p) t -> p i t",
                                                       p=128))
            nc.sync.dma_start(wk0_s[:], wk[:, 0:1, :, :])
            nc.sync.dma_start(wkr_s[:], wk[:, 1:8, :, :])
            # ---- small constants (cheap DMAs, SP queue) ----
            pt_s = consts.tile([128, 128], bf16)
            nc.sync.dma_start(pt_s[:], pmat)
            ones_s = consts.tile([128, 1], bf16)
            nc.sync.dma_start(ones_s[:], ones_c)
            ones_rs = consts.tile([1, 128], f32)
            nc.sync.dma_start(ones_rs[:], ones_r)
            mb_s = consts.tile([128, 4], f32)
            nc.sync.dma_start(mb_s[:], mbias)
            ec_s = consts.tile([128, 8, 8], bf16)
            nc.sync.dma_start(ec_s[:], ecols)
            zcol = consts.tile([128, 1], f32)
            nc.vector.memset(zcol[:], 0.0)
            ep128 = consts.tile([1, 1], f32)
            nc.vector.memset(ep128[:], 128.0 * EPS)
            epsk = consts.tile([128, 1], f32)
            nc.vector.memset(epsk[:], 128.0 * EPS)
            epsq = consts.tile([1, 1], f32)
            nc.vector.memset(epsq[:], EPS)
            if has_bq:
                bq_s = consts.tile([128, 8], f32)
                nc.sync.dma_start(bq_s[:], bq_t)
            if has_bk:
                bk_s = consts.tile([128, 8], f32)
                nc.sync.dma_start(bk_s[:], bk_t)
            if has_bo:
                bo_s = consts.tile([128, 8], f32)
                nc.sync.dma_start(bo_s[:], bo_t)
            if has_bv:
                bv_s = consts.tile([128, DIM], f32)
                nc.sync.dma_start(bv_s[:], bv_t)


            # ============ phase A: K-proj + rope per 512-key block ============
            with tc.tile_pool(name="kpool", bufs=2) as kpool, \
                 tc.tile_pool(name="pp_kv", bufs=2, space="PSUM") as pp_kv, \
                 tc.tile_pool(name="pp_sw", bufs=2, space="PSUM") as pp_sw, \
                 tc.tile_pool(name="pp_rs", bufs=2, space="PSUM") as pp_rs:
                ctk_s = kpool.tile([128, pc], bf16, tag="ctk", bufs=1)
                nc.sync.dma_start(ctk_s[:], ctk)
                stk_s = kpool.tile([128, pc], bf16, tag="stk", bufs=1)
                nc.sync.dma_start(stk_s[:], stk)
                nc.sync.dma_start(memt_blks[1][:],
                                  memT[:, 512:1024].rearrange(
                                      "(i p) t -> p i t", p=128))
                nc.sync.dma_start(wv_s[:], wv)
                nc.sync.dma_start(xt_s[:],
                                  xT.rearrange("(i p) q -> p i q", p=128))
                nc.sync.dma_start(wqo_s[:], wq)
                for b in range(nblk):
                    c0 = b * 512
                    memt = memt_blks[b % 2]
                    if b >= 2:
                        nc.sync.dma_start(
                            memt[:],
                            memT[:, c0:c0 + 512].rearrange("(i p) t -> p i t",
                                                           p=128))
                    yk = kpool.tile([128, 8, 512], bf16, tag="yk")
                    sw = kpool.tile([128, 8, 512], bf16, tag="sw")
                    if pair:
                        ps_rs = pp_rs.tile([1, 512], f32, tag="psrs",
                                           name="ps_rs")
                    else:
                        ps_rs = pp_rs.tile([128, 4], f32, tag="psrs",
                                           name="ps_rs")
                    for o in range(8):
                        ps_y = pp_kv.tile([128, 512], f32, tag="ps")
                        for i in range(8):
                            wsl = (wk0_s[:, 0, i, :] if o == 0
                                   else wkr_s[:, o - 1, i, :])
                            nc.tensor.matmul(ps_y[:], wsl,
                                             memt[:, i, :],
                                             start=(i == 0), stop=(i == 7))
                        if has_bk:
                            nc.scalar.activation(yk[:, o, :], ps_y[:],
                                                 AF.Identity,
                                                 bias=bk_s[:, o:o + 1])
                        else:
                            nc.scalar.activation(yk[:, o, :], ps_y[:], AF.Copy)
                        ps_sw = pp_sw.tile([128, 512], f32, tag="ps")
                        nc.tensor.matmul(ps_sw[:], pt_s[:], yk[:, o, :])
                        nc.scalar.activation(sw[:, o, :], ps_sw[:], AF.Copy)
                        ysq = kpool.tile([128, 512], bf16, tag="ysqo", bufs=2)
                        nc.vector.tensor_mul(ysq[:], yk[:, o, :], yk[:, o, :])
                        if pair:
                            nc.tensor.matmul(ps_rs[:], ones_s[:], ysq[:],
                                             start=(o == 0), stop=(o == 7))
                        else:
                            for tt in range(4):
                                nc.tensor.matmul(
                                    ps_rs[:, tt:tt + 1],
                                    ysq[:, tt * 128:(tt + 1) * 128], ones_s[:],
                                    start=(o == 0), stop=(o == 7))
                    # rs = (1/sqrt(HD))/sqrt(ms+eps) = 1/sqrt(sum/8+128eps)
                    if pair:
                        rroot = kpool.tile([1, 512], f32, tag="rroot")
                        nc.scalar.activation(rroot[:], ps_rs[:], AF.Sqrt,
                                             bias=ep128[:], scale=0.125)
                        rr = kpool.tile([1, 512], f32, tag="rr")
                        nc.vector.reciprocal(rr[:], rroot[:])
                        rsb = kpool.tile([128, 512], f32, tag="rsb")
                        nc.gpsimd.partition_broadcast(rsb[:], rr[:])
                        cts = kpool.tile([128, 512], bf16, tag="cts")
                        nc.vector.tensor_mul(cts[:], ctk_s[:, c0:c0 + 512],
                                             rsb[:])
                        sts = kpool.tile([128, 512], bf16, tag="sts")
                        nc.vector.tensor_mul(sts[:], stk_s[:, c0:c0 + 512],
                                             rsb[:])
                    else:
                        rroot = kpool.tile([128, 4], f32, tag="rroot")
                        nc.scalar.activation(rroot[:], ps_rs[:], AF.Sqrt,
                                             bias=epsk[:], scale=0.125)
                        nc.vector.reciprocal(rs_s[:, b * 4:b * 4 + 4],
                                             rroot[:])
                        cts = ctk_s[:, c0:c0 + 512]
                        sts = stk_s[:, c0:c0 + 512]
                    for o in range(8):
                        t1 = kpool.tile([128, 512], bf16, tag="t1")
                        nc.vector.tensor_mul(t1[:], yk[:, o, :], cts[:])
                        t2 = kpool.tile([128, 512], bf16, tag="t2")
                        nc.vector.tensor_mul(t2[:], sw[:, o, :], sts[:])
                        nc.vector.tensor_add(kr[:, o, c0:c0 + 512], t1[:], t2[:])
                    if b >= 2:
                        # refill this ring slot for phase B (blocks 0/1)
                        c2 = (b - 2) * 512
                        nc.sync.dma_start(
                            memt[:],
                            memT[:, c2:c2 + 512].rearrange("(i p) t -> p i t",
                                                           p=128))

            # ========= phase Q + B (shared scope: no drain between) =========
            with tc.tile_pool(name="qlate", bufs=1) as qlate, \
                 tc.tile_pool(name="bpool", bufs=2) as bpool, \
                 tc.tile_pool(name="ptsp", bufs=3) as ptsp, \
                 tc.tile_pool(name="pp_v", bufs=3, space="PSUM") as pp_v, \
                 tc.tile_pool(name="pp_sc", bufs=2, space="PSUM") as pp_sc, \
                 tc.tile_pool(name="pp_qrs", bufs=1, space="PSUM") as pp_qrs:
                qtab_cm = tc.tile_pool(name="qtab", bufs=1)
                qtab = qtab_cm.__enter__()
                ctq_s = qtab.tile([128, 8, SQ], bf16)
                nc.sync.dma_start(ctq_s[:], ctq)
                stq_s = qtab.tile([128, 8, SQ], bf16)
                nc.sync.dma_start(stq_s[:], stq)
                yq = qlate.tile([128, 8, SQ], bf16)
                swq = qlate.tile([128, 8, SQ], bf16)

                ps_rsq = pp_qrs.tile([1, SQ], f32)
                for o in range(8):
                    ps2q = pp_sc.tile([128, 2, 512], f32, tag="ps2")
                    for i in range(8):
                        nc.tensor.matmul(ps2q[:, 0, :], wqo_s[:, o, i, :],
                                         xt_s[:, i, :],
                                         start=(i == 0), stop=(i == 7))
                    if has_bq:
                        nc.scalar.activation(yq[:, o, :], ps2q[:, 0, :],
                                             AF.Identity,
                                             bias=bq_s[:, o:o + 1])
                    else:
                        nc.scalar.activation(yq[:, o, :], ps2q[:, 0, :],
                                             AF.Copy)
                    nc.tensor.matmul(ps2q[:, 1, :], pt_s[:], yq[:, o, :])
                    nc.scalar.activation(swq[:, o, :], ps2q[:, 1, :], AF.Copy)
                    ysq = qlate.tile([128, SQ], bf16, tag="ysqq", bufs=2)
                    nc.vector.tensor_mul(ysq[:], yq[:, o, :], yq[:, o, :])
                    nc.tensor.matmul(ps_rsq[:], ones_s[:], ysq[:],
                                     start=(o == 0), stop=(o == 7))
                # V-proj helpers (phase B work)
                def v_chain(b, vt, tt, oh):
                    memt = memt_blks[b % 2]
                    ps_v = pp_v.tile([128, 512], f32, tag="ps", name="ps_v")
                    for i in range(8):
                        nc.tensor.matmul(
                            ps_v[:], memt[:, i, tt * 128:(tt + 1) * 128],
                            wv_s[:, i, oh * 512:(oh + 1) * 512],
                            start=(i == 0), stop=(i == 7))
                    if has_bv:
                        nc.vector.tensor_add(
                            vt[:, tt, oh * 512:(oh + 1) * 512], ps_v[:],
                            bv_s[:, oh * 512:(oh + 1) * 512])
                    elif (tt + oh) % 2 == 0:
                        nc.scalar.activation(
                            vt[:, tt, oh * 512:(oh + 1) * 512], ps_v[:],
                            AF.Copy)
                    else:
                        nc.vector.tensor_copy(
                            vt[:, tt, oh * 512:(oh + 1) * 512], ps_v[:])

                def v_proj(b, vt):
                    for tt in range(4):
                        for oh in range(2):
                            v_chain(b, vt, tt, oh)

                vts = [None] * nblk
                for b in range(2):
                    vts[b] = bpool.tile([128, 4, DIM], bf16, tag="vt",
                                        name=f"vt{b}", bufs=3)
                    v_proj(b, vts[b])
                    if b + 2 < nblk:
                        c2 = (b + 2) * 512
                        nc.sync.dma_start(
                            memt_blks[b % 2][:],
                            memT[:, c2:c2 + 512].rearrange("(i p) t -> p i t",
                                                           p=128))
                for b in range(2, nblk):
                    vts[b] = bpool.tile([128, 4, DIM], bf16, tag="vt",
                                        name=f"vt{b}", bufs=3)
                # finish Q: rmsnorm scale + rope (wide in-place DVE ops)
                rrootq = qlate.tile([1, SQ], f32)
                nc.scalar.activation(rrootq[:], ps_rsq[:], AF.Sqrt,
                                     bias=epsq[:], scale=1.0 / DIM)
                nc.vector.reciprocal(rrootq[:], rrootq[:])
                rsbf = qlate.tile([128, SQ], f32)
                nc.gpsimd.partition_broadcast(rsbf[:], rrootq[:])
                nc.vector.tensor_mul(yq[:], yq[:], ctq_s[:])
                nc.vector.tensor_mul(swq[:], swq[:], stq_s[:])
                nc.vector.tensor_add(yq[:], yq[:], swq[:])
                for o in range(8):
                    nc.vector.tensor_mul(qT[:, o, :], yq[:, o, :], rsbf[:])
                qtab_cm.__exit__(None, None, None)

                # ---- attention blocks (V-proj for b+2 interleaved) ----
                for b in range(nblk):
                    vt = vts[b]
                    for h in range(8):
                        pts = ptsp.tile([128, 4, SQ], bf16, tag="pts")
                        for tp in range(2):
                            ps2 = pp_sc.tile([128, 2, 512], f32, tag="ps2")
                            for half in range(2):
                                lt = 2 * tp + half
                                gt = b * 4 + lt
                                nc.tensor.matmul(
                                    ps2[:, half, :],
                                    kr[:, h, gt * 128:(gt + 1) * 128],
                                    qT[:, h, :])
                            if not pair:
                                for half in range(2):
                                    lt = 2 * tp + half
                                    gt = b * 4 + lt
                                    bias_ap = (mb_s[:, lt:lt + 1]
                                               if b == nblk - 1 else zcol[:])
                                    nc.scalar.activation(
                                        pts[:, lt, :], ps2[:, half, :],
                                        AF.Exp, bias=bias_ap,
                                        scale=rs_s[:, gt:gt + 1])
                            elif b == nblk - 1:
                                for half in range(2):
                                    lt = 2 * tp + half
                                    nc.scalar.activation(
                                        pts[:, lt, :], ps2[:, half, :],
                                        AF.Exp, bias=mb_s[:, lt:lt + 1])
                            else:
                                nc.scalar.activation(
                                    pts[:, 2 * tp:2 * tp + 2, :], ps2[:],
                                    AF.Exp, bias=zcol[:])
                        ps_a = pp_v.tile([128, SQ], f32, tag="ps")
                        for tt in range(4):
                            nc.tensor.matmul(
                                ps_a[:], vt[:, tt, h * 128:(h + 1) * 128],
                                pts[:, tt, :], start=(tt == 0), stop=(tt == 3))
                        if b == 0:
                            nc.vector.tensor_copy(nsum[:, h, :], ps_a[:])
                        else:
                            nc.vector.tensor_add(nsum[:, h, :], nsum[:, h, :],
                                                 ps_a[:])
                        # exp-sum partial for this block (bf16 tree)
                        nc.vector.tensor_add(pts[:, 0:2, :], pts[:, 0:2, :],
                                             pts[:, 2:4, :])
                        if b == 0:
                            nc.gpsimd.tensor_add(dsums[:, h, :], pts[:, 0, :],
                                                 pts[:, 1, :])
                        else:
                            nc.vector.tensor_add(pts[:, 0, :], pts[:, 0, :],
                                                 pts[:, 1, :])
                            nc.gpsimd.tensor_add(dsums[:, h, :], dsums[:, h, :],
                                                 pts[:, 0, :])
                        if b == nblk - 1 and h in (3, 5):
                            h0, h1 = (0, 4) if h == 3 else (4, 6)
                            nc.sync.dma_start(
                                cat[h0 * 128:h1 * 128, :].rearrange(
                                    "(h p) q -> p h q", p=128),
                                nsum[:, h0:h1, :])
                            if _sim:
                                nc.gpsimd.dma_start(
                                    cat_sh[h0 * 128:h1 * 128, :],
                                    cat[h0 * 128:h1 * 128, :])
                    if b + 2 < nblk:
                        v_proj(b + 2, vts[b + 2])
                nc.sync.dma_start(
                    cat[768:DIM, :].rearrange("(h p) q -> p h q", p=128),
                    nsum[:, 6:8, :])
                # wo into the wq buffer for the tail
                nc.sync.dma_start(wqo_s[:], wo)

            with tc.tile_pool(name="dpool", bufs=1) as dpool, \
                 tc.tile_pool(name="pp_dn", bufs=1, space="PSUM") as pp_dn:
                ps_den = pp_dn.tile([8, SQ], f32)
                for h in range(8):
                    nc.tensor.matmul(ps_den[:], ec_s[:, h, :], dsums[:, h, :],
                                     start=(h == 0), stop=(h == 7))
                dcp = dpool.tile([8, SQ], bf16)
                nc.vector.tensor_copy(dcp[:], ps_den[:])
                nc.sync.dma_start(cat[DIM:DIM + HEADS, :], dcp[:])

            if _sim:
                nc.gpsimd.dma_start(cat_sh[768:DIM + HEADS, :],
                                    cat[768:DIM + HEADS, :])
            else:
                nc.gpsimd.collective_compute(
                    "AllReduce", mybir.AluOpType.add,
                    replica_groups=[list(range(N_CORES))],
                    ins=[cat[:]], outs=[cat_sh[:]])

            # ========== per-core output projection on its query slice ==========
            with tc.tile_pool(name="tail", bufs=1) as tail, \
                 tc.tile_pool(name="pp_t", bufs=2, space="PSUM") as pp_t:
                nred = tail.tile([128, 8, QS], bf16)
                dred = tail.tile([1, 8, QS], bf16)
                pid = nc.sync.partition_id()
                qoff = pid * QS
                nc.sync.dma_start(
                    nred[:],
                    cat_sh[0:DIM, bass.ds(qoff, QS)].rearrange(
                        "(h p) q -> p h q", p=128))
                nc.sync.dma_start(
                    dred[:],
                    cat_sh[DIM:DIM + HEADS, bass.ds(qoff, QS)].rearrange(
                        "(o h) q -> o h q", o=1))
                rd = tail.tile([1, 8 * QS], f32)
                nc.vector.reciprocal(rd[:], dred.rearrange("o h q -> o (h q)")[:])
                ps_rd = pp_t.tile([128, 8 * QS], f32, tag="psrd")
                nc.tensor.matmul(ps_rd[:], ones_rs[:], rd[:])
                nsc = tail.tile([128, 8, QS], bf16)
                nc.vector.tensor_mul(nsc.rearrange("p h q -> p (h q)")[:],
                                     nred.rearrange("p h q -> p (h q)")[:],
                                     ps_rd[:])
                out_sb = tail.tile([128, 8, QS], f32)
                for e in range(8):
                    ps_o = pp_t.tile([128, QS], f32, tag="ps")
                    for o in range(8):
                        nc.tensor.matmul(ps_o[:], wqo_s[:, e, o, :],
                                         nsc[:, o, :],
                                         start=(o == 0), stop=(o == 7))
                    if has_bo:
                        nc.scalar.activation(out_sb[:, e, :], ps_o[:],
                                             AF.Identity, bias=bo_s[:, e:e + 1])
                    else:
                        nc.scalar.activation(out_sb[:, e, :], ps_o[:], AF.Copy)
                nc.sync.dma_start(
                    outT.rearrange("(e p) q -> p e q", p=128)[:, :, 0:QS],
                    out_sb[:])

    nc.compile()
    _cache[key] = nc
    return nc


def _prep(x, mem, mask, cos_q, sin_q, cos_k, sin_k,
          Wq, bq, Wk, bk, Wv, bv, Wo, bo, gq, gk):
    import ml_dtypes
    f = np.float32
    bf = ml_dtypes.bfloat16
    x = np.asarray(x, f).reshape(SQ, DIM)
    mem = np.asarray(mem, f).reshape(SK, DIM)
    mask = np.asarray(mask).reshape(SK)
    cos_q = np.asarray(cos_q, f)
    sin_q = np.asarray(sin_q, f)
    cos_k = np.asarray(cos_k, f)
    sin_k = np.asarray(sin_k, f)
    Wq, Wk, Wv, Wo = (np.asarray(w, f) for w in (Wq, Wk, Wv, Wo))
    bq, bk, bv, bo, gq, gk = (np.asarray(v, f) for v in (bq, bk, bv, bo, gq, gk))

    if not np.allclose(gk, 1.0):
        gkp = gk.reshape(-1, 2)
        assert np.allclose(gkp[:, 0], gkp[:, 1]), "unsupported non-pairwise gk"

    flags = tuple(bool(np.any(v != 0.0)) for v in (bq, bk, bv, bo))

    # compact keys: keep only unmasked, shard evenly, pad per-core to 512
    idx = np.flatnonzero(mask)
    keep = len(idx)
    percore = -(-keep // N_CORES)                # ceil
    pc = max(512, -(-percore // 512) * 512)      # pad to multiple of 512
    nt = pc // 128

    def tile_w(WT):  # [1024,1024] (in,out of W.T) -> [p, o, i, m]
        return np.ascontiguousarray(
            WT.reshape(8, 128, 8, 128).transpose(1, 2, 0, 3)).astype(bf)

    ii = np.arange(128)
    jj = ii // 2
    partner = ii ^ 1

    # fold gq (and pairwise gk) into the q rope tables; sin pairs with
    # partner's gq
    gq_t = (gq * gk).reshape(8, 128)
    gq_sin = (gq.reshape(8, 128)[:, partner] * gk.reshape(8, 128))
    cq = cos_q[:, jj].T                # [128, SQ]
    sq = sin_q[:, jj].T
    ctq = np.ascontiguousarray(
        (cq[None, :, :] * gq_t[:, :, None]).transpose(1, 0, 2)).astype(bf)
    stq = np.ascontiguousarray(
        (sq[None, :, :] * gq_sin[:, :, None]).transpose(1, 0, 2)).astype(bf)

    PT = np.zeros((128, 128), f)
    even = ii[ii % 2 == 0]
    PT[even + 1, even] = -1.0
    PT[even, even + 1] = 1.0

    ec = np.zeros((128, 8, 8), f)
    for h in range(8):
        ec[:, h, h] = 1.0

    shared = {
        "xT": np.ascontiguousarray(x.T).astype(bf),
        "wq": tile_w(Wq.T), "wk": tile_w(Wk.T), "wo": tile_w(Wo.T),
        "wv": np.ascontiguousarray(
            Wv.T.reshape(8, 128, DIM).transpose(1, 0, 2)).astype(bf),
        "ctq": ctq, "stq": stq,
        "bq_t": np.ascontiguousarray(bq.reshape(8, 128).T),
        "bk_t": np.ascontiguousarray(bk.reshape(8, 128).T),
        "bo_t": np.ascontiguousarray(bo.reshape(8, 128).T),
        "bv_t": np.ascontiguousarray(np.tile(bv, (128, 1))),
        "pmat": PT.astype(bf),
        "ones_c": np.ones((128, 1), bf),
        "ones_r": np.ones((1, 128), f),
        "ecols": ec.astype(bf),
    }

    # global compacted + padded arrays
    tot = N_CORES * pc
    memT_full = np.zeros((DIM, tot), bf)
    ctk_full = np.zeros((HD, tot), bf)
    stk_full = np.zeros((HD, tot), bf)
    mb_full = np.full(tot, NEG, f)

    ck = cos_k[:, jj].T.astype(f)   # [128, SK]
    sk_t = sin_k[:, jj].T.astype(f)
    counts = [keep // N_CORES + (1 if c < keep % N_CORES else 0)
              for c in range(N_CORES)]
    off = 0
    for c in range(N_CORES):
        sl = idx[off:off + counts[c]]
        off += counts[c]
        d0 = c * pc
        memT_full[:, d0:d0 + len(sl)] = mem[sl].T.astype(bf)
        ctk_full[:, d0:d0 + len(sl)] = ck[:, sl].astype(bf)
        stk_full[:, d0:d0 + len(sl)] = sk_t[:, sl].astype(bf)
        mb_full[d0:d0 + len(sl)] = 0.0
        assert pc - len(sl) < 512, "padding must fit in last 4 tiles"

    in_maps = []
    for c in range(N_CORES):
        s = slice(c * pc, (c + 1) * pc)
        m = dict(shared)
        m["memT"] = np.ascontiguousarray(memT_full[:, s])
        m["ctk"] = np.ascontiguousarray(ctk_full[:, s])
        m["stk"] = np.ascontiguousarray(stk_full[:, s])
        # bias columns for the last 4 tiles only
        mb = mb_full[s][-512:].reshape(4, 128).T
        m["mbias"] = np.ascontiguousarray(mb)
        in_maps.append(m)
    return in_maps, nt, flags


def kernel(**inputs):
    from concourse.bass_utils import run_bass_kernel_spmd
    in_maps, nt, flags = _prep(**inputs)
    nc = _build(nt, flags)
    res = run_bass_kernel_spmd(nc, in_maps, list(range(N_CORES)))
    parts = [res.results[c]["outT"][:, 0:QS].T for c in range(N_CORES)]
    out = np.concatenate(parts, axis=0)
    return out[None].astype(np.float32)


# revision 38
# speedup vs baseline: 1.0357x; 1.0048x over previous
"""Trainium2 Bass kernel for nn_MemoryRetriever (cross-attention memory retriever).

v2: mask-compacted keys.  The per-key boolean mask keeps ~half of the 31290
memory tokens; the host gathers only surviving keys (plus their RoPE table
columns) and shards them across the 8 cores (PC keys/core, padded to a
multiple of 512; padding is masked via a -1e30 exp bias confined to the last
4 key tiles).  Each core computes K/V projections + RMSNorm + 3D-RoPE for its
shard, full Q (replicated, small), local masked-softmax partials
(un-normalized numerator + denominator), then one AllReduce combines partials
and each core output-projects its own 64-query slice.

Engine balance (vs v1): all elementwise math in bf16 (2x DVE mode); the
per-key RMSNorm scale and 1/sqrt(hd) are folded into the rope cos/sin tables
so Exp runs with scalar scale/bias over 2-tile-wide PSUM pairs; sum-of-squares
runs as near-free ones-stationary PE matmuls; attnV accumulates in PSUM across
all key tiles per 2-head group and is DMA'd straight from PSUM into the
collective buffer; softmax denominators tree-reduce on DVE in bf16 and pack
into one PSUM bank via indicator-column matmuls.
"""

import sys
import numpy as np

sys.path.insert(0, "/opt/trn_rl_repo")

DIM = 1024
HEADS = 8
HD = 128
SQ = 512
SK = 31290
N_CORES = 8
QS = SQ // N_CORES
EPS = 1e-6
NEG = -1.0e30

_cache = {}


def _build(nt, flags, pair=True):
    key = ("nc", nt, flags, pair)
    if key in _cache:
        return _cache[key]

    import concourse.bass as bass
    import concourse.tile as tile
    from concourse import mybir, bacc

    f32 = mybir.dt.float32
    bf16 = mybir.dt.bfloat16
    AF = mybir.ActivationFunctionType
    has_bq, has_bk, has_bv, has_bo = flags

    pc = nt * 128          # keys per core (padded)
    nblk = nt // 4         # 512-key blocks
    npair = nt // 2        # tile pairs (wide exp)

    nc = bacc.Bacc("TRN2", target_bir_lowering=False, debug=False,
                   num_devices=N_CORES)

    def din(name, shape, dt=f32):
        return nc.dram_tensor(name, list(shape), dt, kind="ExternalInput").ap()

    # per-core sharded inputs
    memT = din("memT", [DIM, pc], bf16)      # compacted mem shard, feature-major
    ctk = din("ctk", [HD, pc], bf16)         # K rope cos (pair-major rows)
    stk = din("stk", [HD, pc], bf16)
    mbias = din("mbias", [128, 4])           # pad bias (0/-1e30) for last 4 tiles
    # shared inputs
    xT = din("xT", [DIM, SQ], bf16)
    wq = din("wq", [128, 8, 8, 128], bf16)   # [p,i,o,m] = Wq.T[i*128+p, o*128+m]
    wk = din("wk", [128, 8, 8, 128], bf16)
    wo = din("wo", [128, 8, 8, 128], bf16)
    wv = din("wv", [128, 8, DIM], bf16)      # [p,i,o] = Wv.T[i*128+p, o]
    ctq = din("ctq", [128, 8, SQ], bf16)     # q rope cos (gq*gk folded, per head)
    stq = din("stq", [128, 8, SQ], bf16)
    bq_t = din("bq_t", [128, 8])
    bk_t = din("bk_t", [128, 8])
    bo_t = din("bo_t", [128, 8])
    bv_t = din("bv_t", [128, DIM])
    pmat = din("pmat", [128, 128], bf16)     # P.T for rope pair swap (+-1)
    ones_c = din("ones_c", [128, 1], bf16)
    ones_r = din("ones_r", [1, 128])
    ecols = din("ecols", [128, 8, 8], bf16)  # ecols[p,h,m] = (m==h)

    outT = nc.dram_tensor("outT", [DIM, SQ], f32, kind="ExternalOutput").ap()

    import os as _os
    _sim = _os.environ.get("KSIM", "0") == "1"
    cat = nc.dram_tensor("cat", [DIM + HEADS, SQ], bf16)
    cat_sh = nc.dram_tensor("cat_sh", [DIM + HEADS, SQ], bf16,
                            addr_space="Shared")

    with tile.TileContext(nc) as tc:
        with tc.tile_pool(name="consts", bufs=1) as consts, \
             tc.tile_pool(name="resid", bufs=1) as resid:

            # ---- resident tensors; DMAs issued in first-use order ----
            wk0_s = resid.tile([128, 1, 8, 128], bf16)
            wkr_s = resid.tile([128, 7, 8, 128], bf16)
            memt_blks = [resid.tile([128, 8, 512], bf16, tag=f"memt{b}",
                                    name=f"memt{b}")
                         for b in range(2)]
            wqo_s = resid.tile([128, 8, 8, 128], bf16)  # wq then wo
            xt_s = resid.tile([128, 8, SQ], bf16)
            wv_s = resid.tile([128, 8, DIM], bf16)
            kr = resid.tile([128, 8, pc], bf16)      # rope'd K (unnormalized)
            qT = resid.tile([128, 8, SQ], bf16)      # rope'd+normalized Q
            nsum = resid.tile([128, 8, SQ], bf16)    # numerator accumulators
            dsums = resid.tile([128, 8, SQ], bf16)   # exp-sum per head
            rs_s = resid.tile([128, nt], f32)        # per-key rms scale

            nc.sync.dma_start(memt_blks[0][:],
                              memT[:, 0:512].rearrange("(i p) t -> p i t",
                                                       p=128))
            nc.sync.dma_start(wk0_s[:], wk[:, 0:1, :, :])
            nc.sync.dma_start(wkr_s[:], wk[:, 1:8, :, :])
            # ---- small constants (cheap DMAs, SP queue) ----
            pt_s = consts.tile([128, 128], bf16)
            nc.sync.dma_start(pt_s[:], pmat)
            ones_s = consts.tile([128, 1], bf16)
            nc.sync.dma_start(ones_s[:], ones_c)
            ones_rs = consts.tile([1, 128], f32)
            nc.sync.dma_start(ones_rs[:], ones_r)
            mb_s = consts.tile([128, 4], f32)
            nc.sync.dma_start(mb_s[:], mbias)
            ec_s = consts.tile([128, 8, 8], bf16)
            nc.sync.dma_start(ec_s[:], ecols)
            zcol = consts.tile([128, 1], f32)
            nc.vector.memset(zcol[:], 0.0)
            ep128 = consts.tile([1, 1], f32)
            nc.vector.memset(ep128[:], 128.0 * EPS)
            epsk = consts.tile([128, 1], f32)
            nc.vector.memset(epsk[:], 128.0 * EPS)
            epsq = consts.tile([1, 1], f32)
            nc.vector.memset(epsq[:], EPS)
            if has_bq:
                bq_s = consts.tile([128, 8], f32)
                nc.sync.dma_start(bq_s[:], bq_t)
            if has_bk:
                bk_s = consts.tile([128, 8], f32)
                nc.sync.dma_start(bk_s[:], bk_t)
            if has_bo:
                bo_s = consts.tile([128, 8], f32)
                nc.sync.dma_start(bo_s[:], bo_t)
            if has_bv:
                bv_s = consts.tile([128, DIM], f32)
                nc.sync.dma_start(bv_s[:], bv_t)


            # ============ phase A: K-proj + rope per 512-key block ============
            with tc.tile_pool(name="kpool", bufs=2) as kpool, \
                 tc.tile_pool(name="pp_kv", bufs=2, space="PSUM") as pp_kv, \
                 tc.tile_pool(name="pp_sw", bufs=2, space="PSUM") as pp_sw, \
                 tc.tile_pool(name="pp_rs", bufs=2, space="PSUM") as pp_rs:
                ctk_s = kpool.tile([128, pc], bf16, tag="ctk", bufs=1)
                nc.sync.dma_start(ctk_s[:], ctk)
                stk_s = kpool.tile([128, pc], bf16, tag="stk", bufs=1)
                nc.sync.dma_start(stk_s[:], stk)
                nc.sync.dma_start(memt_blks[1][:],
                                  memT[:, 512:1024].rearrange(
                                      "(i p) t -> p i t", p=128))
                nc.sync.dma_start(wv_s[:], wv)
                nc.sync.dma_start(xt_s[:],
                                  xT.rearrange("(i p) q -> p i q", p=128))
                nc.sync.dma_start(wqo_s[:], wq)
                for b in range(nblk):
                    c0 = b * 512
                    memt = memt_blks[b % 2]
                    if b >= 2:
                        nc.sync.dma_start(
                            memt[:],
                            memT[:, c0:c0 + 512].rearrange("(i p) t -> p i t",
                                                           p=128))
                    yk = kpool.tile([128, 8, 512], bf16, tag="yk")
                    sw = kpool.tile([128, 8, 512], bf16, tag="sw")
                    if pair:
                        ps_rs = pp_rs.tile([1, 512], f32, tag="psrs",
                                           name="ps_rs")
                    else:
                        ps_rs = pp_rs.tile([128, 4], f32, tag="psrs",
                                           name="ps_rs")
                    for o in range(8):
                        ps_y = pp_kv.tile([128, 512], f32, tag="ps")
                        for i in range(8):
                            wsl = (wk0_s[:, 0, i, :] if o == 0
                                   else wkr_s[:, o - 1, i, :])
                            nc.tensor.matmul(ps_y[:], wsl,
                                             memt[:, i, :],
                                             start=(i == 0), stop=(i == 7))
                        if has_bk:
                            nc.scalar.activation(yk[:, o, :], ps_y[:],
                                                 AF.Identity,
                                                 bias=bk_s[:, o:o + 1])
                        else:
                            nc.scalar.activation(yk[:, o, :], ps_y[:], AF.Copy)
                        ps_sw = pp_sw.tile([128, 512], f32, tag="ps")
                        nc.tensor.matmul(ps_sw[:], pt_s[:], yk[:, o, :])
                        nc.scalar.activation(sw[:, o, :], ps_sw[:], AF.Copy)
                        ysq = kpool.tile([128, 512], bf16, tag="ysqo", bufs=2)
                        nc.vector.tensor_mul(ysq[:], yk[:, o, :], yk[:, o, :])
                        if pair:
                            nc.tensor.matmul(ps_rs[:], ones_s[:], ysq[:],
                                             start=(o == 0), stop=(o == 7))
                        else:
                            for tt in range(4):
                                nc.tensor.matmul(
                                    ps_rs[:, tt:tt + 1],
                                    ysq[:, tt * 128:(tt + 1) * 128], ones_s[:],
                                    start=(o == 0), stop=(o == 7))
                    # rs = (1/sqrt(HD))/sqrt(ms+eps) = 1/sqrt(sum/8+128eps)
                    if pair:
                        rroot = kpool.tile([1, 512], f32, tag="rroot")
                        nc.scalar.activation(rroot[:], ps_rs[:], AF.Sqrt,
                                             bias=ep128[:], scale=0.125)
                        rr = kpool.tile([1, 512], f32, tag="rr")
                        nc.vector.reciprocal(rr[:], rroot[:])
                        rsb = kpool.tile([128, 512], f32, tag="rsb")
                        nc.gpsimd.partition_broadcast(rsb[:], rr[:])
                        cts = kpool.tile([128, 512], bf16, tag="cts")
                        nc.vector.tensor_mul(cts[:], ctk_s[:, c0:c0 + 512],
                                             rsb[:])
                        sts = kpool.tile([128, 512], bf16, tag="sts")
                        nc.vector.tensor_mul(sts[:], stk_s[:, c0:c0 + 512],
                                             rsb[:])
                    else:
                        rroot = kpool.tile([128, 4], f32, tag="rroot")
                        nc.scalar.activation(rroot[:], ps_rs[:], AF.Sqrt,
                                             bias=epsk[:], scale=0.125)
                        nc.vector.reciprocal(rs_s[:, b * 4:b * 4 + 4],
                                             rroot[:])
                        cts = ctk_s[:, c0:c0 + 512]
                        sts = stk_s[:, c0:c0 + 512]
                    for o in range(8):
                        t1 = kpool.tile([128, 512], bf16, tag="t1")
                        nc.vector.tensor_mul(t1[:], yk[:, o, :], cts[:])
                        t2 = kpool.tile([128, 512], bf16, tag="t2")
                        nc.vector.tensor_mul(t2[:], sw[:, o, :], sts[:])
                        nc.vector.tensor_add(kr[:, o, c0:c0 + 512], t1[:], t2[:])
                    if b >= 2:
                        # refill this ring slot for phase B (blocks 0/1)
                        c2 = (b - 2) * 512
                        nc.sync.dma_start(
                            memt[:],
                            memT[:, c2:c2 + 512].rearrange("(i p) t -> p i t",
                                                           p=128))

            # ========= phase Q + B (shared scope: no drain between) =========
            with tc.tile_pool(name="qlate", bufs=1) as qlate, \
                 tc.tile_pool(name="bpool", bufs=2) as bpool, \
                 tc.tile_pool(name="ptsp", bufs=4) as ptsp, \
                 tc.tile_pool(name="pp_v", bufs=3, space="PSUM") as pp_v, \
                 tc.tile_pool(name="pp_sc", bufs=2, space="PSUM") as pp_sc, \
                 tc.tile_pool(name="pp_qrs", bufs=1, space="PSUM") as pp_qrs:
                qtab_cm = tc.tile_pool(name="qtab", bufs=1)
                qtab = qtab_cm.__enter__()
                ctq_s = qtab.tile([128, 8, SQ], bf16)
                nc.sync.dma_start(ctq_s[:], ctq)
                stq_s = qtab.tile([128, 8, SQ], bf16)
                nc.sync.dma_start(stq_s[:], stq)
                yq = qlate.tile([128, 8, SQ], bf16)
                swq = qlate.tile([128, 8, SQ], bf16)

                ps_rsq = pp_qrs.tile([1, SQ], f32)
                for o in range(8):
                    ps2q = pp_sc.tile([128, 2, 512], f32, tag="ps2")
                    for i in range(8):
                        nc.tensor.matmul(ps2q[:, 0, :], wqo_s[:, o, i, :],
                                         xt_s[:, i, :],
                                         start=(i == 0), stop=(i == 7))
                    if has_bq:
                        nc.scalar.activation(yq[:, o, :], ps2q[:, 0, :],
                                             AF.Identity,
                                             bias=bq_s[:, o:o + 1])
                    else:
                        nc.scalar.activation(yq[:, o, :], ps2q[:, 0, :],
                                             AF.Copy)
                    nc.tensor.matmul(ps2q[:, 1, :], pt_s[:], yq[:, o, :])
                    nc.scalar.activation(swq[:, o, :], ps2q[:, 1, :], AF.Copy)
                    ysq = qlate.tile([128, SQ], bf16, tag="ysqq", bufs=2)
                    nc.vector.tensor_mul(ysq[:], yq[:, o, :], yq[:, o, :])
                    nc.tensor.matmul(ps_rsq[:], ones_s[:], ysq[:],
                                     start=(o == 0), stop=(o == 7))
                # V-proj helpers (phase B work)
                def v_chain(b, vt, tt, oh):
                    memt = memt_blks[b % 2]
                    ps_v = pp_v.tile([128, 512], f32, tag="ps", name="ps_v")
                    for i in range(8):
                        nc.tensor.matmul(
                            ps_v[:], memt[:, i, tt * 128:(tt + 1) * 128],
                            wv_s[:, i, oh * 512:(oh + 1) * 512],
                            start=(i == 0), stop=(i == 7))
                    if has_bv:
                        nc.vector.tensor_add(
                            vt[:, tt, oh * 512:(oh + 1) * 512], ps_v[:],
                            bv_s[:, oh * 512:(oh + 1) * 512])
                    elif (tt + oh) % 2 == 0:
                        nc.scalar.activation(
                            vt[:, tt, oh * 512:(oh + 1) * 512], ps_v[:],
                            AF.Copy)
                    else:
                        nc.vector.tensor_copy(
                            vt[:, tt, oh * 512:(oh + 1) * 512], ps_v[:])

                def v_proj(b, vt):
                    for tt in range(4):
                        for oh in range(2):
                            v_chain(b, vt, tt, oh)

                vts = [None] * nblk
                for b in range(2):
                    vts[b] = bpool.tile([128, 4, DIM], bf16, tag="vt",
                                        name=f"vt{b}", bufs=3)
                    v_proj(b, vts[b])
                    if b + 2 < nblk:
                        c2 = (b + 2) * 512
                        nc.sync.dma_start(
                            memt_blks[b % 2][:],
                            memT[:, c2:c2 + 512].rearrange("(i p) t -> p i t",
                                                           p=128))
                for b in range(2, nblk):
                    vts[b] = bpool.tile([128, 4, DIM], bf16, tag="vt",
                                        name=f"vt{b}", bufs=3)
                # finish Q: rmsnorm scale + rope (wide in-place DVE ops)
                rrootq = qlate.tile([1, SQ], f32)
                nc.scalar.activation(rrootq[:], ps_rsq[:], AF.Sqrt,
                                     bias=epsq[:], scale=1.0 / DIM)
                nc.vector.reciprocal(rrootq[:], rrootq[:])
                rsbf = qlate.tile([128, SQ], f32)
                nc.gpsimd.partition_broadcast(rsbf[:], rrootq[:])
                nc.vector.tensor_mul(yq[:], yq[:], ctq_s[:])
                nc.vector.tensor_mul(swq[:], swq[:], stq_s[:])
                nc.vector.tensor_add(yq[:], yq[:], swq[:])
                for o in range(8):
                    nc.vector.tensor_mul(qT[:, o, :], yq[:, o, :], rsbf[:])
                qtab_cm.__exit__(None, None, None)

                # ---- attention blocks (V-proj for b+2 interleaved) ----
                for b in range(nblk):
                    vt = vts[b]
                    for h in range(8):
                        pts = ptsp.tile([128, 4, SQ], bf16, tag="pts")
                        for tp in range(2):
                            ps2 = pp_sc.tile([128, 2, 512], f32, tag="ps2")
                            for half in range(2):
                                lt = 2 * tp + half
                                gt = b * 4 + lt
                                nc.tensor.matmul(
                                    ps2[:, half, :],
                                    kr[:, h, gt * 128:(gt + 1) * 128],
                                    qT[:, h, :])
                            if not pair:
                                for half in range(2):
                                    lt = 2 * tp + half
                                    gt = b * 4 + lt
                                    bias_ap = (mb_s[:, lt:lt + 1]
                                               if b == nblk - 1 else zcol[:])
                                    nc.scalar.activation(
                                        pts[:, lt, :], ps2[:, half, :],
                                        AF.Exp, bias=bias_ap,
                                        scale=rs_s[:, gt:gt + 1])
                            elif b == nblk - 1:
                                for half in range(2):
                                    lt = 2 * tp + half
                                    nc.scalar.activation(
                                        pts[:, lt, :], ps2[:, half, :],
                                        AF.Exp, bias=mb_s[:, lt:lt + 1])
                            else:
                                nc.scalar.activation(
                                    pts[:, 2 * tp:2 * tp + 2, :], ps2[:],
                                    AF.Exp, bias=zcol[:])
                        ps_a = pp_v.tile([128, SQ], f32, tag="ps")
                        for tt in range(4):
                            nc.tensor.matmul(
                                ps_a[:], vt[:, tt, h * 128:(h + 1) * 128],
                                pts[:, tt, :], start=(tt == 0), stop=(tt == 3))
                        if b == 0:
                            nc.vector.tensor_copy(nsum[:, h, :], ps_a[:])
                        else:
                            nc.vector.tensor_add(nsum[:, h, :], nsum[:, h, :],
                                                 ps_a[:])
                        # exp-sum partial for this block (bf16 tree)
                        nc.vector.tensor_add(pts[:, 0:2, :], pts[:, 0:2, :],
                                             pts[:, 2:4, :])
                        if b == 0:
                            nc.vector.tensor_add(dsums[:, h, :], pts[:, 0, :],
                                                 pts[:, 1, :])
                        else:
                            nc.vector.tensor_add(pts[:, 0, :], pts[:, 0, :],
                                                 pts[:, 1, :])
                            nc.vector.tensor_add(dsums[:, h, :], dsums[:, h, :],
                                                 pts[:, 0, :])
                        if b == nblk - 1 and h in (3, 5):
                            h0, h1 = (0, 4) if h == 3 else (4, 6)
                            nc.sync.dma_start(
                                cat[h0 * 128:h1 * 128, :].rearrange(
                                    "(h p) q -> p h q", p=128),
                                nsum[:, h0:h1, :])
                            if _sim:
                                nc.gpsimd.dma_start(
                                    cat_sh[h0 * 128:h1 * 128, :],
                                    cat[h0 * 128:h1 * 128, :])
                    if b + 2 < nblk:
                        v_proj(b + 2, vts[b + 2])
                nc.sync.dma_start(
                    cat[768:DIM, :].rearrange("(h p) q -> p h q", p=128),
                    nsum[:, 6:8, :])
                # wo into the wq buffer for the tail
                nc.sync.dma_start(wqo_s[:], wo)

            with tc.tile_pool(name="dpool", bufs=1) as dpool, \
                 tc.tile_pool(name="pp_dn", bufs=1, space="PSUM") as pp_dn:
                ps_den = pp_dn.tile([8, SQ], f32)
                for h in range(8):
                    nc.tensor.matmul(ps_den[:], ec_s[:, h, :], dsums[:, h, :],
                                     start=(h == 0), stop=(h == 7))
                dcp = dpool.tile([8, SQ], bf16)
                nc.vector.tensor_copy(dcp[:], ps_den[:])
                nc.sync.dma_start(cat[DIM:DIM + HEADS, :], dcp[:])

            if _sim:
                nc.gpsimd.dma_start(cat_sh[768:DIM + HEADS, :],
                                    cat[768:DIM + HEADS, :])
            else:
                nc.gpsimd.collective_compute(
                    "AllReduce", mybir.AluOpType.add,
                    replica_groups=[list(range(N_CORES))],
                    ins=[cat[:]], outs=[cat_sh[:]])

            # ========== per-core output projection on its query slice ==========
            with tc.tile_pool(name="tail", bufs=1) as tail, \
                 tc.tile_pool(name="pp_t", bufs=2, space="PSUM") as pp_t:
                nred = tail.tile([128, 8, QS], bf16)
                dred = tail.tile([1, 8, QS], bf16)
                pid = nc.sync.partition_id()
                qoff = pid * QS
                nc.sync.dma_start(
                    nred[:],
                    cat_sh[0:DIM, bass.ds(qoff, QS)].rearrange(
                        "(h p) q -> p h q", p=128))
                nc.sync.dma_start(
                    dred[:],
                    cat_sh[DIM:DIM + HEADS, bass.ds(qoff, QS)].rearrange(
                        "(o h) q -> o h q", o=1))
                rd = tail.tile([1, 8 * QS], f32)
                nc.vector.reciprocal(rd[:], dred.rearrange("o h q -> o (h q)")[:])
                ps_rd = pp_t.tile([128, 8 * QS], f32, tag="psrd")
                nc.tensor.matmul(ps_rd[:], ones_rs[:], rd[:])
                nsc = tail.tile([128, 8, QS], bf16)
                nc.vector.tensor_mul(nsc.rearrange("p h q -> p (h q)")[:],
                                     nred.rearrange("p h q -> p (h q)")[:],
                                     ps_rd[:])
                out_sb = tail.tile([128, 8, QS], f32)
                for e in range(8):
                    ps_o = pp_t.tile([128, QS], f32, tag="ps")
                    for o in range(8):
                        nc.tensor.matmul(ps_o[:], wqo_s[:, e, o, :],
                                         nsc[:, o, :],
                                         start=(o == 0), stop=(o == 7))
                    if has_bo:
                        nc.scalar.activation(out_sb[:, e, :], ps_o[:],
                                             AF.Identity, bias=bo_s[:, e:e + 1])
                    else:
                        nc.scalar.activation(out_sb[:, e, :], ps_o[:], AF.Copy)
                nc.sync.dma_start(
                    outT.rearrange("(e p) q -> p e q", p=128)[:, :, 0:QS],
                    out_sb[:])

    nc.compile()
    _cache[key] = nc
    return nc


def _prep(x, mem, mask, cos_q, sin_q, cos_k, sin_k,
          Wq, bq, Wk, bk, Wv, bv, Wo, bo, gq, gk):
    import ml_dtypes
    f = np.float32
    bf = ml_dtypes.bfloat16
    x = np.asarray(x, f).reshape(SQ, DIM)
    mem = np.asarray(mem, f).reshape(SK, DIM)
    mask = np.asarray(mask).reshape(SK)
    cos_q = np.asarray(cos_q, f)
    sin_q = np.asarray(sin_q, f)
    cos_k = np.asarray(cos_k, f)
    sin_k = np.asarray(sin_k, f)
    Wq, Wk, Wv, Wo = (np.asarray(w, f) for w in (Wq, Wk, Wv, Wo))
    bq, bk, bv, bo, gq, gk = (np.asarray(v, f) for v in (bq, bk, bv, bo, gq, gk))

    if not np.allclose(gk, 1.0):
        gkp = gk.reshape(-1, 2)
        assert np.allclose(gkp[:, 0], gkp[:, 1]), "unsupported non-pairwise gk"

    flags = tuple(bool(np.any(v != 0.0)) for v in (bq, bk, bv, bo))

    # compact keys: keep only unmasked, shard evenly, pad per-core to 512
    idx = np.flatnonzero(mask)
    keep = len(idx)
    percore = -(-keep // N_CORES)                # ceil
    pc = max(512, -(-percore // 512) * 512)      # pad to multiple of 512
    nt = pc // 128

    def tile_w(WT):  # [1024,1024] (in,out of W.T) -> [p, o, i, m]
        return np.ascontiguousarray(
            WT.reshape(8, 128, 8, 128).transpose(1, 2, 0, 3)).astype(bf)

    ii = np.arange(128)
    jj = ii // 2
    partner = ii ^ 1

    # fold gq (and pairwise gk) into the q rope tables; sin pairs with
    # partner's gq
    gq_t = (gq * gk).reshape(8, 128)
    gq_sin = (gq.reshape(8, 128)[:, partner] * gk.reshape(8, 128))
    cq = cos_q[:, jj].T                # [128, SQ]
    sq = sin_q[:, jj].T
    ctq = np.ascontiguousarray(
        (cq[None, :, :] * gq_t[:, :, None]).transpose(1, 0, 2)).astype(bf)
    stq = np.ascontiguousarray(
        (sq[None, :, :] * gq_sin[:, :, None]).transpose(1, 0, 2)).astype(bf)

    PT = np.zeros((128, 128), f)
    even = ii[ii % 2 == 0]
    PT[even + 1, even] = -1.0
    PT[even, even + 1] = 1.0

    ec = np.zeros((128, 8, 8), f)
    for h in range(8):
        ec[:, h, h] = 1.0

    shared = {
        "xT": np.ascontiguousarray(x.T).astype(bf),
        "wq": tile_w(Wq.T), "wk": tile_w(Wk.T), "wo": tile_w(Wo.T),
        "wv": np.ascontiguousarray(
            Wv.T.reshape(8, 128, DIM).transpose(1, 0, 2)).astype(bf),
        "ctq": ctq, "stq": stq,
        "bq_t": np.ascontiguousarray(bq.reshape(8, 128).T),
        "bk_t": np.ascontiguousarray(bk.reshape(8, 128).T),
        "bo_t": np.ascontiguousarray(bo.reshape(8, 128).T),
        "bv_t": np.ascontiguousarray(np.tile(bv, (128, 1))),
        "pmat": PT.astype(bf),
        "ones_c": np.ones((128, 1), bf),
        "ones_r": np.ones((1, 128), f),
        "ecols": ec.astype(bf),
    }

    # global compacted + padded arrays
    tot = N_CORES * pc
    memT_full = np.zeros((DIM, tot), bf)
    ctk_full = np.zeros((HD, tot), bf)
    stk_full = np.zeros((HD, tot), bf)
    mb_full = np.full(tot, NEG, f)

    ck = cos_k[:, jj].T.astype(f)   # [128, SK]
    sk_t = sin_k[:, jj].T.astype(f)
    counts = [keep // N_CORES + (1 if c < keep % N_CORES else 0)
              for c in range(N_CORES)]
    off = 0
    for c in range(N_CORES):
        sl = idx[off:off + counts[c]]
        off += counts[c]
        d0 = c * pc
        memT_full[:, d0:d0 + len(sl)] = mem[sl].T.astype(bf)
        ctk_full[:, d0:d0 + len(sl)] = ck[:, sl].astype(bf)
        stk_full[:, d0:d0 + len(sl)] = sk_t[:, sl].astype(bf)
        mb_full[d0:d0 + len(sl)] = 0.0
        assert pc - len(sl) < 512, "padding must fit in last 4 tiles"

    in_maps = []
    for c in range(N_CORES):
        s = slice(c * pc, (c + 1) * pc)
        m = dict(shared)
        m["memT"] = np.ascontiguousarray(memT_full[:, s])
        m["ctk"] = np.ascontiguousarray(ctk_full[:, s])
        m["stk"] = np.ascontiguousarray(stk_full[:, s])
        # bias columns for the last 4 tiles only
        mb = mb_full[s][-512:].reshape(4, 128).T
        m["mbias"] = np.ascontiguousarray(mb)
        in_maps.append(m)
    return in_maps, nt, flags


def kernel(**inputs):
    from concourse.bass_utils import run_bass_kernel_spmd
    in_maps, nt, flags = _prep(**inputs)
    nc = _build(nt, flags)
    res = run_bass_kernel_spmd(nc, in_maps, list(range(N_CORES)))
    parts = [res.results[c]["outT"][:, 0:QS].T for c in range(N_CORES)]
    out = np.concatenate(parts, axis=0)
    return out[None].astype(np.float32)


# revision 40
# speedup vs baseline: 1.0709x; 1.0339x over previous
"""Trainium2 Bass kernel for nn_MemoryRetriever (cross-attention memory retriever).

v2: mask-compacted keys.  The per-key boolean mask keeps ~half of the 31290
memory tokens; the host gathers only surviving keys (plus their RoPE table
columns) and shards them across the 8 cores (PC keys/core, padded to a
multiple of 512; padding is masked via a -1e30 exp bias confined to the last
4 key tiles).  Each core computes K/V projections + RMSNorm + 3D-RoPE for its
shard, full Q (replicated, small), local masked-softmax partials
(un-normalized numerator + denominator), then one AllReduce combines partials
and each core output-projects its own 64-query slice.

Engine balance (vs v1): all elementwise math in bf16 (2x DVE mode); the
per-key RMSNorm scale and 1/sqrt(hd) are folded into the rope cos/sin tables
so Exp runs with scalar scale/bias over 2-tile-wide PSUM pairs; sum-of-squares
runs as near-free ones-stationary PE matmuls; attnV accumulates in PSUM across
all key tiles per 2-head group and is DMA'd straight from PSUM into the
collective buffer; softmax denominators tree-reduce on DVE in bf16 and pack
into one PSUM bank via indicator-column matmuls.
"""

import sys
import numpy as np

sys.path.insert(0, "/opt/trn_rl_repo")

DIM = 1024
HEADS = 8
HD = 128
SQ = 512
SK = 31290
N_CORES = 8
QS = SQ // N_CORES
EPS = 1e-6
NEG = -1.0e30

_cache = {}


def _build(nt, flags, pair=True):
    key = ("nc", nt, flags, pair)
    if key in _cache:
        return _cache[key]

    import concourse.bass as bass
    import concourse.tile as tile
    from concourse import mybir, bacc

    f32 = mybir.dt.float32
    bf16 = mybir.dt.bfloat16
    AF = mybir.ActivationFunctionType
    has_bq, has_bk, has_bv, has_bo = flags

    pc = nt * 128          # keys per core (padded)
    nblk = nt // 4         # 512-key blocks
    npair = nt // 2        # tile pairs (wide exp)

    nc = bacc.Bacc("TRN2", target_bir_lowering=False, debug=False,
                   num_devices=N_CORES)

    def din(name, shape, dt=f32):
        return nc.dram_tensor(name, list(shape), dt, kind="ExternalInput").ap()

    # per-core sharded inputs
    memT = din("memT", [DIM, pc], bf16)      # compacted mem shard, feature-major
    ctk = din("ctk", [HD, pc], bf16)         # K rope cos (pair-major rows)
    stk = din("stk", [HD, pc], bf16)
    mbias = din("mbias", [128, 4])           # pad bias (0/-1e30) for last 4 tiles
    # shared inputs
    xT = din("xT", [DIM, SQ], bf16)
    wq = din("wq", [128, 8, 8, 128], bf16)   # [p,i,o,m] = Wq.T[i*128+p, o*128+m]
    wk = din("wk", [128, 8, 8, 128], bf16)
    wo = din("wo", [128, 8, 8, 128], bf16)
    wv = din("wv", [128, 8, DIM], bf16)      # [p,i,o] = Wv.T[i*128+p, o]
    ctq = din("ctq", [128, 8, SQ], bf16)     # q rope cos (gq*gk folded, per head)
    stq = din("stq", [128, 8, SQ], bf16)
    bq_t = din("bq_t", [128, 8])
    bk_t = din("bk_t", [128, 8])
    bo_t = din("bo_t", [128, 8])
    bv_t = din("bv_t", [128, DIM])
    pmat = din("pmat", [128, 128], bf16)     # P.T for rope pair swap (+-1)
    ones_c = din("ones_c", [128, 1], bf16)
    ones_r = din("ones_r", [1, 128])
    ecols = din("ecols", [128, 8, 8], bf16)  # ecols[p,h,m] = (m==h)

    outT = nc.dram_tensor("outT", [DIM, SQ], f32, kind="ExternalOutput").ap()

    import os as _os
    _sim = _os.environ.get("KSIM", "0") == "1"
    cat = nc.dram_tensor("cat", [DIM + HEADS, SQ], bf16)
    cat_sh = nc.dram_tensor("cat_sh", [DIM + HEADS, SQ], bf16,
                            addr_space="Shared")

    with tile.TileContext(nc) as tc:
        with tc.tile_pool(name="consts", bufs=1) as consts, \
             tc.tile_pool(name="resid", bufs=1) as resid:

            # ---- resident tensors; DMAs issued in first-use order ----
            wk0_s = resid.tile([128, 1, 8, 128], bf16)
            wkr_s = resid.tile([128, 7, 8, 128], bf16)
            memt_blks = [resid.tile([128, 8, 512], bf16, tag=f"memt{b}",
                                    name=f"memt{b}")
                         for b in range(2)]
            wqo_s = resid.tile([128, 8, 8, 128], bf16)  # wq then wo
            xt_s = resid.tile([128, 8, SQ], bf16)
            wv_s = resid.tile([128, 8, DIM], bf16)
            kr = resid.tile([128, 8, pc], bf16)      # rope'd K (unnormalized)
            qT = resid.tile([128, 8, SQ], bf16)      # rope'd+normalized Q
            nsum = resid.tile([128, 8, SQ], bf16)    # numerator accumulators
            dsums = resid.tile([128, 8, SQ], bf16)   # exp-sum per head
            rs_s = resid.tile([128, nt], f32)        # per-key rms scale

            nc.sync.dma_start(memt_blks[0][:],
                              memT[:, 0:512].rearrange("(i p) t -> p i t",
                                                       p=128))
            nc.sync.dma_start(wk0_s[:], wk[:, 0:1, :, :])
            nc.sync.dma_start(wkr_s[:], wk[:, 1:8, :, :])
            # ---- small constants (cheap DMAs, SP queue) ----
            pt_s = consts.tile([128, 128], bf16)
            nc.sync.dma_start(pt_s[:], pmat)
            ones_s = consts.tile([128, 1], bf16)
            nc.sync.dma_start(ones_s[:], ones_c)
            ones_rs = consts.tile([1, 128], f32)
            nc.sync.dma_start(ones_rs[:], ones_r)
            mb_s = consts.tile([128, 4], f32)
            nc.sync.dma_start(mb_s[:], mbias)
            ec_s = consts.tile([128, 8, 8], bf16)
            nc.sync.dma_start(ec_s[:], ecols)
            zcol = consts.tile([128, 1], f32)
            nc.vector.memset(zcol[:], 0.0)
            ep128 = consts.tile([1, 1], f32)
            nc.vector.memset(ep128[:], 128.0 * EPS)
            epsk = consts.tile([128, 1], f32)
            nc.vector.memset(epsk[:], 128.0 * EPS)
            epsq = consts.tile([1, 1], f32)
            nc.vector.memset(epsq[:], EPS)
            if has_bq:
                bq_s = consts.tile([128, 8], f32)
                nc.sync.dma_start(bq_s[:], bq_t)
            if has_bk:
                bk_s = consts.tile([128, 8], f32)
                nc.sync.dma_start(bk_s[:], bk_t)
            if has_bo:
                bo_s = consts.tile([128, 8], f32)
                nc.sync.dma_start(bo_s[:], bo_t)
            if has_bv:
                bv_s = consts.tile([128, DIM], f32)
                nc.sync.dma_start(bv_s[:], bv_t)


            # ============ phase A: K-proj + rope per 512-key block ============
            with tc.tile_pool(name="kpool", bufs=2) as kpool, \
                 tc.tile_pool(name="pp_kv", bufs=2, space="PSUM") as pp_kv, \
                 tc.tile_pool(name="pp_sw", bufs=2, space="PSUM") as pp_sw, \
                 tc.tile_pool(name="pp_rs", bufs=2, space="PSUM") as pp_rs:
                ctk_s = kpool.tile([128, pc], bf16, tag="ctk", bufs=1)
                nc.sync.dma_start(ctk_s[:], ctk)
                stk_s = kpool.tile([128, pc], bf16, tag="stk", bufs=1)
                nc.sync.dma_start(stk_s[:], stk)
                nc.sync.dma_start(memt_blks[1][:],
                                  memT[:, 512:1024].rearrange(
                                      "(i p) t -> p i t", p=128))
                nc.sync.dma_start(wv_s[:], wv)
                nc.sync.dma_start(xt_s[:],
                                  xT.rearrange("(i p) q -> p i q", p=128))
                nc.sync.dma_start(wqo_s[:], wq)
                for b in range(nblk):
                    c0 = b * 512
                    memt = memt_blks[b % 2]
                    if b >= 2:
                        nc.sync.dma_start(
                            memt[:],
                            memT[:, c0:c0 + 512].rearrange("(i p) t -> p i t",
                                                           p=128))
                    yk = kpool.tile([128, 8, 512], bf16, tag="yk")
                    sw = kpool.tile([128, 8, 512], bf16, tag="sw")
                    if pair:
                        ps_rs = pp_rs.tile([1, 512], f32, tag="psrs",
                                           name="ps_rs")
                    else:
                        ps_rs = pp_rs.tile([128, 4], f32, tag="psrs",
                                           name="ps_rs")
                    for o in range(8):
                        ps_y = pp_kv.tile([128, 512], f32, tag="ps")
                        for i in range(8):
                            wsl = (wk0_s[:, 0, i, :] if o == 0
                                   else wkr_s[:, o - 1, i, :])
                            nc.tensor.matmul(ps_y[:], wsl,
                                             memt[:, i, :],
                                             start=(i == 0), stop=(i == 7))
                        if has_bk:
                            nc.scalar.activation(yk[:, o, :], ps_y[:],
                                                 AF.Identity,
                                                 bias=bk_s[:, o:o + 1])
                        else:
                            nc.scalar.activation(yk[:, o, :], ps_y[:], AF.Copy)
                        ps_sw = pp_sw.tile([128, 512], f32, tag="ps")
                        nc.tensor.matmul(ps_sw[:], pt_s[:], yk[:, o, :])
                        nc.scalar.activation(sw[:, o, :], ps_sw[:], AF.Copy)
                        ysq = kpool.tile([128, 512], bf16, tag="ysqo", bufs=2)
                        nc.vector.tensor_mul(ysq[:], yk[:, o, :], yk[:, o, :])
                        if pair:
                            nc.tensor.matmul(ps_rs[:], ones_s[:], ysq[:],
                                             start=(o == 0), stop=(o == 7))
                        else:
                            for tt in range(4):
                                nc.tensor.matmul(
                                    ps_rs[:, tt:tt + 1],
                                    ysq[:, tt * 128:(tt + 1) * 128], ones_s[:],
                                    start=(o == 0), stop=(o == 7))
                    # rs = (1/sqrt(HD))/sqrt(ms+eps) = 1/sqrt(sum/8+128eps)
                    if pair:
                        rroot = kpool.tile([1, 512], f32, tag="rroot")
                        nc.scalar.activation(rroot[:], ps_rs[:], AF.Sqrt,
                                             bias=ep128[:], scale=0.125)
                        rr = kpool.tile([1, 512], f32, tag="rr")
                        nc.vector.reciprocal(rr[:], rroot[:])
                        rsb = kpool.tile([128, 512], f32, tag="rsb")
                        nc.gpsimd.partition_broadcast(rsb[:], rr[:])
                        cts = kpool.tile([128, 512], bf16, tag="cts")
                        nc.vector.tensor_mul(cts[:], ctk_s[:, c0:c0 + 512],
                                             rsb[:])
                        sts = kpool.tile([128, 512], bf16, tag="sts")
                        nc.vector.tensor_mul(sts[:], stk_s[:, c0:c0 + 512],
                                             rsb[:])
                    else:
                        rroot = kpool.tile([128, 4], f32, tag="rroot")
                        nc.scalar.activation(rroot[:], ps_rs[:], AF.Sqrt,
                                             bias=epsk[:], scale=0.125)
                        nc.vector.reciprocal(rs_s[:, b * 4:b * 4 + 4],
                                             rroot[:])
                        cts = ctk_s[:, c0:c0 + 512]
                        sts = stk_s[:, c0:c0 + 512]
                    for o in range(8):
                        t1 = kpool.tile([128, 512], bf16, tag="t1")
                        nc.vector.tensor_mul(t1[:], yk[:, o, :], cts[:])
                        t2 = kpool.tile([128, 512], bf16, tag="t2")
                        nc.vector.tensor_mul(t2[:], sw[:, o, :], sts[:])
                        nc.vector.tensor_add(kr[:, o, c0:c0 + 512], t1[:], t2[:])
                    if b >= 2:
                        # refill this ring slot for phase B (blocks 0/1)
                        c2 = (b - 2) * 512
                        nc.sync.dma_start(
                            memt[:],
                            memT[:, c2:c2 + 512].rearrange("(i p) t -> p i t",
                                                           p=128))

            # ========= phase Q + B (shared scope: no drain between) =========
            with tc.tile_pool(name="qlate", bufs=1) as qlate, \
                 tc.tile_pool(name="bpool", bufs=2) as bpool, \
                 tc.tile_pool(name="ptsp", bufs=4) as ptsp, \
                 tc.tile_pool(name="pp_v", bufs=3, space="PSUM") as pp_v, \
                 tc.tile_pool(name="pp_sc", bufs=2, space="PSUM") as pp_sc, \
                 tc.tile_pool(name="pp_qrs", bufs=1, space="PSUM") as pp_qrs:
                qtab_cm = tc.tile_pool(name="qtab", bufs=1)
                qtab = qtab_cm.__enter__()
                ctq_s = qtab.tile([128, 8, SQ], bf16)
                nc.sync.dma_start(ctq_s[:], ctq)
                stq_s = qtab.tile([128, 8, SQ], bf16)
                nc.sync.dma_start(stq_s[:], stq)
                yq = qlate.tile([128, 8, SQ], bf16)
                swq = qlate.tile([128, 8, SQ], bf16)

                qden = pp_qrs.tile([8, SQ], f32)
                ps_rsq = qden[0:1, :]
                ps_den = qden
                for o in range(8):
                    ps2q = pp_sc.tile([128, 2, 512], f32, tag="ps2")
                    for i in range(8):
                        nc.tensor.matmul(ps2q[:, 0, :], wqo_s[:, o, i, :],
                                         xt_s[:, i, :],
                                         start=(i == 0), stop=(i == 7))
                    if has_bq:
                        nc.scalar.activation(yq[:, o, :], ps2q[:, 0, :],
                                             AF.Identity,
                                             bias=bq_s[:, o:o + 1])
                    else:
                        nc.scalar.activation(yq[:, o, :], ps2q[:, 0, :],
                                             AF.Copy)
                    nc.tensor.matmul(ps2q[:, 1, :], pt_s[:], yq[:, o, :])
                    nc.scalar.activation(swq[:, o, :], ps2q[:, 1, :], AF.Copy)
                    ysq = qlate.tile([128, SQ], bf16, tag="ysqq", bufs=2)
                    nc.vector.tensor_mul(ysq[:], yq[:, o, :], yq[:, o, :])
                    nc.tensor.matmul(ps_rsq[:], ones_s[:], ysq[:],
                                     start=(o == 0), stop=(o == 7))
                # V-proj helpers (phase B work)
                def v_chain(b, vt, tt, oh):
                    memt = memt_blks[b % 2]
                    ps_v = pp_v.tile([128, 512], f32, tag="ps", name="ps_v")
                    for i in range(8):
                        nc.tensor.matmul(
                            ps_v[:], memt[:, i, tt * 128:(tt + 1) * 128],
                            wv_s[:, i, oh * 512:(oh + 1) * 512],
                            start=(i == 0), stop=(i == 7))
                    if has_bv:
                        nc.vector.tensor_add(
                            vt[:, tt, oh * 512:(oh + 1) * 512], ps_v[:],
                            bv_s[:, oh * 512:(oh + 1) * 512])
                    elif (tt + oh) % 2 == 0:
                        nc.scalar.activation(
                            vt[:, tt, oh * 512:(oh + 1) * 512], ps_v[:],
                            AF.Copy)
                    else:
                        nc.vector.tensor_copy(
                            vt[:, tt, oh * 512:(oh + 1) * 512], ps_v[:])

                def v_proj(b, vt):
                    for tt in range(4):
                        for oh in range(2):
                            v_chain(b, vt, tt, oh)

                vts = [None] * nblk
                for b in range(2):
                    vts[b] = bpool.tile([128, 4, DIM], bf16, tag="vt",
                                        name=f"vt{b}", bufs=3)
                    v_proj(b, vts[b])
                    if b + 2 < nblk:
                        c2 = (b + 2) * 512
                        nc.sync.dma_start(
                            memt_blks[b % 2][:],
                            memT[:, c2:c2 + 512].rearrange("(i p) t -> p i t",
                                                           p=128))
                for b in range(2, nblk):
                    vts[b] = bpool.tile([128, 4, DIM], bf16, tag="vt",
                                        name=f"vt{b}", bufs=3)
                # finish Q: rmsnorm scale + rope (wide in-place DVE ops)
                rrootq = qlate.tile([1, SQ], f32)
                nc.scalar.activation(rrootq[:], ps_rsq[:], AF.Sqrt,
                                     bias=epsq[:], scale=1.0 / DIM)
                nc.vector.reciprocal(rrootq[:], rrootq[:])
                rsbf = qlate.tile([128, SQ], f32)
                nc.gpsimd.partition_broadcast(rsbf[:], rrootq[:])
                nc.vector.tensor_mul(yq[:], yq[:], ctq_s[:])
                nc.vector.tensor_mul(swq[:], swq[:], stq_s[:])
                nc.vector.tensor_add(yq[:], yq[:], swq[:])
                for o in range(8):
                    nc.vector.tensor_mul(qT[:, o, :], yq[:, o, :], rsbf[:])
                qtab_cm.__exit__(None, None, None)

                # ---- attention blocks (V-proj for b+2 interleaved) ----
                def attn_tail(b, h, pts):
                    vt = vts[b]
                    ps_a = pp_v.tile([128, SQ], f32, tag="ps", name="ps_a")
                    for tt in range(4):
                        nc.tensor.matmul(
                            ps_a[:], vt[:, tt, h * 128:(h + 1) * 128],
                            pts[:, tt, :], start=(tt == 0), stop=(tt == 3))
                    if b == 0:
                        nc.vector.tensor_copy(nsum[:, h, :], ps_a[:])
                    else:
                        nc.vector.tensor_add(nsum[:, h, :], nsum[:, h, :],
                                             ps_a[:])
                    # exp-sum partial for this block (bf16 tree)
                    nc.vector.tensor_add(pts[:, 0:2, :], pts[:, 0:2, :],
                                         pts[:, 2:4, :])
                    if b == 0:
                        nc.vector.tensor_add(dsums[:, h, :], pts[:, 0, :],
                                             pts[:, 1, :])
                    else:
                        nc.vector.tensor_add(pts[:, 0, :], pts[:, 0, :],
                                             pts[:, 1, :])
                        nc.vector.tensor_add(dsums[:, h, :], dsums[:, h, :],
                                             pts[:, 0, :])
                    if b == nblk - 1:
                        nc.tensor.matmul(ps_den[:], ec_s[:, h, :],
                                         dsums[:, h, :],
                                         start=(h == 0), stop=(h == 7))
                    if b == nblk - 1 and h in (3, 5):
                        h0, h1 = (0, 4) if h == 3 else (4, 6)
                        nc.sync.dma_start(
                            cat[h0 * 128:h1 * 128, :].rearrange(
                                "(h p) q -> p h q", p=128),
                            nsum[:, h0:h1, :])
                        if _sim:
                            nc.gpsimd.dma_start(
                                cat_sh[h0 * 128:h1 * 128, :],
                                cat[h0 * 128:h1 * 128, :])

                prev = None
                for b in range(nblk):
                    for h in range(8):
                        pts = ptsp.tile([128, 4, SQ], bf16, tag="pts")
                        for tp in range(2):
                            ps2 = pp_sc.tile([128, 2, 512], f32, tag="ps2")
                            for half in range(2):
                                lt = 2 * tp + half
                                gt = b * 4 + lt
                                nc.tensor.matmul(
                                    ps2[:, half, :],
                                    kr[:, h, gt * 128:(gt + 1) * 128],
                                    qT[:, h, :])
                            if not pair:
                                for half in range(2):
                                    lt = 2 * tp + half
                                    gt = b * 4 + lt
                                    bias_ap = (mb_s[:, lt:lt + 1]
                                               if b == nblk - 1 else zcol[:])
                                    nc.scalar.activation(
                                        pts[:, lt, :], ps2[:, half, :],
                                        AF.Exp, bias=bias_ap,
                                        scale=rs_s[:, gt:gt + 1])
                            elif b == nblk - 1:
                                for half in range(2):
                                    lt = 2 * tp + half
                                    nc.scalar.activation(
                                        pts[:, lt, :], ps2[:, half, :],
                                        AF.Exp, bias=mb_s[:, lt:lt + 1])
                            else:
                                nc.scalar.activation(
                                    pts[:, 2 * tp:2 * tp + 2, :], ps2[:],
                                    AF.Exp, bias=zcol[:])
                        if prev is not None:
                            attn_tail(*prev)
                        prev = (b, h, pts)
                    if b + 2 < nblk:
                        v_proj(b + 2, vts[b + 2])
                attn_tail(*prev)
                nc.sync.dma_start(
                    cat[768:DIM, :].rearrange("(h p) q -> p h q", p=128),
                    nsum[:, 6:8, :])
                # wo into the wq buffer for the tail
                nc.sync.dma_start(wqo_s[:], wo)

            with tc.tile_pool(name="dpool", bufs=1) as dpool:
                dcp = dpool.tile([8, SQ], bf16)
                nc.vector.tensor_copy(dcp[:], ps_den[:])
                nc.sync.dma_start(cat[DIM:DIM + HEADS, :], dcp[:])

            if _sim:
                nc.gpsimd.dma_start(cat_sh[768:DIM + HEADS, :],
                                    cat[768:DIM + HEADS, :])
            else:
                nc.gpsimd.collective_compute(
                    "AllReduce", mybir.AluOpType.add,
                    replica_groups=[list(range(N_CORES))],
                    ins=[cat[:]], outs=[cat_sh[:]])

            # ========== per-core output projection on its query slice ==========
            with tc.tile_pool(name="tail", bufs=1) as tail, \
                 tc.tile_pool(name="pp_t", bufs=2, space="PSUM") as pp_t:
                nred = tail.tile([128, 8, QS], bf16)
                dred = tail.tile([1, 8, QS], bf16)
                pid = nc.sync.partition_id()
                qoff = pid * QS
                nc.sync.dma_start(
                    nred[:],
                    cat_sh[0:DIM, bass.ds(qoff, QS)].rearrange(
                        "(h p) q -> p h q", p=128))
                nc.sync.dma_start(
                    dred[:],
                    cat_sh[DIM:DIM + HEADS, bass.ds(qoff, QS)].rearrange(
                        "(o h) q -> o h q", o=1))
                rd = tail.tile([1, 8 * QS], f32)
                nc.vector.reciprocal(rd[:], dred.rearrange("o h q -> o (h q)")[:])
                ps_rd = pp_t.tile([128, 8 * QS], f32, tag="psrd")
                nc.tensor.matmul(ps_rd[:], ones_rs[:], rd[:])
                nsc = tail.tile([128, 8, QS], bf16)
                nc.vector.tensor_mul(nsc.rearrange("p h q -> p (h q)")[:],
                                     nred.rearrange("p h q -> p (h q)")[:],
                                     ps_rd[:])
                out_sb = tail.tile([128, 8, QS], f32)
                for e in range(8):
                    ps_o = pp_t.tile([128, QS], f32, tag="ps")
                    for o in range(8):
                        nc.tensor.matmul(ps_o[:], wqo_s[:, e, o, :],
                                         nsc[:, o, :],
                                         start=(o == 0), stop=(o == 7))
                    if has_bo:
                        nc.scalar.activation(out_sb[:, e, :], ps_o[:],
                                             AF.Identity, bias=bo_s[:, e:e + 1])
                    else:
                        nc.scalar.activation(out_sb[:, e, :], ps_o[:], AF.Copy)
                nc.sync.dma_start(
                    outT.rearrange("(e p) q -> p e q", p=128)[:, :, 0:QS],
                    out_sb[:])

    nc.compile()
    _cache[key] = nc
    return nc


def _prep(x, mem, mask, cos_q, sin_q, cos_k, sin_k,
          Wq, bq, Wk, bk, Wv, bv, Wo, bo, gq, gk):
    import ml_dtypes
    f = np.float32
    bf = ml_dtypes.bfloat16
    x = np.asarray(x, f).reshape(SQ, DIM)
    mem = np.asarray(mem, f).reshape(SK, DIM)
    mask = np.asarray(mask).reshape(SK)
    cos_q = np.asarray(cos_q, f)
    sin_q = np.asarray(sin_q, f)
    cos_k = np.asarray(cos_k, f)
    sin_k = np.asarray(sin_k, f)
    Wq, Wk, Wv, Wo = (np.asarray(w, f) for w in (Wq, Wk, Wv, Wo))
    bq, bk, bv, bo, gq, gk = (np.asarray(v, f) for v in (bq, bk, bv, bo, gq, gk))

    if not np.allclose(gk, 1.0):
        gkp = gk.reshape(-1, 2)
        assert np.allclose(gkp[:, 0], gkp[:, 1]), "unsupported non-pairwise gk"

    flags = tuple(bool(np.any(v != 0.0)) for v in (bq, bk, bv, bo))

    # compact keys: keep only unmasked, shard evenly, pad per-core to 512
    idx = np.flatnonzero(mask)
    keep = len(idx)
    percore = -(-keep // N_CORES)                # ceil
    pc = max(512, -(-percore // 512) * 512)      # pad to multiple of 512
    nt = pc // 128

    def tile_w(WT):  # [1024,1024] (in,out of W.T) -> [p, o, i, m]
        return np.ascontiguousarray(
            WT.reshape(8, 128, 8, 128).transpose(1, 2, 0, 3)).astype(bf)

    ii = np.arange(128)
    jj = ii // 2
    partner = ii ^ 1

    # fold gq (and pairwise gk) into the q rope tables; sin pairs with
    # partner's gq
    gq_t = (gq * gk).reshape(8, 128)
    gq_sin = (gq.reshape(8, 128)[:, partner] * gk.reshape(8, 128))
    cq = cos_q[:, jj].T                # [128, SQ]
    sq = sin_q[:, jj].T
    ctq = np.ascontiguousarray(
        (cq[None, :, :] * gq_t[:, :, None]).transpose(1, 0, 2)).astype(bf)
    stq = np.ascontiguousarray(
        (sq[None, :, :] * gq_sin[:, :, None]).transpose(1, 0, 2)).astype(bf)

    PT = np.zeros((128, 128), f)
    even = ii[ii % 2 == 0]
    PT[even + 1, even] = -1.0
    PT[even, even + 1] = 1.0

    ec = np.zeros((128, 8, 8), f)
    for h in range(8):
        ec[:, h, h] = 1.0

    shared = {
        "xT": np.ascontiguousarray(x.T).astype(bf),
        "wq": tile_w(Wq.T), "wk": tile_w(Wk.T), "wo": tile_w(Wo.T),
        "wv": np.ascontiguousarray(
            Wv.T.reshape(8, 128, DIM).transpose(1, 0, 2)).astype(bf),
        "ctq": ctq, "stq": stq,
        "bq_t": np.ascontiguousarray(bq.reshape(8, 128).T),
        "bk_t": np.ascontiguousarray(bk.reshape(8, 128).T),
        "bo_t": np.ascontiguousarray(bo.reshape(8, 128).T),
        "bv_t": np.ascontiguousarray(np.tile(bv, (128, 1))),
        "pmat": PT.astype(bf),
        "ones_c": np.ones((128, 1), bf),
        "ones_r": np.ones((1, 128), f),
        "ecols": ec.astype(bf),
    }

    # global compacted + padded arrays
    tot = N_CORES * pc
    memT_full = np.zeros((DIM, tot), bf)
    ctk_full = np.zeros((HD, tot), bf)
    stk_full = np.zeros((HD, tot), bf)
    mb_full = np.full(tot, NEG, f)

    ck = cos_k[:, jj].T.astype(f)   # [128, SK]
    sk_t = sin_k[:, jj].T.astype(f)
    counts = [keep // N_CORES + (1 if c < keep % N_CORES else 0)
              for c in range(N_CORES)]
    off = 0
    for c in range(N_CORES):
        sl = idx[off:off + counts[c]]
        off += counts[c]
        d0 = c * pc
        memT_full[:, d0:d0 + len(sl)] = mem[sl].T.astype(bf)
        ctk_full[:, d0:d0 + len(sl)] = ck[:, sl].astype(bf)
        stk_full[:, d0:d0 + len(sl)] = sk_t[:, sl].astype(bf)
        mb_full[d0:d0 + len(sl)] = 0.0
        assert pc - len(sl) < 512, "padding must fit in last 4 tiles"

    in_maps = []
    for c in range(N_CORES):
        s = slice(c * pc, (c + 1) * pc)
        m = dict(shared)
        m["memT"] = np.ascontiguousarray(memT_full[:, s])
        m["ctk"] = np.ascontiguousarray(ctk_full[:, s])
        m["stk"] = np.ascontiguousarray(stk_full[:, s])
        # bias columns for the last 4 tiles only
        mb = mb_full[s][-512:].reshape(4, 128).T
        m["mbias"] = np.ascontiguousarray(mb)
        in_maps.append(m)
    return in_maps, nt, flags


def kernel(**inputs):
    from concourse.bass_utils import run_bass_kernel_spmd
    in_maps, nt, flags = _prep(**inputs)
    nc = _build(nt, flags)
    res = run_bass_kernel_spmd(nc, in_maps, list(range(N_CORES)))
    parts = [res.results[c]["outT"][:, 0:QS].T for c in range(N_CORES)]
    out = np.concatenate(parts, axis=0)
    return out[None].astype(np.float32)


# revision 41
# speedup vs baseline: 1.0830x; 1.0113x over previous
"""Trainium2 Bass kernel for nn_MemoryRetriever (cross-attention memory retriever).

v2: mask-compacted keys.  The per-key boolean mask keeps ~half of the 31290
memory tokens; the host gathers only surviving keys (plus their RoPE table
columns) and shards them across the 8 cores (PC keys/core, padded to a
multiple of 512; padding is masked via a -1e30 exp bias confined to the last
4 key tiles).  Each core computes K/V projections + RMSNorm + 3D-RoPE for its
shard, full Q (replicated, small), local masked-softmax partials
(un-normalized numerator + denominator), then one AllReduce combines partials
and each core output-projects its own 64-query slice.

Engine balance (vs v1): all elementwise math in bf16 (2x DVE mode); the
per-key RMSNorm scale and 1/sqrt(hd) are folded into the rope cos/sin tables
so Exp runs with scalar scale/bias over 2-tile-wide PSUM pairs; sum-of-squares
runs as near-free ones-stationary PE matmuls; attnV accumulates in PSUM across
all key tiles per 2-head group and is DMA'd straight from PSUM into the
collective buffer; softmax denominators tree-reduce on DVE in bf16 and pack
into one PSUM bank via indicator-column matmuls.
"""

import sys
import numpy as np

sys.path.insert(0, "/opt/trn_rl_repo")

DIM = 1024
HEADS = 8
HD = 128
SQ = 512
SK = 31290
N_CORES = 8
QS = SQ // N_CORES
EPS = 1e-6
NEG = -1.0e30

_cache = {}


def _build(nt, flags, pair=True):
    key = ("nc", nt, flags, pair)
    if key in _cache:
        return _cache[key]

    import concourse.bass as bass
    import concourse.tile as tile
    from concourse import mybir, bacc

    f32 = mybir.dt.float32
    bf16 = mybir.dt.bfloat16
    AF = mybir.ActivationFunctionType
    has_bq, has_bk, has_bv, has_bo = flags

    pc = nt * 128          # keys per core (padded)
    nblk = nt // 4         # 512-key blocks
    npair = nt // 2        # tile pairs (wide exp)

    nc = bacc.Bacc("TRN2", target_bir_lowering=False, debug=False,
                   num_devices=N_CORES)

    def din(name, shape, dt=f32):
        return nc.dram_tensor(name, list(shape), dt, kind="ExternalInput").ap()

    # per-core sharded inputs
    memT = din("memT", [DIM, pc], bf16)      # compacted mem shard, feature-major
    ctk = din("ctk", [HD, pc], bf16)         # K rope cos (pair-major rows)
    stk = din("stk", [HD, pc], bf16)
    mbias = din("mbias", [128, 4])           # pad bias (0/-1e30) for last 4 tiles
    # shared inputs
    xT = din("xT", [DIM, SQ], bf16)
    wq = din("wq", [128, 8, 8, 128], bf16)   # [p,i,o,m] = Wq.T[i*128+p, o*128+m]
    wk = din("wk", [128, 8, 8, 128], bf16)
    wo = din("wo", [128, 8, 8, 128], bf16)
    wv = din("wv", [128, 8, DIM], bf16)      # [p,i,o] = Wv.T[i*128+p, o]
    ctq = din("ctq", [128, 8, SQ], bf16)     # q rope cos (gq*gk folded, per head)
    stq = din("stq", [128, 8, SQ], bf16)
    bq_t = din("bq_t", [128, 8])
    bk_t = din("bk_t", [128, 8])
    bo_t = din("bo_t", [128, 8])
    bv_t = din("bv_t", [128, DIM])
    pmat = din("pmat", [128, 128], bf16)     # P.T for rope pair swap (+-1)
    ones_c = din("ones_c", [128, 1], bf16)
    ones_r = din("ones_r", [1, 128])
    ecols = din("ecols", [128, 8, 8], bf16)  # ecols[p,h,m] = (m==h)

    outT = nc.dram_tensor("outT", [DIM, SQ], f32, kind="ExternalOutput").ap()

    import os as _os
    _sim = _os.environ.get("KSIM", "0") == "1"
    cat = nc.dram_tensor("cat", [DIM + HEADS, SQ], bf16)
    cat_sh = nc.dram_tensor("cat_sh", [DIM + HEADS, SQ], bf16,
                            addr_space="Shared")

    with tile.TileContext(nc) as tc:
        with tc.tile_pool(name="consts", bufs=1) as consts, \
             tc.tile_pool(name="resid", bufs=1) as resid:

            # ---- resident tensors; DMAs issued in first-use order ----
            wk0_s = resid.tile([128, 1, 8, 128], bf16)
            wkr_s = resid.tile([128, 7, 8, 128], bf16)
            memt_blks = [resid.tile([128, 8, 512], bf16, tag=f"memt{b}",
                                    name=f"memt{b}")
                         for b in range(2)]
            wqo_s = resid.tile([128, 8, 8, 128], bf16)  # wq then wo
            xt_s = resid.tile([128, 8, SQ], bf16)
            wv_s = resid.tile([128, 8, DIM], bf16)
            kr = resid.tile([128, 8, pc], bf16)      # rope'd K (unnormalized)
            qT = resid.tile([128, 8, SQ], bf16)      # rope'd+normalized Q
            nsum = resid.tile([128, 8, SQ], bf16)    # numerator accumulators
            dsums = resid.tile([128, 8, SQ], bf16)   # exp-sum per head
            rs_s = resid.tile([128, nt], f32)        # per-key rms scale

            nc.sync.dma_start(memt_blks[0][:],
                              memT[:, 0:512].rearrange("(i p) t -> p i t",
                                                       p=128))
            nc.sync.dma_start(wk0_s[:], wk[:, 0:1, :, :])
            nc.sync.dma_start(wkr_s[:], wk[:, 1:8, :, :])
            # ---- small constants (cheap DMAs, SP queue) ----
            pt_s = consts.tile([128, 128], bf16)
            nc.sync.dma_start(pt_s[:], pmat)
            ones_s = consts.tile([128, 1], bf16)
            nc.sync.dma_start(ones_s[:], ones_c)
            ones_rs = consts.tile([1, 128], f32)
            nc.sync.dma_start(ones_rs[:], ones_r)
            mb_s = consts.tile([128, 4], f32)
            nc.sync.dma_start(mb_s[:], mbias)
            ec_s = consts.tile([128, 8, 8], bf16)
            nc.sync.dma_start(ec_s[:], ecols)
            zcol = consts.tile([128, 1], f32)
            nc.vector.memset(zcol[:], 0.0)
            ep128 = consts.tile([1, 1], f32)
            nc.vector.memset(ep128[:], 128.0 * EPS)
            epsk = consts.tile([128, 1], f32)
            nc.vector.memset(epsk[:], 128.0 * EPS)
            epsq = consts.tile([1, 1], f32)
            nc.vector.memset(epsq[:], EPS)
            if has_bq:
                bq_s = consts.tile([128, 8], f32)
                nc.sync.dma_start(bq_s[:], bq_t)
            if has_bk:
                bk_s = consts.tile([128, 8], f32)
                nc.sync.dma_start(bk_s[:], bk_t)
            if has_bo:
                bo_s = consts.tile([128, 8], f32)
                nc.sync.dma_start(bo_s[:], bo_t)
            if has_bv:
                bv_s = consts.tile([128, DIM], f32)
                nc.sync.dma_start(bv_s[:], bv_t)


            # ============ phase A: K-proj + rope per 512-key block ============
            with tc.tile_pool(name="kpool", bufs=2) as kpool, \
                 tc.tile_pool(name="pp_kv", bufs=2, space="PSUM") as pp_kv, \
                 tc.tile_pool(name="pp_sw", bufs=2, space="PSUM") as pp_sw, \
                 tc.tile_pool(name="pp_rs", bufs=2, space="PSUM") as pp_rs:
                ctk_s = kpool.tile([128, pc], bf16, tag="ctk", bufs=1)
                nc.sync.dma_start(ctk_s[:], ctk)
                stk_s = kpool.tile([128, pc], bf16, tag="stk", bufs=1)
                nc.sync.dma_start(stk_s[:], stk)
                nc.sync.dma_start(memt_blks[1][:],
                                  memT[:, 512:1024].rearrange(
                                      "(i p) t -> p i t", p=128))
                nc.sync.dma_start(wv_s[:], wv)
                nc.sync.dma_start(xt_s[:],
                                  xT.rearrange("(i p) q -> p i q", p=128))
                nc.sync.dma_start(wqo_s[:], wq)
                for b in range(nblk):
                    c0 = b * 512
                    memt = memt_blks[b % 2]
                    if b >= 2:
                        nc.sync.dma_start(
                            memt[:],
                            memT[:, c0:c0 + 512].rearrange("(i p) t -> p i t",
                                                           p=128))
                    yk = kpool.tile([128, 8, 512], bf16, tag="yk")
                    sw = kpool.tile([128, 8, 512], bf16, tag="sw")
                    if pair:
                        ps_rs = pp_rs.tile([1, 512], f32, tag="psrs",
                                           name="ps_rs")
                    else:
                        ps_rs = pp_rs.tile([128, 4], f32, tag="psrs",
                                           name="ps_rs")
                    for o in range(8):
                        ps_y = pp_kv.tile([128, 512], f32, tag="ps")
                        for i in range(8):
                            wsl = (wk0_s[:, 0, i, :] if o == 0
                                   else wkr_s[:, o - 1, i, :])
                            nc.tensor.matmul(ps_y[:], wsl,
                                             memt[:, i, :],
                                             start=(i == 0), stop=(i == 7))
                        if has_bk:
                            nc.scalar.activation(yk[:, o, :], ps_y[:],
                                                 AF.Identity,
                                                 bias=bk_s[:, o:o + 1])
                        else:
                            nc.scalar.activation(yk[:, o, :], ps_y[:], AF.Copy)
                        ps_sw = pp_sw.tile([128, 512], f32, tag="ps")
                        nc.tensor.matmul(ps_sw[:], pt_s[:], yk[:, o, :])
                        nc.scalar.activation(sw[:, o, :], ps_sw[:], AF.Copy)
                        ysq = kpool.tile([128, 512], bf16, tag="ysqo", bufs=2)
                        nc.vector.tensor_mul(ysq[:], yk[:, o, :], yk[:, o, :])
                        if pair:
                            nc.tensor.matmul(ps_rs[:], ones_s[:], ysq[:],
                                             start=(o == 0), stop=(o == 7))
                        else:
                            for tt in range(4):
                                nc.tensor.matmul(
                                    ps_rs[:, tt:tt + 1],
                                    ysq[:, tt * 128:(tt + 1) * 128], ones_s[:],
                                    start=(o == 0), stop=(o == 7))
                    # rs = (1/sqrt(HD))/sqrt(ms+eps) = 1/sqrt(sum/8+128eps)
                    if pair:
                        rroot = kpool.tile([1, 512], f32, tag="rroot")
                        nc.scalar.activation(rroot[:], ps_rs[:], AF.Sqrt,
                                             bias=ep128[:], scale=0.125)
                        rr = kpool.tile([1, 512], f32, tag="rr")
                        nc.vector.reciprocal(rr[:], rroot[:])
                        rsb = kpool.tile([128, 512], f32, tag="rsb")
                        nc.gpsimd.partition_broadcast(rsb[:], rr[:])
                        cts = kpool.tile([128, 512], bf16, tag="cts")
                        nc.vector.tensor_mul(cts[:], ctk_s[:, c0:c0 + 512],
                                             rsb[:])
                        sts = kpool.tile([128, 512], bf16, tag="sts")
                        nc.vector.tensor_mul(sts[:], stk_s[:, c0:c0 + 512],
                                             rsb[:])
                    else:
                        rroot = kpool.tile([128, 4], f32, tag="rroot")
                        nc.scalar.activation(rroot[:], ps_rs[:], AF.Sqrt,
                                             bias=epsk[:], scale=0.125)
                        nc.vector.reciprocal(rs_s[:, b * 4:b * 4 + 4],
                                             rroot[:])
                        cts = ctk_s[:, c0:c0 + 512]
                        sts = stk_s[:, c0:c0 + 512]
                    for o in range(8):
                        t1 = kpool.tile([128, 512], bf16, tag="t1")
                        nc.vector.tensor_mul(t1[:], yk[:, o, :], cts[:])
                        t2 = kpool.tile([128, 512], bf16, tag="t2")
                        nc.vector.tensor_mul(t2[:], sw[:, o, :], sts[:])
                        nc.vector.tensor_add(kr[:, o, c0:c0 + 512], t1[:], t2[:])
                    if b >= 2:
                        # refill this ring slot for phase B (blocks 0/1)
                        c2 = (b - 2) * 512
                        nc.sync.dma_start(
                            memt[:],
                            memT[:, c2:c2 + 512].rearrange("(i p) t -> p i t",
                                                           p=128))

            # ========= phase Q + B (shared scope: no drain between) =========
            with tc.tile_pool(name="qlate", bufs=1) as qlate, \
                 tc.tile_pool(name="bpool", bufs=2) as bpool, \
                 tc.tile_pool(name="ptsp", bufs=4) as ptsp, \
                 tc.tile_pool(name="pp_v", bufs=3, space="PSUM") as pp_v, \
                 tc.tile_pool(name="pp_sc", bufs=2, space="PSUM") as pp_sc, \
                 tc.tile_pool(name="pp_qrs", bufs=1, space="PSUM") as pp_qrs:
                qtab_cm = tc.tile_pool(name="qtab", bufs=1)
                qtab = qtab_cm.__enter__()
                ctq_s = qtab.tile([128, 8, SQ], bf16)
                nc.sync.dma_start(ctq_s[:], ctq)
                stq_s = qtab.tile([128, 8, SQ], bf16)
                nc.sync.dma_start(stq_s[:], stq)
                yq = qlate.tile([128, 8, SQ], bf16)
                swq = qlate.tile([128, 8, SQ], bf16)

                qden = pp_qrs.tile([8, SQ], f32)
                ps_rsq = qden[0:1, :]
                ps_den = qden
                for o in range(8):
                    ps2q = pp_sc.tile([128, 2, 512], f32, tag="ps2")
                    for i in range(8):
                        nc.tensor.matmul(ps2q[:, 0, :], wqo_s[:, o, i, :],
                                         xt_s[:, i, :],
                                         start=(i == 0), stop=(i == 7))
                    if has_bq:
                        nc.scalar.activation(yq[:, o, :], ps2q[:, 0, :],
                                             AF.Identity,
                                             bias=bq_s[:, o:o + 1])
                    else:
                        nc.scalar.activation(yq[:, o, :], ps2q[:, 0, :],
                                             AF.Copy)
                    nc.tensor.matmul(ps2q[:, 1, :], pt_s[:], yq[:, o, :])
                    nc.scalar.activation(swq[:, o, :], ps2q[:, 1, :], AF.Copy)
                    ysq = qlate.tile([128, SQ], bf16, tag="ysqq", bufs=2)
                    nc.vector.tensor_mul(ysq[:], yq[:, o, :], yq[:, o, :])
                    nc.tensor.matmul(ps_rsq[:], ones_s[:], ysq[:],
                                     start=(o == 0), stop=(o == 7))
                # V-proj helpers (phase B work)
                def v_chain(b, vt, tt, oh):
                    memt = memt_blks[b % 2]
                    ps_v = pp_v.tile([128, 512], f32, tag="ps", name="ps_v")
                    for i in range(8):
                        nc.tensor.matmul(
                            ps_v[:], memt[:, i, tt * 128:(tt + 1) * 128],
                            wv_s[:, i, oh * 512:(oh + 1) * 512],
                            start=(i == 0), stop=(i == 7))
                    if has_bv:
                        nc.vector.tensor_add(
                            vt[:, tt, oh * 512:(oh + 1) * 512], ps_v[:],
                            bv_s[:, oh * 512:(oh + 1) * 512])
                    elif (tt + oh) % 2 == 0:
                        nc.scalar.activation(
                            vt[:, tt, oh * 512:(oh + 1) * 512], ps_v[:],
                            AF.Copy)
                    else:
                        nc.vector.tensor_copy(
                            vt[:, tt, oh * 512:(oh + 1) * 512], ps_v[:])

                def v_proj(b, vt):
                    for tt in range(4):
                        for oh in range(2):
                            v_chain(b, vt, tt, oh)

                vts = [None] * nblk
                for b in range(2):
                    vts[b] = bpool.tile([128, 4, DIM], bf16, tag="vt",
                                        name=f"vt{b}", bufs=3)
                    v_proj(b, vts[b])
                    if b + 2 < nblk:
                        c2 = (b + 2) * 512
                        nc.sync.dma_start(
                            memt_blks[b % 2][:],
                            memT[:, c2:c2 + 512].rearrange("(i p) t -> p i t",
                                                           p=128))
                for b in range(2, nblk):
                    vts[b] = bpool.tile([128, 4, DIM], bf16, tag="vt",
                                        name=f"vt{b}", bufs=3)
                # finish Q: rmsnorm scale + rope (wide in-place DVE ops)
                rrootq = qlate.tile([1, SQ], f32)
                nc.scalar.activation(rrootq[:], ps_rsq[:], AF.Sqrt,
                                     bias=epsq[:], scale=1.0 / DIM)
                nc.vector.reciprocal(rrootq[:], rrootq[:])
                rsbf = qlate.tile([128, SQ], f32)
                nc.gpsimd.partition_broadcast(rsbf[:], rrootq[:])
                nc.vector.tensor_mul(yq[:], yq[:], ctq_s[:])
                nc.vector.tensor_mul(swq[:], swq[:], stq_s[:])
                nc.vector.tensor_add(yq[:], yq[:], swq[:])
                for o in range(8):
                    nc.vector.tensor_mul(qT[:, o, :], yq[:, o, :], rsbf[:])
                qtab_cm.__exit__(None, None, None)

                # ---- attention blocks (V-proj for b+2 interleaved) ----
                def attn_tail(b, h, pts):
                    vt = vts[b]
                    ps_a = pp_v.tile([128, SQ], f32, tag="ps", name="ps_a")
                    for tt in range(4):
                        nc.tensor.matmul(
                            ps_a[:], vt[:, tt, h * 128:(h + 1) * 128],
                            pts[:, tt, :], start=(tt == 0), stop=(tt == 3))
                    if b == 0:
                        nc.vector.tensor_copy(nsum[:, h, :], ps_a[:])
                    else:
                        nc.vector.tensor_add(nsum[:, h, :], nsum[:, h, :],
                                             ps_a[:])
                    # exp-sum partial for this block (bf16 tree)
                    nc.vector.tensor_add(pts[:, 0:2, :], pts[:, 0:2, :],
                                         pts[:, 2:4, :])
                    if b == 0:
                        nc.vector.tensor_add(dsums[:, h, :], pts[:, 0, :],
                                             pts[:, 1, :])
                    else:
                        nc.vector.tensor_add(pts[:, 0, :], pts[:, 0, :],
                                             pts[:, 1, :])
                        nc.vector.tensor_add(dsums[:, h, :], dsums[:, h, :],
                                             pts[:, 0, :])
                    if b == nblk - 1:
                        nc.tensor.matmul(ps_den[:], ec_s[:, h, :],
                                         dsums[:, h, :],
                                         start=(h == 0), stop=(h == 7))
                    if b == nblk - 1 and h in (3, 5):
                        h0, h1 = (0, 4) if h == 3 else (4, 6)
                        nc.sync.dma_start(
                            cat[h0 * 128:h1 * 128, :].rearrange(
                                "(h p) q -> p h q", p=128),
                            nsum[:, h0:h1, :])
                        if _sim:
                            nc.gpsimd.dma_start(
                                cat_sh[h0 * 128:h1 * 128, :],
                                cat[h0 * 128:h1 * 128, :])

                pend = []
                for b in range(nblk):
                    for h in range(8):
                        pts = ptsp.tile([128, 4, SQ], bf16, tag="pts")
                        for tp in range(2):
                            ps2 = pp_sc.tile([128, 2, 512], f32, tag="ps2")
                            for half in range(2):
                                lt = 2 * tp + half
                                gt = b * 4 + lt
                                nc.tensor.matmul(
                                    ps2[:, half, :],
                                    kr[:, h, gt * 128:(gt + 1) * 128],
                                    qT[:, h, :])
                            if not pair:
                                for half in range(2):
                                    lt = 2 * tp + half
                                    gt = b * 4 + lt
                                    bias_ap = (mb_s[:, lt:lt + 1]
                                               if b == nblk - 1 else zcol[:])
                                    nc.scalar.activation(
                                        pts[:, lt, :], ps2[:, half, :],
                                        AF.Exp, bias=bias_ap,
                                        scale=rs_s[:, gt:gt + 1])
                            elif b == nblk - 1:
                                for half in range(2):
                                    lt = 2 * tp + half
                                    nc.scalar.activation(
                                        pts[:, lt, :], ps2[:, half, :],
                                        AF.Exp, bias=mb_s[:, lt:lt + 1])
                            else:
                                nc.scalar.activation(
                                    pts[:, 2 * tp:2 * tp + 2, :], ps2[:],
                                    AF.Exp, bias=zcol[:])
                        pend.append((b, h, pts))
                        if len(pend) > 2:
                            attn_tail(*pend.pop(0))
                    if b + 2 < nblk:
                        v_proj(b + 2, vts[b + 2])
                for p_ in pend:
                    attn_tail(*p_)
                nc.sync.dma_start(
                    cat[768:DIM, :].rearrange("(h p) q -> p h q", p=128),
                    nsum[:, 6:8, :])
                # wo into the wq buffer for the tail
                nc.sync.dma_start(wqo_s[:], wo)

            with tc.tile_pool(name="dpool", bufs=1) as dpool:
                dcp = dpool.tile([8, SQ], bf16)
                nc.vector.tensor_copy(dcp[:], ps_den[:])
                nc.sync.dma_start(cat[DIM:DIM + HEADS, :], dcp[:])

            if _sim:
                nc.gpsimd.dma_start(cat_sh[768:DIM + HEADS, :],
                                    cat[768:DIM + HEADS, :])
            else:
                nc.gpsimd.collective_compute(
                    "AllReduce", mybir.AluOpType.add,
                    replica_groups=[list(range(N_CORES))],
                    ins=[cat[:]], outs=[cat_sh[:]])

            # ========== per-core output projection on its query slice ==========
            with tc.tile_pool(name="tail", bufs=1) as tail, \
                 tc.tile_pool(name="pp_t", bufs=2, space="PSUM") as pp_t:
                nred = tail.tile([128, 8, QS], bf16)
                dred = tail.tile([1, 8, QS], bf16)
                pid = nc.sync.partition_id()
                qoff = pid * QS
                nc.sync.dma_start(
                    nred[:],
                    cat_sh[0:DIM, bass.ds(qoff, QS)].rearrange(
                        "(h p) q -> p h q", p=128))
                nc.sync.dma_start(
                    dred[:],
                    cat_sh[DIM:DIM + HEADS, bass.ds(qoff, QS)].rearrange(
                        "(o h) q -> o h q", o=1))
                rd = tail.tile([1, 8 * QS], f32)
                nc.vector.reciprocal(rd[:], dred.rearrange("o h q -> o (h q)")[:])
                ps_rd = pp_t.tile([128, 8 * QS], f32, tag="psrd")
                nc.tensor.matmul(ps_rd[:], ones_rs[:], rd[:])
                nsc = tail.tile([128, 8, QS], bf16)
                nc.vector.tensor_mul(nsc.rearrange("p h q -> p (h q)")[:],
                                     nred.rearrange("p h q -> p (h q)")[:],
                                     ps_rd[:])
                out_sb = tail.tile([128, 8, QS], f32)
                for e in range(8):
                    ps_o = pp_t.tile([128, QS], f32, tag="ps")
                    for o in range(8):
                        nc.tensor.matmul(ps_o[:], wqo_s[:, e, o, :],
                                         nsc[:, o, :],
                                         start=(o == 0), stop=(o == 7))
                    if has_bo:
                        nc.scalar.activation(out_sb[:, e, :], ps_o[:],
                                             AF.Identity, bias=bo_s[:, e:e + 1])
                    else:
                        nc.scalar.activation(out_sb[:, e, :], ps_o[:], AF.Copy)
                nc.sync.dma_start(
                    outT.rearrange("(e p) q -> p e q", p=128)[:, :, 0:QS],
                    out_sb[:])

    nc.compile()
    _cache[key] = nc
    return nc


def _prep(x, mem, mask, cos_q, sin_q, cos_k, sin_k,
          Wq, bq, Wk, bk, Wv, bv, Wo, bo, gq, gk):
    import ml_dtypes
    f = np.float32
    bf = ml_dtypes.bfloat16
    x = np.asarray(x, f).reshape(SQ, DIM)
    mem = np.asarray(mem, f).reshape(SK, DIM)
    mask = np.asarray(mask).reshape(SK)
    cos_q = np.asarray(cos_q, f)
    sin_q = np.asarray(sin_q, f)
    cos_k = np.asarray(cos_k, f)
    sin_k = np.asarray(sin_k, f)
    Wq, Wk, Wv, Wo = (np.asarray(w, f) for w in (Wq, Wk, Wv, Wo))
    bq, bk, bv, bo, gq, gk = (np.asarray(v, f) for v in (bq, bk, bv, bo, gq, gk))

    if not np.allclose(gk, 1.0):
        gkp = gk.reshape(-1, 2)
        assert np.allclose(gkp[:, 0], gkp[:, 1]), "unsupported non-pairwise gk"

    flags = tuple(bool(np.any(v != 0.0)) for v in (bq, bk, bv, bo))

    # compact keys: keep only unmasked, shard evenly, pad per-core to 512
    idx = np.flatnonzero(mask)
    keep = len(idx)
    percore = -(-keep // N_CORES)                # ceil
    pc = max(512, -(-percore // 512) * 512)      # pad to multiple of 512
    nt = pc // 128

    def tile_w(WT):  # [1024,1024] (in,out of W.T) -> [p, o, i, m]
        return np.ascontiguousarray(
            WT.reshape(8, 128, 8, 128).transpose(1, 2, 0, 3)).astype(bf)

    ii = np.arange(128)
    jj = ii // 2
    partner = ii ^ 1

    # fold gq (and pairwise gk) into the q rope tables; sin pairs with
    # partner's gq
    gq_t = (gq * gk).reshape(8, 128)
    gq_sin = (gq.reshape(8, 128)[:, partner] * gk.reshape(8, 128))
    cq = cos_q[:, jj].T                # [128, SQ]
    sq = sin_q[:, jj].T
    ctq = np.ascontiguousarray(
        (cq[None, :, :] * gq_t[:, :, None]).transpose(1, 0, 2)).astype(bf)
    stq = np.ascontiguousarray(
        (sq[None, :, :] * gq_sin[:, :, None]).transpose(1, 0, 2)).astype(bf)

    PT = np.zeros((128, 128), f)
    even = ii[ii % 2 == 0]
    PT[even + 1, even] = -1.0
    PT[even, even + 1] = 1.0

    ec = np.zeros((128, 8, 8), f)
    for h in range(8):
        ec[:, h, h] = 1.0

    shared = {
        "xT": np.ascontiguousarray(x.T).astype(bf),
        "wq": tile_w(Wq.T), "wk": tile_w(Wk.T), "wo": tile_w(Wo.T),
        "wv": np.ascontiguousarray(
            Wv.T.reshape(8, 128, DIM).transpose(1, 0, 2)).astype(bf),
        "ctq": ctq, "stq": stq,
        "bq_t": np.ascontiguousarray(bq.reshape(8, 128).T),
        "bk_t": np.ascontiguousarray(bk.reshape(8, 128).T),
        "bo_t": np.ascontiguousarray(bo.reshape(8, 128).T),
        "bv_t": np.ascontiguousarray(np.tile(bv, (128, 1))),
        "pmat": PT.astype(bf),
        "ones_c": np.ones((128, 1), bf),
        "ones_r": np.ones((1, 128), f),
        "ecols": ec.astype(bf),
    }

    # global compacted + padded arrays
    tot = N_CORES * pc
    memT_full = np.zeros((DIM, tot), bf)
    ctk_full = np.zeros((HD, tot), bf)
    stk_full = np.zeros((HD, tot), bf)
    mb_full = np.full(tot, NEG, f)

    ck = cos_k[:, jj].T.astype(f)   # [128, SK]
    sk_t = sin_k[:, jj].T.astype(f)
    counts = [keep // N_CORES + (1 if c < keep % N_CORES else 0)
              for c in range(N_CORES)]
    off = 0
    for c in range(N_CORES):
        sl = idx[off:off + counts[c]]
        off += counts[c]
        d0 = c * pc
        memT_full[:, d0:d0 + len(sl)] = mem[sl].T.astype(bf)
        ctk_full[:, d0:d0 + len(sl)] = ck[:, sl].astype(bf)
        stk_full[:, d0:d0 + len(sl)] = sk_t[:, sl].astype(bf)
        mb_full[d0:d0 + len(sl)] = 0.0
        assert pc - len(sl) < 512, "padding must fit in last 4 tiles"

    in_maps = []
    for c in range(N_CORES):
        s = slice(c * pc, (c + 1) * pc)
        m = dict(shared)
        m["memT"] = np.ascontiguousarray(memT_full[:, s])
        m["ctk"] = np.ascontiguousarray(ctk_full[:, s])
        m["stk"] = np.ascontiguousarray(stk_full[:, s])
        # bias columns for the last 4 tiles only
        mb = mb_full[s][-512:].reshape(4, 128).T
        m["mbias"] = np.ascontiguousarray(mb)
        in_maps.append(m)
    return in_maps, nt, flags


def kernel(**inputs):
    from concourse.bass_utils import run_bass_kernel_spmd
    in_maps, nt, flags = _prep(**inputs)
    nc = _build(nt, flags)
    res = run_bass_kernel_spmd(nc, in_maps, list(range(N_CORES)))
    parts = [res.results[c]["outT"][:, 0:QS].T for c in range(N_CORES)]
    out = np.concatenate(parts, axis=0)
    return out[None].astype(np.float32)


# revision 42
# speedup vs baseline: 1.0833x; 1.0003x over previous
"""Trainium2 Bass kernel for nn_MemoryRetriever (cross-attention memory retriever).

v2: mask-compacted keys.  The per-key boolean mask keeps ~half of the 31290
memory tokens; the host gathers only surviving keys (plus their RoPE table
columns) and shards them across the 8 cores (PC keys/core, padded to a
multiple of 512; padding is masked via a -1e30 exp bias confined to the last
4 key tiles).  Each core computes K/V projections + RMSNorm + 3D-RoPE for its
shard, full Q (replicated, small), local masked-softmax partials
(un-normalized numerator + denominator), then one AllReduce combines partials
and each core output-projects its own 64-query slice.

Engine balance (vs v1): all elementwise math in bf16 (2x DVE mode); the
per-key RMSNorm scale and 1/sqrt(hd) are folded into the rope cos/sin tables
so Exp runs with scalar scale/bias over 2-tile-wide PSUM pairs; sum-of-squares
runs as near-free ones-stationary PE matmuls; attnV accumulates in PSUM across
all key tiles per 2-head group and is DMA'd straight from PSUM into the
collective buffer; softmax denominators tree-reduce on DVE in bf16 and pack
into one PSUM bank via indicator-column matmuls.
"""

import sys
import numpy as np

sys.path.insert(0, "/opt/trn_rl_repo")

DIM = 1024
HEADS = 8
HD = 128
SQ = 512
SK = 31290
N_CORES = 8
QS = SQ // N_CORES
EPS = 1e-6
NEG = -1.0e30

_cache = {}


def _build(nt, flags, pair=True):
    key = ("nc", nt, flags, pair)
    if key in _cache:
        return _cache[key]

    import concourse.bass as bass
    import concourse.tile as tile
    from concourse import mybir, bacc

    f32 = mybir.dt.float32
    bf16 = mybir.dt.bfloat16
    AF = mybir.ActivationFunctionType
    has_bq, has_bk, has_bv, has_bo = flags

    pc = nt * 128          # keys per core (padded)
    nblk = nt // 4         # 512-key blocks
    npair = nt // 2        # tile pairs (wide exp)

    nc = bacc.Bacc("TRN2", target_bir_lowering=False, debug=False,
                   num_devices=N_CORES)

    def din(name, shape, dt=f32):
        return nc.dram_tensor(name, list(shape), dt, kind="ExternalInput").ap()

    # per-core sharded inputs
    memT = din("memT", [DIM, pc], bf16)      # compacted mem shard, feature-major
    ctk = din("ctk", [HD, pc], bf16)         # K rope cos (pair-major rows)
    stk = din("stk", [HD, pc], bf16)
    mbias = din("mbias", [128, 4])           # pad bias (0/-1e30) for last 4 tiles
    # shared inputs
    xT = din("xT", [DIM, SQ], bf16)
    wq = din("wq", [128, 8, 8, 128], bf16)   # [p,i,o,m] = Wq.T[i*128+p, o*128+m]
    wk = din("wk", [128, 8, 8, 128], bf16)
    wo = din("wo", [128, 8, 8, 128], bf16)
    wv = din("wv", [128, 8, DIM], bf16)      # [p,i,o] = Wv.T[i*128+p, o]
    ctq = din("ctq", [128, 8, SQ], bf16)     # q rope cos (gq*gk folded, per head)
    stq = din("stq", [128, 8, SQ], bf16)
    bq_t = din("bq_t", [128, 8])
    bk_t = din("bk_t", [128, 8])
    bo_t = din("bo_t", [128, 8])
    bv_t = din("bv_t", [128, DIM])
    pmat = din("pmat", [128, 128], bf16)     # P.T for rope pair swap (+-1)
    ones_c = din("ones_c", [128, 1], bf16)
    ones_r = din("ones_r", [1, 128])
    ecols = din("ecols", [128, 8, 8], bf16)  # ecols[p,h,m] = (m==h)

    outT = nc.dram_tensor("outT", [DIM, SQ], f32, kind="ExternalOutput").ap()

    import os as _os
    _sim = _os.environ.get("KSIM", "0") == "1"
    cat = nc.dram_tensor("cat", [DIM + HEADS, SQ], bf16)
    cat_sh = nc.dram_tensor("cat_sh", [DIM + HEADS, SQ], bf16,
                            addr_space="Shared")

    with tile.TileContext(nc) as tc:
        with tc.tile_pool(name="consts", bufs=1) as consts, \
             tc.tile_pool(name="resid", bufs=1) as resid:

            # ---- resident tensors; DMAs issued in first-use order ----
            wk0_s = resid.tile([128, 1, 8, 128], bf16)
            wkr_s = resid.tile([128, 7, 8, 128], bf16)
            memt_blks = [resid.tile([128, 8, 512], bf16, tag=f"memt{b}",
                                    name=f"memt{b}")
                         for b in range(2)]
            wqo_s = resid.tile([128, 8, 8, 128], bf16)  # wq then wo
            xt_s = resid.tile([128, 8, SQ], bf16)
            wv_s = resid.tile([128, 8, DIM], bf16)
            kr = resid.tile([128, 8, pc], bf16)      # rope'd K (unnormalized)
            qT = resid.tile([128, 8, SQ], bf16)      # rope'd+normalized Q
            nsum = resid.tile([128, 8, SQ], bf16)    # numerator accumulators
            dsums = resid.tile([128, 8, SQ], bf16)   # exp-sum per head
            rs_s = resid.tile([128, nt], f32)        # per-key rms scale

            nc.sync.dma_start(memt_blks[0][:],
                              memT[:, 0:512].rearrange("(i p) t -> p i t",
                                                       p=128))
            nc.sync.dma_start(wk0_s[:], wk[:, 0:1, :, :])
            nc.sync.dma_start(wkr_s[:], wk[:, 1:8, :, :])
            # ---- small constants (cheap DMAs, SP queue) ----
            pt_s = consts.tile([128, 128], bf16)
            nc.sync.dma_start(pt_s[:], pmat)
            ones_s = consts.tile([128, 1], bf16)
            nc.sync.dma_start(ones_s[:], ones_c)
            ones_rs = consts.tile([1, 128], f32)
            nc.sync.dma_start(ones_rs[:], ones_r)
            mb_s = consts.tile([128, 4], f32)
            nc.sync.dma_start(mb_s[:], mbias)
            ec_s = consts.tile([128, 8, 8], bf16)
            nc.sync.dma_start(ec_s[:], ecols)
            zcol = consts.tile([128, 1], f32)
            nc.vector.memset(zcol[:], 0.0)
            ep128 = consts.tile([1, 1], f32)
            nc.vector.memset(ep128[:], 128.0 * EPS)
            epsk = consts.tile([128, 1], f32)
            nc.vector.memset(epsk[:], 128.0 * EPS)
            epsq = consts.tile([1, 1], f32)
            nc.vector.memset(epsq[:], EPS)
            if has_bq:
                bq_s = consts.tile([128, 8], f32)
                nc.sync.dma_start(bq_s[:], bq_t)
            if has_bk:
                bk_s = consts.tile([128, 8], f32)
                nc.sync.dma_start(bk_s[:], bk_t)
            if has_bo:
                bo_s = consts.tile([128, 8], f32)
                nc.sync.dma_start(bo_s[:], bo_t)
            if has_bv:
                bv_s = consts.tile([128, DIM], f32)
                nc.sync.dma_start(bv_s[:], bv_t)


            # ============ phase A: K-proj + rope per 512-key block ============
            with tc.tile_pool(name="kpool", bufs=2) as kpool, \
                 tc.tile_pool(name="pp_kv", bufs=2, space="PSUM") as pp_kv, \
                 tc.tile_pool(name="pp_sw", bufs=2, space="PSUM") as pp_sw, \
                 tc.tile_pool(name="pp_rs", bufs=2, space="PSUM") as pp_rs:
                ctk_s = kpool.tile([128, pc], bf16, tag="ctk", bufs=1)
                nc.sync.dma_start(ctk_s[:], ctk)
                stk_s = kpool.tile([128, pc], bf16, tag="stk", bufs=1)
                nc.sync.dma_start(stk_s[:], stk)
                nc.sync.dma_start(memt_blks[1][:],
                                  memT[:, 512:1024].rearrange(
                                      "(i p) t -> p i t", p=128))
                nc.sync.dma_start(wv_s[:], wv)
                nc.sync.dma_start(xt_s[:],
                                  xT.rearrange("(i p) q -> p i q", p=128))
                nc.sync.dma_start(wqo_s[:], wq)
                for b in range(nblk):
                    c0 = b * 512
                    memt = memt_blks[b % 2]
                    if b >= 2:
                        nc.sync.dma_start(
                            memt[:],
                            memT[:, c0:c0 + 512].rearrange("(i p) t -> p i t",
                                                           p=128))
                    yk = kpool.tile([128, 8, 512], bf16, tag="yk")
                    sw = kpool.tile([128, 8, 512], bf16, tag="sw")
                    if pair:
                        ps_rs = pp_rs.tile([1, 512], f32, tag="psrs",
                                           name="ps_rs")
                    else:
                        ps_rs = pp_rs.tile([128, 4], f32, tag="psrs",
                                           name="ps_rs")
                    for o in range(8):
                        ps_y = pp_kv.tile([128, 512], f32, tag="ps")
                        for i in range(8):
                            wsl = (wk0_s[:, 0, i, :] if o == 0
                                   else wkr_s[:, o - 1, i, :])
                            nc.tensor.matmul(ps_y[:], wsl,
                                             memt[:, i, :],
                                             start=(i == 0), stop=(i == 7))
                        if has_bk:
                            nc.scalar.activation(yk[:, o, :], ps_y[:],
                                                 AF.Identity,
                                                 bias=bk_s[:, o:o + 1])
                        else:
                            nc.scalar.activation(yk[:, o, :], ps_y[:], AF.Copy)
                        ps_sw = pp_sw.tile([128, 512], f32, tag="ps")
                        nc.tensor.matmul(ps_sw[:], pt_s[:], yk[:, o, :])
                        nc.scalar.activation(sw[:, o, :], ps_sw[:], AF.Copy)
                        ysq = kpool.tile([128, 512], bf16, tag="ysqo", bufs=2)
                        nc.vector.tensor_mul(ysq[:], yk[:, o, :], yk[:, o, :])
                        if pair:
                            nc.tensor.matmul(ps_rs[:], ones_s[:], ysq[:],
                                             start=(o == 0), stop=(o == 7))
                        else:
                            for tt in range(4):
                                nc.tensor.matmul(
                                    ps_rs[:, tt:tt + 1],
                                    ysq[:, tt * 128:(tt + 1) * 128], ones_s[:],
                                    start=(o == 0), stop=(o == 7))
                    # rs = (1/sqrt(HD))/sqrt(ms+eps) = 1/sqrt(sum/8+128eps)
                    if pair:
                        rroot = kpool.tile([1, 512], f32, tag="rroot")
                        nc.scalar.activation(rroot[:], ps_rs[:], AF.Sqrt,
                                             bias=ep128[:], scale=0.125)
                        rr = kpool.tile([1, 512], f32, tag="rr")
                        nc.vector.reciprocal(rr[:], rroot[:])
                        rsb = kpool.tile([128, 512], f32, tag="rsb")
                        nc.gpsimd.partition_broadcast(rsb[:], rr[:])
                        cts = kpool.tile([128, 512], bf16, tag="cts")
                        nc.vector.tensor_mul(cts[:], ctk_s[:, c0:c0 + 512],
                                             rsb[:])
                        sts = kpool.tile([128, 512], bf16, tag="sts")
                        nc.vector.tensor_mul(sts[:], stk_s[:, c0:c0 + 512],
                                             rsb[:])
                    else:
                        rroot = kpool.tile([128, 4], f32, tag="rroot")
                        nc.scalar.activation(rroot[:], ps_rs[:], AF.Sqrt,
                                             bias=epsk[:], scale=0.125)
                        nc.vector.reciprocal(rs_s[:, b * 4:b * 4 + 4],
                                             rroot[:])
                        cts = ctk_s[:, c0:c0 + 512]
                        sts = stk_s[:, c0:c0 + 512]
                    for o in range(8):
                        t1 = kpool.tile([128, 512], bf16, tag="t1")
                        nc.vector.tensor_mul(t1[:], yk[:, o, :], cts[:])
                        t2 = kpool.tile([128, 512], bf16, tag="t2")
                        nc.vector.tensor_mul(t2[:], sw[:, o, :], sts[:])
                        nc.vector.tensor_add(kr[:, o, c0:c0 + 512], t1[:], t2[:])
                    if b >= 2:
                        # refill this ring slot for phase B (blocks 0/1)
                        c2 = (b - 2) * 512
                        nc.sync.dma_start(
                            memt[:],
                            memT[:, c2:c2 + 512].rearrange("(i p) t -> p i t",
                                                           p=128))

            # ========= phase Q + B (shared scope: no drain between) =========
            with tc.tile_pool(name="qlate", bufs=1) as qlate, \
                 tc.tile_pool(name="bpool", bufs=2) as bpool, \
                 tc.tile_pool(name="ptsp", bufs=5) as ptsp, \
                 tc.tile_pool(name="pp_v", bufs=3, space="PSUM") as pp_v, \
                 tc.tile_pool(name="pp_sc", bufs=2, space="PSUM") as pp_sc, \
                 tc.tile_pool(name="pp_qrs", bufs=1, space="PSUM") as pp_qrs:
                qtab_cm = tc.tile_pool(name="qtab", bufs=1)
                qtab = qtab_cm.__enter__()
                ctq_s = qtab.tile([128, 8, SQ], bf16)
                nc.sync.dma_start(ctq_s[:], ctq)
                stq_s = qtab.tile([128, 8, SQ], bf16)
                nc.sync.dma_start(stq_s[:], stq)
                yq = qlate.tile([128, 8, SQ], bf16)
                swq = qlate.tile([128, 8, SQ], bf16)

                qden = pp_qrs.tile([8, SQ], f32)
                ps_rsq = qden[0:1, :]
                ps_den = qden
                for o in range(8):
                    ps2q = pp_sc.tile([128, 2, 512], f32, tag="ps2")
                    for i in range(8):
                        nc.tensor.matmul(ps2q[:, 0, :], wqo_s[:, o, i, :],
                                         xt_s[:, i, :],
                                         start=(i == 0), stop=(i == 7))
                    if has_bq:
                        nc.scalar.activation(yq[:, o, :], ps2q[:, 0, :],
                                             AF.Identity,
                                             bias=bq_s[:, o:o + 1])
                    else:
                        nc.scalar.activation(yq[:, o, :], ps2q[:, 0, :],
                                             AF.Copy)
                    nc.tensor.matmul(ps2q[:, 1, :], pt_s[:], yq[:, o, :])
                    nc.scalar.activation(swq[:, o, :], ps2q[:, 1, :], AF.Copy)
                    ysq = qlate.tile([128, SQ], bf16, tag="ysqq", bufs=2)
                    nc.vector.tensor_mul(ysq[:], yq[:, o, :], yq[:, o, :])
                    nc.tensor.matmul(ps_rsq[:], ones_s[:], ysq[:],
                                     start=(o == 0), stop=(o == 7))
                # V-proj helpers (phase B work)
                def v_chain(b, vt, tt, oh):
                    memt = memt_blks[b % 2]
                    ps_v = pp_v.tile([128, 512], f32, tag="ps", name="ps_v")
                    for i in range(8):
                        nc.tensor.matmul(
                            ps_v[:], memt[:, i, tt * 128:(tt + 1) * 128],
                            wv_s[:, i, oh * 512:(oh + 1) * 512],
                            start=(i == 0), stop=(i == 7))
                    if has_bv:
                        nc.vector.tensor_add(
                            vt[:, tt, oh * 512:(oh + 1) * 512], ps_v[:],
                            bv_s[:, oh * 512:(oh + 1) * 512])
                    elif (tt + oh) % 2 == 0:
                        nc.scalar.activation(
                            vt[:, tt, oh * 512:(oh + 1) * 512], ps_v[:],
                            AF.Copy)
                    else:
                        nc.vector.tensor_copy(
                            vt[:, tt, oh * 512:(oh + 1) * 512], ps_v[:])

                def v_proj(b, vt):
                    for tt in range(4):
                        for oh in range(2):
                            v_chain(b, vt, tt, oh)

                vts = [None] * nblk
                for b in range(2):
                    vts[b] = bpool.tile([128, 4, DIM], bf16, tag="vt",
                                        name=f"vt{b}", bufs=3)
                    v_proj(b, vts[b])
                    if b + 2 < nblk:
                        c2 = (b + 2) * 512
                        nc.sync.dma_start(
                            memt_blks[b % 2][:],
                            memT[:, c2:c2 + 512].rearrange("(i p) t -> p i t",
                                                           p=128))
                for b in range(2, nblk):
                    vts[b] = bpool.tile([128, 4, DIM], bf16, tag="vt",
                                        name=f"vt{b}", bufs=3)
                # finish Q: rmsnorm scale + rope (wide in-place DVE ops)
                rrootq = qlate.tile([1, SQ], f32)
                nc.scalar.activation(rrootq[:], ps_rsq[:], AF.Sqrt,
                                     bias=epsq[:], scale=1.0 / DIM)
                nc.vector.reciprocal(rrootq[:], rrootq[:])
                rsbf = qlate.tile([128, SQ], f32)
                nc.gpsimd.partition_broadcast(rsbf[:], rrootq[:])
                nc.vector.tensor_mul(yq[:], yq[:], ctq_s[:])
                nc.vector.tensor_mul(swq[:], swq[:], stq_s[:])
                nc.vector.tensor_add(yq[:], yq[:], swq[:])
                for o in range(8):
                    nc.vector.tensor_mul(qT[:, o, :], yq[:, o, :], rsbf[:])
                qtab_cm.__exit__(None, None, None)

                # ---- attention blocks (V-proj for b+2 interleaved) ----
                def attn_tail(b, h, pts):
                    vt = vts[b]
                    ps_a = pp_v.tile([128, SQ], f32, tag="ps", name="ps_a")
                    for tt in range(4):
                        nc.tensor.matmul(
                            ps_a[:], vt[:, tt, h * 128:(h + 1) * 128],
                            pts[:, tt, :], start=(tt == 0), stop=(tt == 3))
                    if b == 0:
                        nc.vector.tensor_copy(nsum[:, h, :], ps_a[:])
                    else:
                        nc.vector.tensor_add(nsum[:, h, :], nsum[:, h, :],
                                             ps_a[:])
                    # exp-sum partial for this block (bf16 tree)
                    nc.vector.tensor_add(pts[:, 0:2, :], pts[:, 0:2, :],
                                         pts[:, 2:4, :])
                    if b == 0:
                        nc.vector.tensor_add(dsums[:, h, :], pts[:, 0, :],
                                             pts[:, 1, :])
                    else:
                        nc.vector.tensor_add(pts[:, 0, :], pts[:, 0, :],
                                             pts[:, 1, :])
                        nc.vector.tensor_add(dsums[:, h, :], dsums[:, h, :],
                                             pts[:, 0, :])
                    if b == nblk - 1:
                        nc.tensor.matmul(ps_den[:], ec_s[:, h, :],
                                         dsums[:, h, :],
                                         start=(h == 0), stop=(h == 7))
                    if b == nblk - 1 and h in (3, 5):
                        h0, h1 = (0, 4) if h == 3 else (4, 6)
                        nc.sync.dma_start(
                            cat[h0 * 128:h1 * 128, :].rearrange(
                                "(h p) q -> p h q", p=128),
                            nsum[:, h0:h1, :])
                        if _sim:
                            nc.gpsimd.dma_start(
                                cat_sh[h0 * 128:h1 * 128, :],
                                cat[h0 * 128:h1 * 128, :])

                pend = []
                for b in range(nblk):
                    for h in range(8):
                        pts = ptsp.tile([128, 4, SQ], bf16, tag="pts")
                        for tp in range(2):
                            ps2 = pp_sc.tile([128, 2, 512], f32, tag="ps2")
                            for half in range(2):
                                lt = 2 * tp + half
                                gt = b * 4 + lt
                                nc.tensor.matmul(
                                    ps2[:, half, :],
                                    kr[:, h, gt * 128:(gt + 1) * 128],
                                    qT[:, h, :])
                            if not pair:
                                for half in range(2):
                                    lt = 2 * tp + half
                                    gt = b * 4 + lt
                                    bias_ap = (mb_s[:, lt:lt + 1]
                                               if b == nblk - 1 else zcol[:])
                                    nc.scalar.activation(
                                        pts[:, lt, :], ps2[:, half, :],
                                        AF.Exp, bias=bias_ap,
                                        scale=rs_s[:, gt:gt + 1])
                            elif b == nblk - 1:
                                for half in range(2):
                                    lt = 2 * tp + half
                                    nc.scalar.activation(
                                        pts[:, lt, :], ps2[:, half, :],
                                        AF.Exp, bias=mb_s[:, lt:lt + 1])
                            else:
                                nc.scalar.activation(
                                    pts[:, 2 * tp:2 * tp + 2, :], ps2[:],
                                    AF.Exp, bias=zcol[:])
                        pend.append((b, h, pts))
                        if len(pend) > 3:
                            attn_tail(*pend.pop(0))
                    if b + 2 < nblk:
                        v_proj(b + 2, vts[b + 2])
                for p_ in pend:
                    attn_tail(*p_)
                nc.sync.dma_start(
                    cat[768:DIM, :].rearrange("(h p) q -> p h q", p=128),
                    nsum[:, 6:8, :])
                # wo into the wq buffer for the tail
                nc.sync.dma_start(wqo_s[:], wo)

            with tc.tile_pool(name="dpool", bufs=1) as dpool:
                dcp = dpool.tile([8, SQ], bf16)
                nc.vector.tensor_copy(dcp[:], ps_den[:])
                nc.sync.dma_start(cat[DIM:DIM + HEADS, :], dcp[:])

            if _sim:
                nc.gpsimd.dma_start(cat_sh[768:DIM + HEADS, :],
                                    cat[768:DIM + HEADS, :])
            else:
                nc.gpsimd.collective_compute(
                    "AllReduce", mybir.AluOpType.add,
                    replica_groups=[list(range(N_CORES))],
                    ins=[cat[:]], outs=[cat_sh[:]])

            # ========== per-core output projection on its query slice ==========
            with tc.tile_pool(name="tail", bufs=1) as tail, \
                 tc.tile_pool(name="pp_t", bufs=2, space="PSUM") as pp_t:
                nred = tail.tile([128, 8, QS], bf16)
                dred = tail.tile([1, 8, QS], bf16)
                pid = nc.sync.partition_id()
                qoff = pid * QS
                nc.sync.dma_start(
                    nred[:],
                    cat_sh[0:DIM, bass.ds(qoff, QS)].rearrange(
                        "(h p) q -> p h q", p=128))
                nc.sync.dma_start(
                    dred[:],
                    cat_sh[DIM:DIM + HEADS, bass.ds(qoff, QS)].rearrange(
                        "(o h) q -> o h q", o=1))
                rd = tail.tile([1, 8 * QS], f32)
                nc.vector.reciprocal(rd[:], dred.rearrange("o h q -> o (h q)")[:])
                ps_rd = pp_t.tile([128, 8 * QS], f32, tag="psrd")
                nc.tensor.matmul(ps_rd[:], ones_rs[:], rd[:])
                nsc = tail.tile([128, 8, QS], bf16)
                nc.vector.tensor_mul(nsc.rearrange("p h q -> p (h q)")[:],
                                     nred.rearrange("p h q -> p (h q)")[:],
                                     ps_rd[:])
                out_sb = tail.tile([128, 8, QS], f32)
                for e in range(8):
                    ps_o = pp_t.tile([128, QS], f32, tag="ps")
                    for o in range(8):
                        nc.tensor.matmul(ps_o[:], wqo_s[:, e, o, :],
                                         nsc[:, o, :],
                                         start=(o == 0), stop=(o == 7))
                    if has_bo:
                        nc.scalar.activation(out_sb[:, e, :], ps_o[:],
                                             AF.Identity, bias=bo_s[:, e:e + 1])
                    else:
                        nc.scalar.activation(out_sb[:, e, :], ps_o[:], AF.Copy)
                nc.sync.dma_start(
                    outT.rearrange("(e p) q -> p e q", p=128)[:, :, 0:QS],
                    out_sb[:])

    nc.compile()
    _cache[key] = nc
    return nc


def _prep(x, mem, mask, cos_q, sin_q, cos_k, sin_k,
          Wq, bq, Wk, bk, Wv, bv, Wo, bo, gq, gk):
    import ml_dtypes
    f = np.float32
    bf = ml_dtypes.bfloat16
    x = np.asarray(x, f).reshape(SQ, DIM)
    mem = np.asarray(mem, f).reshape(SK, DIM)
    mask = np.asarray(mask).reshape(SK)
    cos_q = np.asarray(cos_q, f)
    sin_q = np.asarray(sin_q, f)
    cos_k = np.asarray(cos_k, f)
    sin_k = np.asarray(sin_k, f)
    Wq, Wk, Wv, Wo = (np.asarray(w, f) for w in (Wq, Wk, Wv, Wo))
    bq, bk, bv, bo, gq, gk = (np.asarray(v, f) for v in (bq, bk, bv, bo, gq, gk))

    if not np.allclose(gk, 1.0):
        gkp = gk.reshape(-1, 2)
        assert np.allclose(gkp[:, 0], gkp[:, 1]), "unsupported non-pairwise gk"

    flags = tuple(bool(np.any(v != 0.0)) for v in (bq, bk, bv, bo))

    # compact keys: keep only unmasked, shard evenly, pad per-core to 512
    idx = np.flatnonzero(mask)
    keep = len(idx)
    percore = -(-keep // N_CORES)                # ceil
    pc = max(512, -(-percore // 512) * 512)      # pad to multiple of 512
    nt = pc // 128

    def tile_w(WT):  # [1024,1024] (in,out of W.T) -> [p, o, i, m]
        return np.ascontiguousarray(
            WT.reshape(8, 128, 8, 128).transpose(1, 2, 0, 3)).astype(bf)

    ii = np.arange(128)
    jj = ii // 2
    partner = ii ^ 1

    # fold gq (and pairwise gk) into the q rope tables; sin pairs with
    # partner's gq
    gq_t = (gq * gk).reshape(8, 128)
    gq_sin = (gq.reshape(8, 128)[:, partner] * gk.reshape(8, 128))
    cq = cos_q[:, jj].T                # [128, SQ]
    sq = sin_q[:, jj].T
    ctq = np.ascontiguousarray(
        (cq[None, :, :] * gq_t[:, :, None]).transpose(1, 0, 2)).astype(bf)
    stq = np.ascontiguousarray(
        (sq[None, :, :] * gq_sin[:, :, None]).transpose(1, 0, 2)).astype(bf)

    PT = np.zeros((128, 128), f)
    even = ii[ii % 2 == 0]
    PT[even + 1, even] = -1.0
    PT[even, even + 1] = 1.0

    ec = np.zeros((128, 8, 8), f)
    for h in range(8):
        ec[:, h, h] = 1.0

    shared = {
        "xT": np.ascontiguousarray(x.T).astype(bf),
        "wq": tile_w(Wq.T), "wk": tile_w(Wk.T), "wo": tile_w(Wo.T),
        "wv": np.ascontiguousarray(
            Wv.T.reshape(8, 128, DIM).transpose(1, 0, 2)).astype(bf),
        "ctq": ctq, "stq": stq,
        "bq_t": np.ascontiguousarray(bq.reshape(8, 128).T),
        "bk_t": np.ascontiguousarray(bk.reshape(8, 128).T),
        "bo_t": np.ascontiguousarray(bo.reshape(8, 128).T),
        "bv_t": np.ascontiguousarray(np.tile(bv, (128, 1))),
        "pmat": PT.astype(bf),
        "ones_c": np.ones((128, 1), bf),
        "ones_r": np.ones((1, 128), f),
        "ecols": ec.astype(bf),
    }

    # global compacted + padded arrays
    tot = N_CORES * pc
    memT_full = np.zeros((DIM, tot), bf)
    ctk_full = np.zeros((HD, tot), bf)
    stk_full = np.zeros((HD, tot), bf)
    mb_full = np.full(tot, NEG, f)

    ck = cos_k[:, jj].T.astype(f)   # [128, SK]
    sk_t = sin_k[:, jj].T.astype(f)
    counts = [keep // N_CORES + (1 if c < keep % N_CORES else 0)
              for c in range(N_CORES)]
    off = 0
    for c in range(N_CORES):
        sl = idx[off:off + counts[c]]
        off += counts[c]
        d0 = c * pc
        memT_full[:, d0:d0 + len(sl)] = mem[sl].T.astype(bf)
        ctk_full[:, d0:d0 + len(sl)] = ck[:, sl].astype(bf)
        stk_full[:, d0:d0 + len(sl)] = sk_t[:, sl].astype(bf)
        mb_full[d0:d0 + len(sl)] = 0.0
        assert pc - len(sl) < 512, "padding must fit in last 4 tiles"

    in_maps = []
    for c in range(N_CORES):
        s = slice(c * pc, (c + 1) * pc)
        m = dict(shared)
        m["memT"] = np.ascontiguousarray(memT_full[:, s])
        m["ctk"] = np.ascontiguousarray(ctk_full[:, s])
        m["stk"] = np.ascontiguousarray(stk_full[:, s])
        # bias columns for the last 4 tiles only
        mb = mb_full[s][-512:].reshape(4, 128).T
        m["mbias"] = np.ascontiguousarray(mb)
        in_maps.append(m)
    return in_maps, nt, flags


def kernel(**inputs):
    from concourse.bass_utils import run_bass_kernel_spmd
    in_maps, nt, flags = _prep(**inputs)
    nc = _build(nt, flags)
    res = run_bass_kernel_spmd(nc, in_maps, list(range(N_CORES)))
    parts = [res.results[c]["outT"][:, 0:QS].T for c in range(N_CORES)]
    out = np.concatenate(parts, axis=0)
    return out[None].astype(np.float32)
